# revision 8
# baseline (speedup 1.0000x reference)
"""Trainium2 Bass kernel for nn_BambaMixerDecoderLayer_84696755077458.

Tensor-parallel over 8 NeuronCores (vLLM-style), v2 (bf16):
  - in_proj / gate_up column-sharded, out_proj / down row-sharded
  - heads + conv channels sharded with d_inner; B/C conv streams replicated
  - SSM scan via chunked SSD (Mamba2): intra-chunk matmuls + small
    cross-chunk state recurrence.
  - bf16 weights/activations for all large GEMMs, scratch and collectives;
    fp32 for stats, decay rows and the SSD state.
  - Single merged in_proj pass; MLP (gate_up+down) fused in one pass.
  - Collectives chunked 8x along tokens and issued inline so they overlap
    with compute (no global barrier between SSD and MLP phases).
Everything on-device is feature-major ([feature, token]); host does layout
transforms (transpose / shard / concat) only.

Self-contained: hardcodes all shapes; needs only /opt/trn_rl_repo on sys.path.
"""
import sys
from contextlib import ExitStack

if '/opt/trn_rl_repo' not in sys.path:
    sys.path.insert(0, '/opt/trn_rl_repo')

import numpy as np

# ---------------------------------------------------------------- constants
H = 2048          # hidden
DIN = 4096        # mamba intermediate
DS = 128          # ssm state
DCONV = 4
NH = 64
HD = 64
FF = 8192
EPS = 1e-5
B, L = 2, 2048
T = B * L                         # 4096 tokens
CONV_DIM = DIN + 2 * DS           # 4352
D_IN_PROJ = 2 * DIN + 2 * DS + NH  # 8512

TP = 8
NHr = NH // TP                    # 8 heads / core
DINr = DIN // TP                  # 512
FFr = FF // TP                    # 1024
CONVr = DINr + 2 * DS             # 768 conv channels / core
MPROJ = DINr + CONVr + NHr        # 1288 in_proj cols / core

Q = 128                           # SSD chunk
NT = 512                          # token tile (also the collective chunk)
NEG = -3.0e38
SIM_SILU = False   # True: emit sigmoid+mul instead of Silu (CoreSim support)


def _f32(x):
    return np.ascontiguousarray(np.asarray(x, dtype=np.float32))


def _bf16(x):
    import ml_dtypes
    return np.ascontiguousarray(
        np.asarray(x, dtype=np.float32).astype(ml_dtypes.bfloat16))


# ================================================================ host prep
def host_constants():
    import ml_dtypes
    identb = np.eye(128, dtype=ml_dtypes.bfloat16)
    i8 = np.eye(8, dtype=np.float32)
    sel8 = np.zeros((8, 8 * 128), np.float32)
    for h in range(8):
        sel8[h, h * 128:(h + 1) * 128] = 1.0
    negselpair = np.zeros((8, 4 * 256), np.float32)
    for p in range(4):
        negselpair[2 * p, p * 256:p * 256 + 128] = -1.0
        negselpair[2 * p + 1, p * 256 + 128:p * 256 + 256] = -1.0
    ones8 = np.ones((8, 128), np.float32)
    ones1 = np.ones((1, 128), np.float32)
    ones128 = np.ones((128, 1), np.float32)
    tri = np.where(np.arange(Q)[:, None] > np.arange(Q)[None, :], NEG, 0.0)
    trimask2 = np.concatenate([tri, tri], axis=1).astype(np.float32)
    return dict(c_identb=identb, c_i8=i8, c_sel8=sel8, c_negselpair=negselpair,
                c_ones8=ones8, c_ones1=ones1, c_ones128=ones128,
                c_trimask2=trimask2)


def shard_core_inputs(inputs, r):
    """Build the per-core input map (all feature-major)."""
    w_in = _f32(inputs['w_in'])
    zs = slice(DINr * r, DINr * (r + 1))
    xs = slice(DIN + DINr * r, DIN + DINr * (r + 1))
    bs = slice(2 * DIN, 2 * DIN + DS)
    cs = slice(2 * DIN + DS, 2 * DIN + 2 * DS)
    dts = slice(2 * DIN + 2 * DS + NHr * r, 2 * DIN + 2 * DS + NHr * (r + 1))
    w_in_r = np.concatenate(
        [w_in[:, zs], w_in[:, xs], w_in[:, bs], w_in[:, cs], w_in[:, dts]], axis=1)

    conv_w = _f32(inputs['conv_w'])
    conv_w_r = np.concatenate([conv_w[DINr * r:DINr * (r + 1)], conv_w[DIN:]], axis=0)
    conv_b = _f32(inputs['conv_b'])
    conv_b_r = np.concatenate([conv_b[DINr * r:DINr * (r + 1)], conv_b[DIN:]], axis=0)

    hs = _f32(inputs['hidden_states'])
    hs = hs.reshape(-1, H)

    A_r = _f32(inputs['A_log'])[NHr * r:NHr * (r + 1)]
    dtb_r = _f32(inputs['dt_bias'])[NHr * r:NHr * (r + 1)]
    D_r = _f32(inputs['D_ssm'])[NHr * r:NHr * (r + 1)]

    m = dict(host_constants())
    m['hsT'] = _bf16(hs.T)                                      # [2048, T]
    m['w_in'] = _bf16(w_in_r)                                   # [2048, 1288]
    # per-k-tile column form of per-feature vectors: [128, n_tiles]
    m['ln1_c'] = np.ascontiguousarray(_f32(inputs['ln1_w']).reshape(16, 128).T)
    m['ln2_c'] = np.ascontiguousarray(_f32(inputs['ln2_w']).reshape(16, 128).T)
    m['normw_c'] = np.ascontiguousarray(
        _f32(inputs['norm_w'])[DINr * r:DINr * (r + 1)].reshape(4, 128).T)
    m['dssm_c'] = np.ascontiguousarray(
        np.repeat(D_r, HD).reshape(4, 128).T)                   # [128, 4]
    # conv weights: [128, 6*4] with [p, pt*4+d]
    m['conv_w'] = np.ascontiguousarray(
        conv_w_r.reshape(6, 128, DCONV).transpose(1, 0, 2).reshape(128, 6 * DCONV))
    m['conv_b'] = np.ascontiguousarray(conv_b_r.reshape(6, 128).T)  # [128, 6]
    m['a_col'] = np.ascontiguousarray((-np.exp(A_r))[:, None])   # [8,1]
    m['dtb_col'] = np.ascontiguousarray(dtb_r[:, None])          # [8,1]
    m['w_out'] = _bf16(_f32(inputs['w_out'])[DINr * r:DINr * (r + 1)])
    wgu = _f32(inputs['w_gate_up'])
    m['w_gate'] = _bf16(wgu[:, FFr * r:FFr * (r + 1)])
    m['w_up'] = _bf16(wgu[:, FF + FFr * r:FF + FFr * (r + 1)])
    m['w_down'] = _bf16(_f32(inputs['w_down'])[FFr * r:FFr * (r + 1)])
    return m


# ================================================================ the kernel
def build(world=TP, debug=False, T_=T):
    import concourse.mybir as mybir
    import concourse.tile as tile
    from concourse import bacc
    from concourse.alu_op_type import AluOpType as Op

    AF = mybir.ActivationFunctionType
    F32 = mybir.dt.float32
    BF16 = mybir.dt.bfloat16

    nc = bacc.Bacc("TRN2", target_bir_lowering=False, debug=False,
                   num_devices=world)

    F32R = mybir.dt.float32r
    n8 = T_ // NT

    def din(name, shape, dt):
        return nc.dram_tensor(name, list(shape), dt, kind="ExternalInput").ap()

    BIN = {'hsT', 'w_in', 'w_out', 'w_gate', 'w_up', 'w_down', 'c_identb'}
    RIN = {'c_i8', 'c_sel8', 'c_negselpair', 'c_ones8', 'c_ones1', 'c_ones128'}
    io = {}
    for name, shape in [
            ('hsT', (H, T_)), ('w_in', (H, MPROJ)),
            ('ln1_c', (128, 16)), ('ln2_c', (128, 16)),
            ('normw_c', (128, 4)), ('dssm_c', (128, 4)),
            ('conv_w', (128, 24)), ('conv_b', (128, 6)),
            ('a_col', (8, 1)), ('dtb_col', (8, 1)),
            ('w_out', (DINr, H)), ('w_gate', (H, FFr)), ('w_up', (H, FFr)),
            ('w_down', (FFr, H)),
            ('c_identb', (128, 128)), ('c_i8', (8, 8)), ('c_sel8', (8, 1024)),
            ('c_negselpair', (8, 1024)), ('c_ones8', (8, 128)),
            ('c_ones1', (1, 128)), ('c_ones128', (128, 1)),
            ('c_trimask2', (128, 256))]:
        dt = BF16 if name in BIN else (F32R if name in RIN else F32)
        io[name] = din(name, shape, dt)

    io['out1T'] = nc.dram_tensor("out1T", [H // world, T_], BF16,
                                 kind="ExternalOutput").ap()
    io['resid2T'] = nc.dram_tensor("resid2T", [H, T_], BF16,
                                   kind="ExternalOutput").ap()

    skind = "ExternalOutput" if debug else "Internal"
    scr = {}
    scr['z'] = nc.dram_tensor("z_s", [DINr, T_], BF16, kind=skind).ap()
    scr['x'] = nc.dram_tensor("x_s", [DINr, T_], BF16, kind=skind).ap()
    scr['b'] = nc.dram_tensor("b_s", [DS, T_], BF16, kind=skind).ap()
    scr['c'] = nc.dram_tensor("c_s", [DS, T_], BF16, kind=skind).ap()
    scr['ar1_in8'] = [
        nc.dram_tensor(f"ar1_in{q}", [H, NT], BF16, kind="Internal").ap()
        for q in range(n8)]
    scr['ar1_out8'] = [
        nc.dram_tensor(f"ar1_out{q}", [H, NT], BF16, kind="Internal",
                       addr_space="Shared").ap() for q in range(n8)]
    scr['ssq_in8'] = [
        nc.dram_tensor(f"ssq_in{q}", [1, NT], F32, kind="Internal").ap()
        for q in range(n8)]
    scr['ssq_out8'] = [
        nc.dram_tensor(f"ssq_out{q}", [1, NT], F32, kind="Internal",
                       addr_space="Shared").ap() for q in range(n8)]
    scr['rs2_in8'] = [
        nc.dram_tensor(f"rs2_in{q}", [H, NT], BF16, kind="Internal").ap()
        for q in range(n8)]
    scr['rs2_out8'] = [
        nc.dram_tensor(f"rs2_out{q}", [H // world, NT], BF16,
                       kind="Internal").ap() for q in range(n8)]

    with tile.TileContext(nc) as tc:
        _body(tc, io, scr, world, debug, mybir, tile, Op, AF, F32, T_)

    nc.compile()
    return nc


def _body(tc, io, scr, world, debug, mybir, tile, Op, AF, F32, T_):
    nc = tc.nc
    F32R = mybir.dt.float32r
    BF16 = mybir.dt.bfloat16
    n8 = T_ // NT
    NCHUNK = T_ // Q
    CPS = (T_ // B) // Q          # chunks per sequence

    def mm(out, lhsT, rhs, start, stop, skip=False):
        if lhsT.dtype == F32:
            lhsT = lhsT.bitcast(F32R)
        if rhs.dtype == F32:
            rhs = rhs.bitcast(F32R)
        nc.tensor.matmul(out, lhsT, rhs, start=start, stop=stop,
                         skip_group_check=skip)

    def silu(out_ap, in_ap, bias=0.0, pool=None, tag="silu_tmp"):
        if SIM_SILU:
            tmp = pool.tile(list(out_ap.shape), F32, tag=tag, name=tag)
            nc.scalar.activation(tmp[:], in_ap, AF.Sigmoid, bias=bias, scale=1.0)
            if isinstance(bias, float) and bias == 0.0:
                nc.vector.tensor_tensor(out_ap, in_ap, tmp[:], Op.mult)
            else:
                raise NotImplementedError("SIM_SILU with bias AP")
        else:
            nc.scalar.activation(out_ap, in_ap, AF.Silu, bias=bias, scale=1.0)

    def allreduce(in_ap, out_ap):
        if world > 1:
            nc.gpsimd.collective_compute(
                "AllReduce", Op.add, replica_groups=[list(range(world))],
                ins=[in_ap], outs=[out_ap])
        else:
            nc.sync.dma_start(out_ap, in_ap)

    with ExitStack() as ES:
        cpool = ES.enter_context(tc.tile_pool(name="consts", bufs=1))

        # -------------------------------------------------------- constants
        C = {}
        RT = {'c_i8', 'c_sel8', 'c_negselpair', 'c_ones8', 'c_ones1',
              'c_ones128'}
        for nm, shape in [('c_identb', (128, 128)), ('c_i8', (8, 8)),
                          ('c_sel8', (8, 1024)), ('c_negselpair', (8, 1024)),
                          ('c_ones8', (8, 128)), ('c_ones1', (1, 128)),
                          ('c_ones128', (128, 1)), ('c_trimask2', (128, 256)),
                          ('ln1_c', (128, 16)), ('ln2_c', (128, 16)),
                          ('normw_c', (128, 4)), ('dssm_c', (128, 4)),
                          ('conv_w', (128, 24)), ('conv_b', (128, 6)),
                          ('a_col', (8, 1)), ('dtb_col', (8, 1))]:
            dt = BF16 if nm == 'c_identb' else (F32R if nm in RT else F32)
            t = cpool.tile(list(shape), dt, tag=nm)
            nc.sync.dma_start(t[:], io[nm])
            C[nm] = t
        identb, i8 = C['c_identb'], C['c_i8']
        sel8, negselp = C['c_sel8'], C['c_negselpair']
        ones8, ones1, ones128 = C['c_ones8'], C['c_ones1'], C['c_ones128']
        trimask2 = C['c_trimask2']

        eps1 = cpool.tile([1, 1], F32, tag="eps1", name="eps1")
        nc.vector.memset(eps1[:], float(EPS))

        # ======================================================== Phase 1
        # merged single pass over hsT: ln1 stats + z + dt + xBC + conv
        rows_a_es = ExitStack()
        rows_a = rows_a_es.enter_context(tc.tile_pool(name="rows_a", bufs=1))
        dt_rows = rows_a.tile([8, T_], F32R, tag="dt_rows", name="dt_rows")
        lA_rows = rows_a.tile([8, T_], F32R, tag="lA_rows", name="lA_rows")
        ssq_yz_row = rows_a.tile([1, T_], F32, tag="ssq_yz", name="ssq_yz")

        with tc.tile_pool(name="p1w", bufs=1) as p1w, \
             tc.tile_pool(name="p1", bufs=2) as p1, \
             tc.tile_pool(name="convp", bufs=2) as convp, \
             tc.tile_pool(name="p1ps_s", bufs=1, space="PSUM") as p1ps_s, \
             tc.tile_pool(name="p1ps_m", bufs=2, space="PSUM") as p1ps_m:

            # all in_proj columns per core: [z | xBC | dt] = 1288
            w1 = p1w.tile([128, 16, MPROJ], BF16, tag="w1", name="w1")
            nc.sync.dma_start(
                w1[:], io['w_in'].rearrange("(kt p) m -> p kt m", p=128))
            for k in range(16):
                nc.vector.tensor_scalar_mul(w1[:, k, :], w1[:, k, :],
                                            C['ln1_c'][:, k:k + 1])

            halo_prev = None
            for nt in range(n8):
                tok0 = nt * NT
                seq_start = (tok0 % (T_ // B)) == 0
                hst = p1.tile([128, 16, NT], BF16, tag="hst", name="hst")
                nc.sync.dma_start(hst[:], io['hsT'][:, tok0:tok0 + NT]
                                  .rearrange("(kt p) n -> p kt n", p=128))
                # ln1 stats (ACT squares; matmuls never wait on these)
                ssq_ps = p1ps_s.tile([1, NT], F32, tag="ssq", name="ssq")
                for k in range(16):
                    sq = p1.tile([128, NT], F32R, tag="sq", name="sq")
                    nc.scalar.activation(sq[:], hst[:, k, :], AF.Square)
                    mm(ssq_ps[:], ones128[:], sq[:],
                       start=(k == 0), stop=(k == 15))
                sr0 = p1.tile([1, NT], F32, tag="sr0", name="sr0")
                nc.scalar.activation(sr0[:], ssq_ps[:], AF.Ln,
                                     bias=eps1[:], scale=float(1.0 / H))
                s_row = p1.tile([1, NT], F32R, tag="s_row", name="s_row")
                nc.scalar.activation(s_row[:], sr0[:], AF.Exp, scale=-0.5)
                sb_ps = p1ps_s.tile([128, NT], F32, tag="sbps", name="sbps")
                mm(sb_ps[:], ones1[:], s_row[:], start=True, stop=True)
                sb = p1.tile([128, NT], F32, tag="sb", name="sb")
                nc.any.tensor_copy(sb[:], sb_ps[:])
                # z m-tiles: matmul on RAW hidden, scale on the way out
                for mi in range(4):
                    ps = p1ps_m.tile([128, NT], F32, tag="mt", name="mt")
                    for k in range(16):
                        mm(ps[:], w1[:, k, mi * 128:(mi + 1) * 128],
                           hst[:, k, :], start=(k == 0), stop=(k == 15))
                    zt = p1.tile([128, NT], BF16, tag="z", name="z")
                    nc.vector.tensor_tensor(zt[:], ps[:], sb[:], Op.mult)
                    nc.sync.dma_start(
                        scr['z'][mi * 128:(mi + 1) * 128, tok0:tok0 + NT], zt[:])
                # dt m-tile (8 wide)
                dtp = p1ps_s.tile([8, NT], F32, tag="mtdt", name="mtdt")
                for k in range(16):
                    mm(dtp[:], w1[:, k, DINr + CONVr:MPROJ], hst[:, k, :],
                       start=(k == 0), stop=(k == 15))
                dt_raw = p1.tile([8, NT], F32, tag="dtraw", name="dtraw")
                nc.vector.tensor_tensor(dt_raw[:], dtp[:], sb[:8, :], Op.mult)
                e8 = p1.tile([8, NT], F32, tag="e8", name="e8")
                nc.scalar.activation(e8[:], dt_raw[:], AF.Exp,
                                     bias=C['dtb_col'][:], scale=1.0)
                e8p = p1.tile([8, NT], F32, tag="e8p", name="e8p")
                nc.vector.tensor_scalar_add(e8p[:], e8[:], 1.0)
                nc.scalar.activation(dt_rows[:, tok0:tok0 + NT], e8p[:], AF.Ln)
                logda = p1.tile([8, NT], F32, tag="logda", name="logda")
                nc.vector.tensor_scalar_mul(logda[:], dt_rows[:, tok0:tok0 + NT],
                                            C['a_col'][:])
                for c in range(NT // Q):
                    nc.vector.tensor_tensor_scan(
                        lA_rows[:, tok0 + c * Q:tok0 + (c + 1) * Q],
                        ones8[:, :Q].bitcast(F32), logda[:, c * Q:(c + 1) * Q],
                        0.0, Op.mult, Op.add)
                # xBC m-tiles + causal conv
                halo = [convp.tile([128, NT + 3], BF16, tag=f"halo{pt}",
                                   name=f"halo{pt}") for pt in range(6)]
                for pt in range(6):
                    ps = p1ps_m.tile([128, NT], F32, tag="mt", name="mt")
                    for k in range(16):
                        mm(ps[:], w1[:, k, DINr + pt * 128:DINr + (pt + 1) * 128],
                           hst[:, k, :], start=(k == 0), stop=(k == 15))
                    nc.vector.tensor_tensor(halo[pt][:, 3:3 + NT], ps[:], sb[:],
                                            Op.mult)
                for pt in range(6):
                    if seq_start:
                        nc.vector.memset(halo[pt][:, 0:3], 0.0)
                    else:
                        nc.vector.tensor_copy(halo[pt][:, 0:3],
                                              halo_prev[pt][:, NT:NT + 3])
                    acc = convp.tile([128, NT], BF16, tag="cacc", name="cacc")
                    nc.vector.tensor_scalar_mul(
                        acc[:], halo[pt][:, 0:NT],
                        C['conv_w'][:, pt * 4:pt * 4 + 1])
                    for d in range(1, 4):
                        nc.vector.scalar_tensor_tensor(
                            acc[:], halo[pt][:, d:d + NT],
                            C['conv_w'][:, pt * 4 + d:pt * 4 + d + 1],
                            acc[:], Op.mult, Op.add)
                    cact = convp.tile([128, NT], BF16, tag="cact", name="cact")
                    if SIM_SILU:
                        nc.vector.tensor_scalar_add(acc[:], acc[:],
                                                    C['conv_b'][:, pt:pt + 1])
                        silu(cact[:], acc[:], pool=convp, tag="cvsig")
                    else:
                        nc.scalar.activation(cact[:], acc[:], AF.Silu,
                                             bias=C['conv_b'][:, pt:pt + 1],
                                             scale=1.0)
                    if pt < 4:
                        nc.sync.dma_start(
                            scr['x'][pt * 128:(pt + 1) * 128, tok0:tok0 + NT],
                            cact[:])
                    elif pt == 4:
                        nc.sync.dma_start(scr['b'][:, tok0:tok0 + NT], cact[:])
                    else:
                        nc.sync.dma_start(scr['c'][:, tok0:tok0 + NT], cact[:])
                halo_prev = halo

        # ============================================ Phase 2: SSD + gated
        # norm + out_proj, fused per token-tile. out_proj runs on UNSCALED
        # yz — the rms scale s3 commutes through the matmul and the
        # AllReduce, and is applied in Phase 4. AR chunks issued inline.
        with tc.tile_pool(name="p2", bufs=3) as p2, \
             tc.tile_pool(name="p2s", bufs=2) as p2s, \
             tc.tile_pool(name="state", bufs=1) as spool, \
             tc.tile_pool(name="p3f", bufs=2) as p3f, \
             tc.tile_pool(name="p3w", bufs=1) as p3w, \
             tc.tile_pool(name="p2ps1", bufs=1, space="PSUM") as p2ps1, \
             tc.tile_pool(name="p2ps2", bufs=1, space="PSUM") as p2ps2, \
             tc.tile_pool(name="p3ps", bufs=1, space="PSUM") as p3ps:

            w_out_t = p3w.tile([128, 4, H], BF16, tag="w_out_t", name="w_out_t")
            nc.sync.dma_start(w_out_t[:],
                              io['w_out'].rearrange("(kt p) m -> p kt m", p=128))
            for k in range(4):
                nc.vector.tensor_scalar_mul(w_out_t[:, k, :], w_out_t[:, k, :],
                                            C['normw_c'][:, k:k + 1])

            S_all = spool.tile([128, NHr * HD], F32R, tag="S_all", name="S_all")
            nc.vector.memset(S_all[:].bitcast(F32), 0.0)

            for nt in range(n8):
                y_sb = p3f.tile([128, 4, NT], F32, tag="ysb", name="ysb")
                for cc_ in range(NT // Q):
                    ch = nt * (NT // Q) + cc_
                    t0 = ch * Q
                    xf = p2.tile([128, 4, Q], BF16, tag="xf", name="xf")
                    nc.sync.dma_start(xf[:], scr['x'][:, t0:t0 + Q]
                                      .rearrange("(pt p) n -> p pt n", p=128))
                    bf = p2.tile([128, Q], BF16, tag="bf", name="bf")
                    nc.sync.dma_start(bf[:], scr['b'][:, t0:t0 + Q])
                    cf = p2.tile([128, Q], BF16, tag="cf", name="cf")
                    nc.sync.dma_start(cf[:], scr['c'][:, t0:t0 + Q])

                    lrow = lA_rows[:, t0:t0 + Q]
                    dtrow = dt_rows[:, t0:t0 + Q]

                    expl = p2s.tile([8, Q], F32R, tag="expl", name="expl")
                    nc.scalar.activation(expl[:], lrow, AF.Exp)
                    ddr0 = p2s.tile([8, Q], F32, tag="ddr0", name="ddr0")
                    nc.vector.tensor_scalar(ddr0[:], lrow, -1.0,
                                            lrow[:, Q - 1:Q].bitcast(F32),
                                            Op.mult, Op.add)
                    dd_rows = p2s.tile([8, Q], F32R, tag="ddrows", name="ddrows")
                    nc.scalar.activation(dd_rows[:], ddr0[:], AF.Exp)
                    nc.vector.tensor_tensor(dd_rows[:], dd_rows[:], dtrow,
                                            Op.mult)
                    dg = p2s.tile([8, 8], F32R, tag="dg", name="dg")
                    nc.vector.tensor_scalar_mul(dg[:], i8[:],
                                                expl[:, Q - 1:Q].bitcast(F32))

                    misc = p2ps1.tile([128, 256], F32, tag="misc", name="misc")
                    g_ps = misc[:, 0:128]
                    ddcol_ps = misc[:, 128:136]
                    decay_ps = misc[:, 136:144]
                    dtcol_ps = misc[:, 144:152]

                    mm(g_ps, bf[:], cf[:], start=True, stop=True)
                    mm(ddcol_ps, dd_rows[:], i8[:], start=True, stop=True)
                    mm(decay_ps, ones8[:], dg[:], start=True, stop=True)
                    mm(dtcol_ps, dtrow, i8[:], start=True, stop=True)

                    tps = p2ps1.tile([128, 5, 128], BF16, tag="xtm", name="xtm")
                    nc.tensor.transpose(tps[:, 4, :], bf[:], identb[:])
                    btm = p2s.tile([128, Q], BF16, tag="btm", name="btm")
                    nc.any.tensor_copy(btm[:], tps[:, 4, :])

                    for pt in range(4):
                        nc.tensor.transpose(tps[:, pt, :],
                                            xf[:, pt, :], identb[:])
                    xtm = p2s.tile([128, NHr * HD], BF16, tag="xtm_sb",
                                   name="xtm_sb")
                    nc.any.tensor_copy(xtm[:], tps[:, 0:4, :])
                    xw = p2s.tile([128, NHr * HD], BF16, tag="xw", name="xw")
                    for h in range(NHr):
                        nc.vector.tensor_scalar_mul(
                            xw[:, h * HD:(h + 1) * HD],
                            xtm[:, h * HD:(h + 1) * HD], ddcol_ps[:, h:h + 1])

                    y_ps = [p2ps1.tile([64, 512], F32, tag=f"y{half}",
                                       name=f"y{half}") for half in range(2)]
                    for pr in range(4):
                        h0, h1 = 2 * pr, 2 * pr + 1
                        pairps = p2ps2.tile([128, 512], F32, tag="pairps",
                                            name="pairps")
                        dpair = pairps[:, 0:256]
                        d2 = pairps[:, 256:512]
                        for i, h in enumerate((h0, h1)):
                            half = dpair[:, i * 128:(i + 1) * 128]
                            mm(half, sel8[:, h * 128:(h + 1) * 128], lrow,
                               start=True, stop=False)
                            mm(half, lrow,
                               negselp[:, pr * 256 + i * 128:
                                       pr * 256 + (i + 1) * 128],
                               start=False, stop=True)
                        dmask = p2s.tile([128, 256], F32, tag="dmask",
                                         name="dmask")
                        nc.vector.tensor_tensor(dmask[:], dpair, trimask2[:],
                                                Op.add)
                        w0 = p2s.tile([128, 256], F32, tag="w0", name="w0")
                        nc.scalar.activation(w0[:], dmask[:], AF.Exp)
                        mm(d2[:, 0:128], sel8[:, h0 * 128:(h0 + 1) * 128],
                           expl[:], start=True, stop=True)
                        mm(d2[:, 128:256], sel8[:, h1 * 128:(h1 + 1) * 128],
                           expl[:], start=True, stop=True)
                        for i, h in enumerate((h0, h1)):
                            wt = p2s.tile([128, Q], BF16, tag="wt", name="wt")
                            nc.vector.scalar_tensor_tensor(
                                wt[:], w0[:, i * 128:(i + 1) * 128],
                                dtcol_ps[:, h:h + 1], g_ps, Op.mult, Op.mult)
                            ce = p2s.tile([128, Q], F32R, tag="ce", name="ce")
                            nc.vector.tensor_tensor(
                                ce[:], d2[:, i * 128:(i + 1) * 128], cf[:],
                                Op.mult)
                            yp = y_ps[h // 4]
                            ysl = yp[:, (h % 4) * 128:(h % 4 + 1) * 128]
                            mm(ysl, xtm[:, h * HD:(h + 1) * HD], wt[:],
                               start=True, stop=False)
                            mm(ysl, S_all[:, h * HD:(h + 1) * HD], ce[:],
                               start=False, stop=True)

                    tp_ps = p2ps1.tile([128, 512], F32, tag="tp", name="tp")
                    mm(tp_ps[:], btm[:], xw[:], start=True, stop=True)
                    for h in range(NHr):
                        nc.vector.scalar_tensor_tensor(
                            S_all[:, h * HD:(h + 1) * HD],
                            S_all[:, h * HD:(h + 1) * HD],
                            decay_ps[:, h:h + 1], tp_ps[:, h * HD:(h + 1) * HD],
                            Op.mult, Op.add)

                    for pt in range(4):
                        yp = y_ps[pt // 2]
                        base = (pt % 2) * 256
                        ysl0 = y_sb[0:64, pt, cc_ * Q:(cc_ + 1) * Q]
                        ysl1 = y_sb[64:128, pt, cc_ * Q:(cc_ + 1) * Q]
                        nc.vector.scalar_tensor_tensor(
                            ysl0, xf[0:64, pt, :], C['dssm_c'][0:64, pt:pt + 1],
                            yp[0:64, base:base + 128], Op.mult, Op.add)
                        nc.vector.scalar_tensor_tensor(
                            ysl1, xf[64:128, pt, :],
                            C['dssm_c'][64:128, pt:pt + 1],
                            yp[0:64, base + 128:base + 256], Op.mult, Op.add)

                    if (ch + 1) % CPS == 0 and ch + 1 < NCHUNK:
                        nc.vector.memset(S_all[:].bitcast(F32), 0.0)

                # gated product + stats + out_proj for this token tile
                tok0 = nt * NT
                zt = p3f.tile([128, 4, NT], BF16, tag="zt", name="zt")
                nc.sync.dma_start(zt[:], scr['z'][:, tok0:tok0 + NT]
                                  .rearrange("(pt p) n -> p pt n", p=128))
                yz_all = p3f.tile([128, 4, NT], BF16, tag="yzall", name="yzall")
                ssq_ps = p3ps.tile([1, NT], F32, tag="ssqyz", name="ssqyz")
                for pt in range(4):
                    sz = p3f.tile([128, NT], BF16, tag="sz", name="sz")
                    silu(sz[:], zt[:, pt, :], pool=p3f, tag="szsig")
                    nc.vector.tensor_tensor(yz_all[:, pt, :], y_sb[:, pt, :],
                                            sz[:], Op.mult)
                    sqz = p3f.tile([128, NT], F32R, tag="sqz", name="sqz")
                    nc.scalar.activation(sqz[:], yz_all[:, pt, :], AF.Square)
                    mm(ssq_ps[:], ones128[:], sqz[:],
                       start=(pt == 0), stop=(pt == 3))
                nc.any.tensor_copy(ssq_yz_row[:, tok0:tok0 + NT], ssq_ps[:])

                for mi in range(16):
                    ps = p3ps.tile([128, NT], F32, tag="mt3", name="mt3")
                    for k in range(4):
                        mm(ps[:], w_out_t[:, k, mi * 128:(mi + 1) * 128],
                           yz_all[:, k, :], start=(k == 0), stop=(k == 3))
                    ot = p3f.tile([128, NT], BF16, tag="ot", name="ot")
                    nc.any.tensor_copy(ot[:], ps[:])
                    nc.sync.dma_start(
                        scr['ar1_in8'][nt][mi * 128:(mi + 1) * 128, :], ot[:])

                # inline chunked collectives: tiny stats AR then the big AR
                nc.sync.dma_start(scr['ssq_in8'][nt],
                                  ssq_yz_row[:, tok0:tok0 + NT])
                allreduce(scr['ssq_in8'][nt], scr['ssq_out8'][nt])
                allreduce(scr['ar1_in8'][nt], scr['ar1_out8'][nt])

        rows_a_es.close()

        # ================================= Phase 4: resid + ln2 + MLP + RS
        with tc.tile_pool(name="p4w", bufs=1) as p4w, \
             tc.tile_pool(name="p4", bufs=2) as p4, \
             tc.tile_pool(name="p4mt", bufs=1) as p4mt, \
             tc.tile_pool(name="p4row", bufs=1) as p4row, \
             tc.tile_pool(name="p4av", bufs=1) as p4av, \
             tc.tile_pool(name="p4ps_s", bufs=1, space="PSUM") as p4ps_s, \
             tc.tile_pool(name="p4ps_g", bufs=1, space="PSUM") as p4ps_g, \
             tc.tile_pool(name="p4ps_d", bufs=2, space="PSUM") as p4ps_d:
            wg_t = p4w.tile([128, 16, FFr], BF16, tag="wg_t", name="wg_t")
            nc.sync.dma_start(wg_t[:],
                              io['w_gate'].rearrange("(kt p) m -> p kt m", p=128))
            wu_t = p4w.tile([128, 16, FFr], BF16, tag="wu_t", name="wu_t")
            nc.sync.dma_start(wu_t[:],
                              io['w_up'].rearrange("(kt p) m -> p kt m", p=128))
            for k in range(16):
                nc.vector.tensor_scalar_mul(wg_t[:, k, :], wg_t[:, k, :],
                                            C['ln2_c'][:, k:k + 1])
                nc.vector.tensor_scalar_mul(wu_t[:, k, :], wu_t[:, k, :],
                                            C['ln2_c'][:, k:k + 1])
            wd_t = p4w.tile([128, 8, H], BF16, tag="wd_t", name="wd_t")
            nc.sync.dma_start(wd_t[:],
                              io['w_down'].rearrange("(kt p) m -> p kt m", p=128))

            for nt in range(n8):
                tok0 = nt * NT
                mt = p4mt.tile([128, 16, NT], BF16, tag="mt", name="mt")
                nc.sync.dma_start(mt[:], scr['ar1_out8'][nt]
                                  .rearrange("(kt p) n -> p kt n", p=128))
                # deferred gated-norm scale s3 for this chunk
                ssq_t = p4row.tile([1, NT], F32, tag="ssq_t", name="ssq_t")
                nc.sync.dma_start(ssq_t[:], scr['ssq_out8'][nt])
                ssq_l = p4row.tile([1, NT], F32, tag="ssq_l", name="ssq_l")
                nc.scalar.activation(ssq_l[:], ssq_t[:], AF.Ln,
                                     bias=eps1[:], scale=float(1.0 / DIN))
                s3_row = p4row.tile([1, NT], F32R, tag="s3row", name="s3row")
                nc.scalar.activation(s3_row[:], ssq_l[:], AF.Exp, scale=-0.5)
                s3b_ps = p4ps_s.tile([128, NT], F32, tag="s3bps", name="s3bps")
                mm(s3b_ps[:], ones1[:], s3_row[:], start=True, stop=True)
                s3b = p4.tile([128, NT], BF16, tag="s3b", name="s3b")
                nc.any.tensor_copy(s3b[:], s3b_ps[:])
                # s3-scale + residual add + ln2 stats
                ssq_ps = p4ps_s.tile([1, NT], F32, tag="ssq", name="ssq")
                for k in range(16):
                    ht = p4.tile([128, NT], BF16, tag="ht", name="ht")
                    nc.sync.dma_start(
                        ht[:], io['hsT'][k * 128:(k + 1) * 128, tok0:tok0 + NT])
                    nc.vector.tensor_tensor(mt[:, k, :], mt[:, k, :], s3b[:],
                                            Op.mult)
                    nc.vector.tensor_tensor(mt[:, k, :], mt[:, k, :], ht[:],
                                            Op.add)
                    nc.sync.dma_start(
                        io['resid2T'][k * 128:(k + 1) * 128, tok0:tok0 + NT],
                        mt[:, k, :])
                    sq = p4.tile([128, NT], F32R, tag="sq", name="sq")
                    nc.scalar.activation(sq[:], mt[:, k, :], AF.Square)
                    mm(ssq_ps[:], ones128[:], sq[:],
                       start=(k == 0), stop=(k == 15))
                sr0 = p4row.tile([1, NT], F32, tag="sr0", name="sr0")
                nc.scalar.activation(sr0[:], ssq_ps[:], AF.Ln,
                                     bias=eps1[:], scale=float(1.0 / H))
                s_row = p4row.tile([1, NT], F32R, tag="srow", name="srow")
                nc.scalar.activation(s_row[:], sr0[:], AF.Exp, scale=-0.5)
                sb_ps = p4ps_s.tile([128, NT], F32, tag="sbps", name="sbps")
                mm(sb_ps[:], ones1[:], s_row[:], start=True, stop=True)
                sb = p4.tile([128, NT], BF16, tag="sb", name="sb")
                nc.any.tensor_copy(sb[:], sb_ps[:])
                mtn = p4.tile([128, 16, NT], BF16, tag="mtn", name="mtn")
                for k in range(16):
                    nc.vector.tensor_tensor(mtn[:, k, :], mt[:, k, :], sb[:],
                                            Op.mult)
                # gate_up + silu*up (av kept in SBUF as down-proj k-tiles)
                av = p4av.tile([128, 8, NT], BF16, tag="av", name="av")
                for mi in range(8):
                    gp = p4ps_g.tile([128, NT], F32, tag="gp", name="gp")
                    up = p4ps_g.tile([128, NT], F32, tag="up", name="up")
                    for k in range(16):
                        mm(gp[:], wg_t[:, k, mi * 128:(mi + 1) * 128],
                           mtn[:, k, :], start=(k == 0), stop=(k == 15))
                    for k in range(16):
                        mm(up[:], wu_t[:, k, mi * 128:(mi + 1) * 128],
                           mtn[:, k, :], start=(k == 0), stop=(k == 15))
                    sg = p4.tile([128, NT], BF16, tag="sg", name="sg")
                    silu(sg[:], gp[:], pool=p4, tag="sgsig")
                    nc.vector.tensor_tensor(av[:, mi, :], sg[:], up[:], Op.mult)
                # down proj -> ReduceScatter chunk (host concats slices)
                for mo in range(16):
                    ps = p4ps_d.tile([128, NT], F32, tag="dps", name="dps")
                    for k in range(8):
                        mm(ps[:], wd_t[:, k, mo * 128:(mo + 1) * 128],
                           av[:, k, :], start=(k == 0), stop=(k == 7))
                    ot = p4.tile([128, NT], BF16, tag="ot4", name="ot4")
                    nc.any.tensor_copy(ot[:], ps[:])
                    nc.sync.dma_start(
                        scr['rs2_in8'][nt][mo * 128:(mo + 1) * 128, :], ot[:])
                if world > 1:
                    nc.gpsimd.collective_compute(
                        "ReduceScatter", Op.add,
                        replica_groups=[list(range(world))],
                        ins=[scr['rs2_in8'][nt]], outs=[scr['rs2_out8'][nt]])
                else:
                    nc.sync.dma_start(scr['rs2_out8'][nt],
                                      scr['rs2_in8'][nt][0:H // world, :])
                nc.sync.dma_start(io['out1T'][:, tok0:tok0 + NT],
                                  scr['rs2_out8'][nt])


# ================================================================ entry point
def kernel(**inputs):
    from concourse import bass_utils

    nc = build(world=TP, debug=False)
    in_maps = [shard_core_inputs(inputs, r) for r in range(TP)]
    res = bass_utils.run_bass_kernel_spmd(nc, in_maps, core_ids=list(range(TP)))
    out1T = np.concatenate(
        [np.asarray(res.results[r]['out1T'], dtype=np.float32)
         for r in range(TP)], axis=0)                # [H, T] feature-major
    out1 = np.ascontiguousarray(out1T.T).reshape(B, L, H)
    resid2 = np.ascontiguousarray(
        np.asarray(res.results[0]['resid2T'], dtype=np.float32).T
    ).reshape(B, L, H)
    return out1, resid2


if __name__ == '__main__':
    nc = build(world=1)
    print("built ok")


# revision 48
# speedup vs baseline: 1.1341x; 1.1341x over previous
"""Trainium2 Bass kernel for nn_BambaMixerDecoderLayer_84696755077458.

Tensor-parallel over 8 NeuronCores (vLLM-style), v2 (bf16):
  - in_proj / gate_up column-sharded, out_proj / down row-sharded
  - heads + conv channels sharded with d_inner; B/C conv streams replicated
  - SSM scan via chunked SSD (Mamba2): intra-chunk matmuls + small
    cross-chunk state recurrence.
  - bf16 weights/activations for all large GEMMs, scratch and collectives;
    fp32 for stats, decay rows and the SSD state.
  - Single merged in_proj pass; MLP (gate_up+down) fused in one pass.
  - Collectives chunked 8x along tokens and issued inline so they overlap
    with compute (no global barrier between SSD and MLP phases).
Everything on-device is feature-major ([feature, token]); host does layout
transforms (transpose / shard / concat) only.

Self-contained: hardcodes all shapes; needs only /opt/trn_rl_repo on sys.path.
"""
import sys
from contextlib import ExitStack

if '/opt/trn_rl_repo' not in sys.path:
    sys.path.insert(0, '/opt/trn_rl_repo')

import numpy as np

# ---------------------------------------------------------------- constants
H = 2048          # hidden
DIN = 4096        # mamba intermediate
DS = 128          # ssm state
DCONV = 4
NH = 64
HD = 64
FF = 8192
EPS = 1e-5
B, L = 2, 2048
T = B * L                         # 4096 tokens
CONV_DIM = DIN + 2 * DS           # 4352
D_IN_PROJ = 2 * DIN + 2 * DS + NH  # 8512

TP = 8
NHr = NH // TP                    # 8 heads / core
DINr = DIN // TP                  # 512
FFr = FF // TP                    # 1024
CONVr = DINr + 2 * DS             # 768 conv channels / core
MPROJ = DINr + CONVr + NHr        # 1288 in_proj cols / core

Q = 128                           # SSD chunk
NT = 512                          # token tile (also the collective chunk)
NEG = -3.0e38
SIM_SILU = False   # True: emit sigmoid+mul instead of Silu (CoreSim support)


def _f32(x):
    return np.ascontiguousarray(np.asarray(x, dtype=np.float32))


def _bf16(x):
    import ml_dtypes
    return np.ascontiguousarray(
        np.asarray(x, dtype=np.float32).astype(ml_dtypes.bfloat16))


# ================================================================ host prep
def host_constants():
    import ml_dtypes
    identb = np.eye(128, dtype=ml_dtypes.bfloat16)
    i8 = np.eye(8, dtype=np.float32)
    sel8 = np.zeros((8, 8 * 128), np.float32)
    for h in range(8):
        sel8[h, h * 128:(h + 1) * 128] = 1.0
    negselpair = np.zeros((8, 4 * 256), np.float32)
    for p in range(4):
        negselpair[2 * p, p * 256:p * 256 + 128] = -1.0
        negselpair[2 * p + 1, p * 256 + 128:p * 256 + 256] = -1.0
    ones8 = np.ones((8, 128), np.float32)
    ones1 = np.ones((1, 128), np.float32)
    ones128 = np.ones((128, 1), np.float32)
    tri = np.where(np.arange(Q)[:, None] > np.arange(Q)[None, :], NEG, 0.0)
    trimask2 = np.concatenate([tri, tri], axis=1).astype(np.float32)
    return dict(c_identb=identb, c_i8=i8, c_sel8=sel8, c_negselpair=negselpair,
                c_ones8=ones8, c_ones1=ones1, c_ones128=ones128,
                c_trimask2=trimask2)


def shard_core_inputs(inputs, r):
    """Build the per-core input map (all feature-major)."""
    w_in = _f32(inputs['w_in'])
    zs = slice(DINr * r, DINr * (r + 1))
    xs = slice(DIN + DINr * r, DIN + DINr * (r + 1))
    bs = slice(2 * DIN, 2 * DIN + DS)
    cs = slice(2 * DIN + DS, 2 * DIN + 2 * DS)
    dts = slice(2 * DIN + 2 * DS + NHr * r, 2 * DIN + 2 * DS + NHr * (r + 1))
    w_in_r = np.concatenate(
        [w_in[:, zs], w_in[:, xs], w_in[:, bs], w_in[:, cs], w_in[:, dts]], axis=1)

    conv_w = _f32(inputs['conv_w'])
    conv_w_r = np.concatenate([conv_w[DINr * r:DINr * (r + 1)], conv_w[DIN:]], axis=0)
    conv_b = _f32(inputs['conv_b'])
    conv_b_r = np.concatenate([conv_b[DINr * r:DINr * (r + 1)], conv_b[DIN:]], axis=0)

    hs = _f32(inputs['hidden_states'])
    hs = hs.reshape(-1, H)

    A_r = _f32(inputs['A_log'])[NHr * r:NHr * (r + 1)]
    dtb_r = _f32(inputs['dt_bias'])[NHr * r:NHr * (r + 1)]
    D_r = _f32(inputs['D_ssm'])[NHr * r:NHr * (r + 1)]

    m = dict(host_constants())
    m['hsT'] = _bf16(hs.T)                                      # [2048, T]
    m['w_in'] = _bf16(w_in_r)                                   # [2048, 1288]
    # per-k-tile column form of per-feature vectors: [128, n_tiles]
    m['ln1_c'] = np.ascontiguousarray(_f32(inputs['ln1_w']).reshape(16, 128).T)
    m['ln2_c'] = np.ascontiguousarray(_f32(inputs['ln2_w']).reshape(16, 128).T)
    m['normw_c'] = np.ascontiguousarray(
        _f32(inputs['norm_w'])[DINr * r:DINr * (r + 1)].reshape(4, 128).T)
    m['dssm_c'] = np.ascontiguousarray(
        np.repeat(D_r, HD).reshape(4, 128).T)                   # [128, 4]
    # conv weights: [128, 6*4] with [p, pt*4+d]
    m['conv_w'] = np.ascontiguousarray(
        conv_w_r.reshape(6, 128, DCONV).transpose(1, 0, 2).reshape(128, 6 * DCONV))
    m['conv_b'] = np.ascontiguousarray(conv_b_r.reshape(6, 128).T)  # [128, 6]
    m['a_col'] = np.ascontiguousarray((-np.exp(A_r))[:, None])   # [8,1]
    m['dtb_col'] = np.ascontiguousarray(dtb_r[:, None])          # [8,1]
    m['w_out'] = _bf16(_f32(inputs['w_out'])[DINr * r:DINr * (r + 1)])
    wgu = _f32(inputs['w_gate_up'])
    m['w_gate'] = _bf16(wgu[:, FFr * r:FFr * (r + 1)])
    m['w_up'] = _bf16(wgu[:, FF + FFr * r:FF + FFr * (r + 1)])
    m['w_down'] = _bf16(_f32(inputs['w_down'])[FFr * r:FFr * (r + 1)])
    return m


# ================================================================ the kernel
def build(world=TP, debug=False, T_=T):
    import concourse.mybir as mybir
    import concourse.tile as tile
    from concourse import bacc
    from concourse.alu_op_type import AluOpType as Op

    AF = mybir.ActivationFunctionType
    F32 = mybir.dt.float32
    BF16 = mybir.dt.bfloat16

    nc = bacc.Bacc("TRN2", target_bir_lowering=False, debug=False,
                   num_devices=world)

    F32R = mybir.dt.float32r
    n8 = T_ // NT

    def din(name, shape, dt):
        return nc.dram_tensor(name, list(shape), dt, kind="ExternalInput").ap()

    BIN = {'hsT', 'w_in', 'w_out', 'w_gate', 'w_up', 'w_down', 'c_identb'}
    RIN = {'c_i8', 'c_sel8', 'c_negselpair', 'c_ones8', 'c_ones1', 'c_ones128'}
    io = {}
    for name, shape in [
            ('hsT', (H, T_)), ('w_in', (H, MPROJ)),
            ('ln1_c', (128, 16)), ('ln2_c', (128, 16)),
            ('normw_c', (128, 4)), ('dssm_c', (128, 4)),
            ('conv_w', (128, 24)), ('conv_b', (128, 6)),
            ('a_col', (8, 1)), ('dtb_col', (8, 1)),
            ('w_out', (DINr, H)), ('w_gate', (H, FFr)), ('w_up', (H, FFr)),
            ('w_down', (FFr, H)),
            ('c_identb', (128, 128)), ('c_i8', (8, 8)), ('c_sel8', (8, 1024)),
            ('c_negselpair', (8, 1024)), ('c_ones8', (8, 128)),
            ('c_ones1', (1, 128)), ('c_ones128', (128, 1)),
            ('c_trimask2', (128, 256))]:
        dt = BF16 if name in BIN else (F32R if name in RIN else F32)
        io[name] = din(name, shape, dt)

    io['out1T'] = nc.dram_tensor("out1T", [H // world, T_], BF16,
                                 kind="ExternalOutput").ap()
    io['resid2T'] = nc.dram_tensor("resid2T", [H, T_], BF16,
                                   kind="ExternalOutput").ap()

    skind = "ExternalOutput" if debug else "Internal"
    scr = {}
    scr['z'] = nc.dram_tensor("z_s", [DINr, T_], BF16, kind=skind).ap()
    scr['x'] = nc.dram_tensor("x_s", [DINr, T_], BF16, kind=skind).ap()
    scr['b'] = nc.dram_tensor("b_s", [DS, T_], BF16, kind=skind).ap()
    scr['c'] = nc.dram_tensor("c_s", [DS, T_], BF16, kind=skind).ap()
    scr['ar1_in8'] = [
        nc.dram_tensor(f"ar1_in{q}", [H, NT], BF16, kind="Internal").ap()
        for q in range(n8)]
    scr['ar1_out8'] = [
        nc.dram_tensor(f"ar1_out{q}", [H, NT], BF16, kind="Internal",
                       addr_space="Shared").ap() for q in range(n8)]
    scr['ssq_in8'] = [
        nc.dram_tensor(f"ssq_in{q}", [1, NT], F32, kind="Internal").ap()
        for q in range(n8)]
    scr['ssq_out8'] = [
        nc.dram_tensor(f"ssq_out{q}", [1, NT], F32, kind="Internal",
                       addr_space="Shared").ap() for q in range(n8)]
    scr['rs2_in8'] = [
        nc.dram_tensor(f"rs2_in{q}", [H, NT], BF16, kind="Internal").ap()
        for q in range(n8)]
    scr['rs2_out8'] = [
        nc.dram_tensor(f"rs2_out{q}", [H // world, NT], BF16,
                       kind="Internal").ap() for q in range(n8)]
    scr['mtn0'] = nc.dram_tensor("mtn0_s", [H, NT], BF16, kind="Internal").ap()

    with tile.TileContext(nc) as tc:
        _body(tc, io, scr, world, debug, mybir, tile, Op, AF, F32, T_)

    nc.compile()
    return nc


def _body(tc, io, scr, world, debug, mybir, tile, Op, AF, F32, T_):
    nc = tc.nc
    F32R = mybir.dt.float32r
    BF16 = mybir.dt.bfloat16
    n8 = T_ // NT
    NCHUNK = T_ // Q
    CPS = (T_ // B) // Q          # chunks per sequence

    def mm(out, lhsT, rhs, start, stop, skip=False):
        if lhsT.dtype == F32:
            lhsT = lhsT.bitcast(F32R)
        if rhs.dtype == F32:
            rhs = rhs.bitcast(F32R)
        nc.tensor.matmul(out, lhsT, rhs, start=start, stop=stop,
                         skip_group_check=skip)

    def silu(out_ap, in_ap, bias=0.0, pool=None, tag="silu_tmp"):
        if SIM_SILU:
            tmp = pool.tile(list(out_ap.shape), F32, tag=tag, name=tag)
            nc.scalar.activation(tmp[:], in_ap, AF.Sigmoid, bias=bias, scale=1.0)
            if isinstance(bias, float) and bias == 0.0:
                nc.vector.tensor_tensor(out_ap, in_ap, tmp[:], Op.mult)
            else:
                raise NotImplementedError("SIM_SILU with bias AP")
        else:
            nc.scalar.activation(out_ap, in_ap, AF.Silu, bias=bias, scale=1.0)

    def allreduce(in_ap, out_ap):
        if world > 1:
            nc.gpsimd.collective_compute(
                "AllReduce", Op.add, replica_groups=[list(range(world))],
                ins=[in_ap], outs=[out_ap])
        else:
            nc.sync.dma_start(out_ap, in_ap)

    with ExitStack() as ES:
        cpool = ES.enter_context(tc.tile_pool(name="consts", bufs=1))

        # -------------------------------------------------------- constants
        C = {}
        RT = {'c_i8', 'c_sel8', 'c_negselpair', 'c_ones8', 'c_ones1',
              'c_ones128'}
        for nm, shape in [('c_identb', (128, 128)), ('c_i8', (8, 8)),
                          ('c_sel8', (8, 1024)), ('c_negselpair', (8, 1024)),
                          ('c_ones8', (8, 128)), ('c_ones1', (1, 128)),
                          ('c_ones128', (128, 1)), ('c_trimask2', (128, 256)),
                          ('ln1_c', (128, 16)), ('ln2_c', (128, 16)),
                          ('normw_c', (128, 4)), ('dssm_c', (128, 4)),
                          ('conv_w', (128, 24)), ('conv_b', (128, 6)),
                          ('a_col', (8, 1)), ('dtb_col', (8, 1))]:
            dt = BF16 if nm == 'c_identb' else (F32R if nm in RT else F32)
            t = cpool.tile(list(shape), dt, tag=nm)
            nc.sync.dma_start(t[:], io[nm])
            C[nm] = t
        identb, i8 = C['c_identb'], C['c_i8']
        sel8, negselp = C['c_sel8'], C['c_negselpair']
        ones8, ones1, ones128 = C['c_ones8'], C['c_ones1'], C['c_ones128']
        trimask2 = C['c_trimask2']

        eps1 = cpool.tile([1, 1], F32, tag="eps1", name="eps1")
        nc.vector.memset(eps1[:], float(EPS))

        # ======================================================== Phase 1
        # merged single pass over hsT: ln1 stats + z + dt + xBC + conv
        rows_a_es = ExitStack()
        rows_a = rows_a_es.enter_context(tc.tile_pool(name="rows_a", bufs=1))
        dt_rows = rows_a.tile([8, T_], F32R, tag="dt_rows", name="dt_rows")
        lA_rows = rows_a.tile([8, T_], F32R, tag="lA_rows", name="lA_rows")
        ssq_yz_row = rows_a.tile([1, T_], F32, tag="ssq_yz", name="ssq_yz")

        with tc.tile_pool(name="p1w", bufs=1) as p1w, \
             tc.tile_pool(name="p1", bufs=2) as p1, \
             tc.tile_pool(name="convp", bufs=2) as convp, \
             tc.tile_pool(name="p1ps_s", bufs=1, space="PSUM") as p1ps_s, \
             tc.tile_pool(name="p1ps_m", bufs=2, space="PSUM") as p1ps_m:

            # first token tile is prefetched BEFORE the weights so the ln1
            # stats matmuls warm up the PE while w_in streams in
            hst0 = p1.tile([128, 16, NT], BF16, tag="hst", name="hst")
            nc.sync.dma_start(hst0[:], io['hsT'][:, 0:NT]
                              .rearrange("(kt p) n -> p kt n", p=128))
            # all in_proj columns per core: [z | xBC | dt] = 1288
            w1 = p1w.tile([128, 16, MPROJ], BF16, tag="w1", name="w1")
            nc.sync.dma_start(
                w1[:], io['w_in'].rearrange("(kt p) m -> p kt m", p=128))
            for k in range(16):
                nc.vector.tensor_scalar_mul(w1[:, k, :], w1[:, k, :],
                                            C['ln1_c'][:, k:k + 1])

            halo_prev = None
            for nt in range(n8):
                tok0 = nt * NT
                seq_start = (tok0 % (T_ // B)) == 0
                if nt == 0:
                    hst = hst0
                else:
                    hst = p1.tile([128, 16, NT], BF16, tag="hst", name="hst")
                    nc.sync.dma_start(hst[:], io['hsT'][:, tok0:tok0 + NT]
                                      .rearrange("(kt p) n -> p kt n", p=128))
                # ln1 stats (ACT squares; matmuls never wait on these)
                ssq_ps = p1ps_s.tile([1, NT], F32, tag="ssq", name="ssq")
                for k in range(16):
                    sq = p1.tile([128, NT], F32R, tag="sq", name="sq")
                    nc.scalar.activation(sq[:], hst[:, k, :], AF.Square)
                    mm(ssq_ps[:], ones128[:], sq[:],
                       start=(k == 0), stop=(k == 15))
                sr0 = p1.tile([1, NT], F32, tag="sr0", name="sr0", bufs=1)
                nc.scalar.activation(sr0[:], ssq_ps[:], AF.Ln,
                                     bias=eps1[:], scale=float(1.0 / H))
                s_row = p1.tile([1, NT], F32R, tag="s_row", name="s_row",
                                bufs=1)
                nc.scalar.activation(s_row[:], sr0[:], AF.Exp, scale=-0.5)
                sb_ps = p1ps_s.tile([128, NT], F32, tag="sbps", name="sbps")
                mm(sb_ps[:], ones1[:], s_row[:], start=True, stop=True)
                sb = p1.tile([128, NT], F32, tag="sb", name="sb")
                nc.any.tensor_copy(sb[:], sb_ps[:])
                # z m-tiles: matmul on RAW hidden, scale on the way out
                for mi in range(4):
                    ps = p1ps_m.tile([128, NT], F32, tag="mt", name="mt")
                    for k in range(16):
                        mm(ps[:], w1[:, k, mi * 128:(mi + 1) * 128],
                           hst[:, k, :], start=(k == 0), stop=(k == 15))
                    zt = p1.tile([128, NT], BF16, tag="z", name="z")
                    nc.vector.tensor_tensor(zt[:], ps[:], sb[:], Op.mult)
                    nc.sync.dma_start(
                        scr['z'][mi * 128:(mi + 1) * 128, tok0:tok0 + NT], zt[:])
                # dt m-tile (8 wide)
                dtp = p1ps_s.tile([8, NT], F32, tag="mtdt", name="mtdt")
                for k in range(16):
                    mm(dtp[:], w1[:, k, DINr + CONVr:MPROJ], hst[:, k, :],
                       start=(k == 0), stop=(k == 15))
                dt_raw = p1.tile([8, NT], F32, tag="dtraw", name="dtraw",
                                 bufs=1)
                nc.vector.tensor_tensor(dt_raw[:], dtp[:], sb[:8, :], Op.mult)
                e8 = p1.tile([8, NT], F32, tag="e8", name="e8", bufs=1)
                nc.scalar.activation(e8[:], dt_raw[:], AF.Exp,
                                     bias=C['dtb_col'][:], scale=1.0)
                e8p = p1.tile([8, NT], F32, tag="e8p", name="e8p", bufs=1)
                nc.vector.tensor_scalar_add(e8p[:], e8[:], 1.0)
                nc.scalar.activation(dt_rows[:, tok0:tok0 + NT], e8p[:], AF.Ln)
                logda = p1.tile([8, NT], F32, tag="logda", name="logda",
                                bufs=1)
                nc.vector.tensor_scalar_mul(logda[:], dt_rows[:, tok0:tok0 + NT],
                                            C['a_col'][:])
                for c in range(NT // Q):
                    nc.vector.tensor_tensor_scan(
                        lA_rows[:, tok0 + c * Q:tok0 + (c + 1) * Q],
                        ones8[:, :Q].bitcast(F32), logda[:, c * Q:(c + 1) * Q],
                        0.0, Op.mult, Op.add)

                # xBC m-tiles + causal conv
                halo = [convp.tile([128, NT + 3], BF16, tag=f"halo{pt}",
                                   name=f"halo{pt}") for pt in range(6)]
                for pt in range(6):
                    ps = p1ps_m.tile([128, NT], F32, tag="mt", name="mt")
                    for k in range(16):
                        mm(ps[:], w1[:, k, DINr + pt * 128:DINr + (pt + 1) * 128],
                           hst[:, k, :], start=(k == 0), stop=(k == 15))
                    nc.vector.tensor_tensor(halo[pt][:, 3:3 + NT], ps[:], sb[:],
                                            Op.mult)
                for pt in range(6):
                    if seq_start:
                        nc.vector.memset(halo[pt][:, 0:3], 0.0)
                    else:
                        nc.vector.tensor_copy(halo[pt][:, 0:3],
                                              halo_prev[pt][:, NT:NT + 3])
                    acc = convp.tile([128, NT], BF16, tag="cacc", name="cacc")
                    nc.vector.tensor_scalar_mul(
                        acc[:], halo[pt][:, 0:NT],
                        C['conv_w'][:, pt * 4:pt * 4 + 1])
                    for d in range(1, 4):
                        nc.vector.scalar_tensor_tensor(
                            acc[:], halo[pt][:, d:d + NT],
                            C['conv_w'][:, pt * 4 + d:pt * 4 + d + 1],
                            acc[:], Op.mult, Op.add)
                    cact = convp.tile([128, NT], BF16, tag="cact", name="cact")
                    if SIM_SILU:
                        nc.vector.tensor_scalar_add(acc[:], acc[:],
                                                    C['conv_b'][:, pt:pt + 1])
                        silu(cact[:], acc[:], pool=convp, tag="cvsig")
                    else:
                        nc.scalar.activation(cact[:], acc[:], AF.Silu,
                                             bias=C['conv_b'][:, pt:pt + 1],
                                             scale=1.0)
                    if pt < 4:
                        nc.sync.dma_start(
                            scr['x'][pt * 128:(pt + 1) * 128, tok0:tok0 + NT],
                            cact[:])
                    elif pt == 4:
                        nc.sync.dma_start(scr['b'][:, tok0:tok0 + NT], cact[:])
                    else:
                        nc.sync.dma_start(scr['c'][:, tok0:tok0 + NT], cact[:])
                halo_prev = halo

        # ============================================ Phase 2: SSD + gated
        # norm + out_proj, fused per token-tile. out_proj runs on UNSCALED
        # yz — the rms scale s3 commutes through the matmul and the
        # AllReduce, and is applied in Phase 4. AR chunks issued inline.
        with tc.tile_pool(name="p2", bufs=3) as p2, \
             tc.tile_pool(name="p2s", bufs=2) as p2s, \
             tc.tile_pool(name="state", bufs=1) as spool, \
             tc.tile_pool(name="p2pre", bufs=1) as p2pre, \
             tc.tile_pool(name="p3f", bufs=2) as p3f, \
             tc.tile_pool(name="p3w", bufs=1) as p3w, \
             tc.tile_pool(name="p2ps1", bufs=1, space="PSUM") as p2ps1, \
             tc.tile_pool(name="p2ps2", bufs=2, space="PSUM") as p2ps2, \
             tc.tile_pool(name="p3ps", bufs=2, space="PSUM") as p3ps:

            w_out_t = p3w.tile([128, 4, H], BF16, tag="w_out_t", name="w_out_t")
            nc.sync.dma_start(w_out_t[:],
                              io['w_out'].rearrange("(kt p) m -> p kt m", p=128))
            for k in range(4):
                nc.vector.tensor_scalar_mul(w_out_t[:, k, :], w_out_t[:, k, :],
                                            C['normw_c'][:, k:k + 1])


            S_all = spool.tile([128, NHr * HD], F32R, tag="S_all", name="S_all")
            nc.vector.memset(S_all[:].bitcast(F32), 0.0)

            for nt in range(n8):
                y_sb = p3f.tile([128, 4, NT], BF16, tag="ysb", name="ysb")
                for cc_ in range(NT // Q):
                    ch = nt * (NT // Q) + cc_
                    t0 = ch * Q
                    xf = p2.tile([128, 4, Q], BF16, tag="xf", name="xf")
                    nc.sync.dma_start(xf[:], scr['x'][:, t0:t0 + Q]
                                      .rearrange("(pt p) n -> p pt n", p=128))
                    bf = p2.tile([128, Q], BF16, tag="bf", name="bf")
                    nc.sync.dma_start(bf[:], scr['b'][:, t0:t0 + Q])
                    cf = p2.tile([128, Q], BF16, tag="cf", name="cf")
                    nc.sync.dma_start(cf[:], scr['c'][:, t0:t0 + Q])

                    lrow = lA_rows[:, t0:t0 + Q]
                    dtrow = dt_rows[:, t0:t0 + Q]

                    expl = p2s.tile([8, Q], F32R, tag="expl", name="expl")
                    nc.scalar.activation(expl[:], lrow, AF.Exp)
                    ddr0 = p2s.tile([8, Q], F32, tag="ddr0", name="ddr0")
                    nc.vector.tensor_scalar(ddr0[:], lrow, -1.0,
                                            lrow[:, Q - 1:Q].bitcast(F32),
                                            Op.mult, Op.add)
                    dd_rows = p2s.tile([8, Q], F32R, tag="ddrows", name="ddrows")
                    nc.scalar.activation(dd_rows[:], ddr0[:], AF.Exp)
                    nc.vector.tensor_tensor(dd_rows[:], dd_rows[:], dtrow,
                                            Op.mult)
                    dg = p2s.tile([8, 8], F32R, tag="dg", name="dg")
                    nc.vector.tensor_scalar_mul(dg[:], i8[:],
                                                expl[:, Q - 1:Q].bitcast(F32))

                    misc = p2ps1.tile([128, 160], F32, tag="misc", name="misc")
                    g_ps = misc[:, 0:128]
                    ddcol_ps = misc[:, 128:136]
                    decay_ps = misc[:, 136:144]
                    dtcol_ps = misc[:, 144:152]

                    mm(g_ps, bf[:], cf[:], start=True, stop=True)
                    mm(ddcol_ps, dd_rows[:], i8[:], start=True, stop=True)
                    mm(decay_ps, ones8[:], dg[:], start=True, stop=True)
                    mm(dtcol_ps, dtrow, i8[:], start=True, stop=True)
                    g2 = p2s.tile([128, 256], F32R, tag="g2", name="g2")
                    nc.any.tensor_copy(g2[:, 0:128], g_ps)
                    nc.any.tensor_copy(g2[:, 128:256], g_ps)
                    cf2 = p2s.tile([128, 256], BF16, tag="cf2", name="cf2")
                    nc.vector.tensor_copy(cf2[:, 0:128], cf[:])
                    nc.vector.tensor_copy(cf2[:, 128:256], cf[:])
                    # late-read scalars leave PSUM early so misc can
                    # single-buffer without serializing chunks
                    dsc = p2s.tile([128, 16], F32, tag="dsc", name="dsc")
                    nc.any.tensor_copy(dsc[:], misc[:, 136:152])
                    decay_sb = dsc[:, 0:8]
                    dtcol_sb = dsc[:, 8:16]

                    tps = p2ps1.tile([128, 5, 128], BF16, tag="xtm", name="xtm")
                    nc.tensor.transpose(tps[:, 4, :], bf[:], identb[:])
                    btm = p2s.tile([128, Q], BF16, tag="btm", name="btm")
                    nc.any.tensor_copy(btm[:], tps[:, 4, :])

                    for pt in range(4):
                        nc.tensor.transpose(tps[:, pt, :],
                                            xf[:, pt, :], identb[:])
                    xtm = p2s.tile([128, NHr * HD], BF16, tag="xtm_sb",
                                   name="xtm_sb")
                    nc.any.tensor_copy(xtm[:], tps[:, 0:4, :])
                    xw = p2s.tile([128, NHr * HD], BF16, tag="xw", name="xw")
                    for h in range(NHr):
                        nc.vector.tensor_scalar_mul(
                            xw[:, h * HD:(h + 1) * HD],
                            xtm[:, h * HD:(h + 1) * HD], ddcol_ps[:, h:h + 1])

                    def y_readout(yp, pt):
                        base = (pt % 2) * 256
                        ysl0 = y_sb[0:64, pt, cc_ * Q:(cc_ + 1) * Q]
                        ysl1 = y_sb[64:128, pt, cc_ * Q:(cc_ + 1) * Q]
                        nc.vector.scalar_tensor_tensor(
                            ysl0, xf[0:64, pt, :], C['dssm_c'][0:64, pt:pt + 1],
                            yp[0:64, base:base + 128], Op.mult, Op.add)
                        nc.vector.scalar_tensor_tensor(
                            ysl1, xf[64:128, pt, :],
                            C['dssm_c'][64:128, pt:pt + 1],
                            yp[0:64, base + 128:base + 256], Op.mult, Op.add)

                    y_cur = None
                    for pr in range(4):
                        h0, h1 = 2 * pr, 2 * pr + 1
                        if pr % 2 == 0:
                            y_cur = p2ps1.tile([64, 512], F32, tag="y",
                                               name="y")
                        pairps = p2ps2.tile([128, 512], F32, tag="pairps",
                                            name="pairps")
                        dpair = pairps[:, 0:256]
                        d2 = pairps[:, 256:512]
                        for i, h in enumerate((h0, h1)):
                            half = dpair[:, i * 128:(i + 1) * 128]
                            mm(half, sel8[:, h * 128:(h + 1) * 128], lrow,
                               start=True, stop=False)
                            mm(half, lrow,
                               negselp[:, pr * 256 + i * 128:
                                       pr * 256 + (i + 1) * 128],
                               start=False, stop=True)
                        dmask = p2s.tile([128, 256], F32, tag="dmask",
                                         name="dmask")
                        nc.vector.tensor_tensor(dmask[:], dpair, trimask2[:],
                                                Op.add)
                        w0 = p2s.tile([128, 256], F32, tag="w0", name="w0")
                        nc.scalar.activation(w0[:], dmask[:], AF.Exp)
                        mm(d2[:, 0:128], sel8[:, h0 * 128:(h0 + 1) * 128],
                           expl[:], start=True, stop=True)
                        mm(d2[:, 128:256], sel8[:, h1 * 128:(h1 + 1) * 128],
                           expl[:], start=True, stop=True)
                        wt = p2s.tile([128, 256], BF16, tag="wt", name="wt")
                        for i, h in enumerate((h0, h1)):
                            nc.vector.scalar_tensor_tensor(
                                wt[:, i * 128:(i + 1) * 128],
                                w0[:, i * 128:(i + 1) * 128],
                                dtcol_sb[:, h:h + 1],
                                g2[:, i * 128:(i + 1) * 128],
                                Op.mult, Op.mult)
                        ce = p2s.tile([128, 256], F32R, tag="ce", name="ce")
                        nc.vector.tensor_tensor(ce[:], d2, cf2[:], Op.mult)
                        for i, h in enumerate((h0, h1)):
                            ysl = y_cur[:, (h % 4) * 128:(h % 4 + 1) * 128]
                            mm(ysl, xtm[:, h * HD:(h + 1) * HD],
                               wt[:, i * 128:(i + 1) * 128],
                               start=True, stop=False)
                            mm(ysl, S_all[:, h * HD:(h + 1) * HD],
                               ce[:, i * 128:(i + 1) * 128],
                               start=False, stop=True)
                        if pr % 2 == 1:
                            y_readout(y_cur, pr // 2 * 2)
                            y_readout(y_cur, pr // 2 * 2 + 1)

                    tp_ps = p2ps1.tile([128, 512], F32, tag="tp", name="tp")
                    mm(tp_ps[:], btm[:], xw[:], start=True, stop=True)
                    for h in range(NHr):
                        nc.vector.scalar_tensor_tensor(
                            S_all[:, h * HD:(h + 1) * HD],
                            S_all[:, h * HD:(h + 1) * HD],
                            decay_sb[:, h:h + 1], tp_ps[:, h * HD:(h + 1) * HD],
                            Op.mult, Op.add)

                    if (ch + 1) % CPS == 0 and ch + 1 < NCHUNK:
                        nc.vector.memset(S_all[:].bitcast(F32), 0.0)

                # gated product + stats + out_proj for this token tile
                tok0 = nt * NT
                zt = p3f.tile([128, 4, NT], BF16, tag="zt", name="zt")
                nc.sync.dma_start(zt[:], scr['z'][:, tok0:tok0 + NT]
                                  .rearrange("(pt p) n -> p pt n", p=128))
                yz_all = p3f.tile([128, 4, NT], BF16, tag="yzall", name="yzall")
                ssq_full = p3ps.tile([128, NT], F32, tag="mt3", name="mt3ssq")
                ssq_ps = ssq_full[0:1, :]
                for pt in range(4):
                    sz = p3f.tile([128, NT], BF16, tag="sz", name="sz")
                    silu(sz[:], zt[:, pt, :], pool=p3f, tag="szsig")
                    nc.vector.tensor_tensor(yz_all[:, pt, :], y_sb[:, pt, :],
                                            sz[:], Op.mult)
                    sqz = p3f.tile([128, NT], F32R, tag="sqz", name="sqz")
                    nc.scalar.activation(sqz[:], yz_all[:, pt, :], AF.Square)
                    mm(ssq_ps[:], ones128[:], sqz[:],
                       start=(pt == 0), stop=(pt == 3))
                nc.any.tensor_copy(ssq_yz_row[:, tok0:tok0 + NT], ssq_ps[:])

                for mi in range(16):
                    ps = p3ps.tile([128, NT], F32, tag="mt3", name="mt3")
                    for k in range(4):
                        mm(ps[:], w_out_t[:, k, mi * 128:(mi + 1) * 128],
                           yz_all[:, k, :], start=(k == 0), stop=(k == 3))
                    ot = p3f.tile([128, NT], BF16, tag="ot", name="ot")
                    nc.any.tensor_copy(ot[:], ps[:])
                    nc.sync.dma_start(
                        scr['ar1_in8'][nt][mi * 128:(mi + 1) * 128, :], ot[:])

                # inline chunked collectives: tiny stats AR then the big AR
                nc.sync.dma_start(scr['ssq_in8'][nt],
                                  ssq_yz_row[:, tok0:tok0 + NT])
                allreduce(scr['ssq_in8'][nt], scr['ssq_out8'][nt])
                allreduce(scr['ar1_in8'][nt], scr['ar1_out8'][nt])

                if nt == max(0, n8 - 2):
                    # Phase-4 prep for tile 0, overlapped with the P2 tail.
                    # Result (mtn for tile 0) is bounced via DRAM.
                    mt0 = p2pre.tile([128, 16, NT], BF16, tag="mt0",
                                     name="mt0")
                    nc.sync.dma_start(mt0[:], scr['ar1_out8'][0]
                                      .rearrange("(kt p) n -> p kt n", p=128))
                    sqt0 = p2s.tile([1, NT], F32, tag="sqt0", name="sqt0")
                    nc.sync.dma_start(sqt0[:], scr['ssq_out8'][0])
                    sql0 = p2s.tile([1, NT], F32, tag="sql0", name="sql0")
                    nc.scalar.activation(sql0[:], sqt0[:], AF.Ln,
                                         bias=eps1[:], scale=float(1.0 / DIN))
                    s3r0 = p2s.tile([1, NT], F32R, tag="s3r0", name="s3r0")
                    nc.scalar.activation(s3r0[:], sql0[:], AF.Exp, scale=-0.5)
                    bps = p3ps.tile([128, NT], F32, tag="mt3", name="mt3s3b")
                    mm(bps[:], ones1[:], s3r0[:], start=True, stop=True)
                    s3b0 = p2pre.tile([128, NT], BF16, tag="s3b0", name="s3b0")
                    nc.any.tensor_copy(s3b0[:], bps[:])
                    sqf = p3ps.tile([128, NT], F32, tag="mt3", name="mt3ssq0")
                    for k in range(16):
                        ht0 = p2.tile([128, NT], BF16, tag="ht0", name="ht0")
                        nc.sync.dma_start(ht0[:],
                                          io['hsT'][k * 128:(k + 1) * 128,
                                                    0:NT])
                        nc.vector.tensor_tensor(mt0[:, k, :], mt0[:, k, :],
                                                s3b0[:], Op.mult)
                        nc.vector.tensor_tensor(mt0[:, k, :], mt0[:, k, :],
                                                ht0[:], Op.add)
                        nc.sync.dma_start(
                            io['resid2T'][k * 128:(k + 1) * 128, 0:NT],
                            mt0[:, k, :])
                        sq0 = p2s.tile([128, NT], F32R, tag="sq0", name="sq0")
                        nc.scalar.activation(sq0[:], mt0[:, k, :], AF.Square)
                        mm(sqf[0:1, :], ones128[:], sq0[:],
                           start=(k == 0), stop=(k == 15))
                    slr0 = p2s.tile([1, NT], F32, tag="slr0", name="slr0")
                    nc.scalar.activation(slr0[:], sqf[0:1, :], AF.Ln,
                                         bias=eps1[:], scale=float(1.0 / H))
                    sr_0 = p2s.tile([1, NT], F32R, tag="sr_0", name="sr_0")
                    nc.scalar.activation(sr_0[:], slr0[:], AF.Exp, scale=-0.5)
                    sbp0 = p3ps.tile([128, NT], F32, tag="mt3", name="mt3sb0")
                    mm(sbp0[:], ones1[:], sr_0[:], start=True, stop=True)
                    sb0 = p2pre.tile([128, NT], BF16, tag="sb0", name="sb0")
                    nc.any.tensor_copy(sb0[:], sbp0[:])
                    mtn0 = p2pre.tile([128, 16, NT], BF16, tag="mtn0",
                                      name="mtn0")
                    for k in range(16):
                        nc.vector.scalar_tensor_tensor(
                            mtn0[:, k, :], mt0[:, k, :], C['ln2_c'][:, k:k + 1],
                            sb0[:], Op.mult, Op.mult)
                    nc.sync.dma_start(
                        scr['mtn0'].rearrange("(kt p) n -> p kt n", p=128),
                        mtn0[:])

        rows_a_es.close()

        # ================================= Phase 4: resid + ln2 + MLP + RS
        # Software-pipelined: tile j+1's resid/stats/mtn prep is emitted
        # between tile j's gate_up and down matmuls; tile 0's prep ran in
        # Phase 2 (bounced via scr['mtn0']).
        with tc.tile_pool(name="p4w", bufs=1) as p4w, \
             tc.tile_pool(name="p4", bufs=2) as p4, \
             tc.tile_pool(name="p4mt", bufs=1) as p4mt, \
             tc.tile_pool(name="p4row", bufs=1) as p4row, \
             tc.tile_pool(name="p4av", bufs=1) as p4av, \
             tc.tile_pool(name="p4ps_s", bufs=1, space="PSUM") as p4ps_s, \
             tc.tile_pool(name="p4ps_g", bufs=1, space="PSUM") as p4ps_g, \
             tc.tile_pool(name="p4ps_d", bufs=2, space="PSUM") as p4ps_d:
            wg_t = p4w.tile([128, 16, FFr], BF16, tag="wg_t", name="wg_t")
            nc.sync.dma_start(wg_t[:],
                              io['w_gate'].rearrange("(kt p) m -> p kt m", p=128))
            wu_t = p4w.tile([128, 16, FFr], BF16, tag="wu_t", name="wu_t")
            nc.sync.dma_start(wu_t[:],
                              io['w_up'].rearrange("(kt p) m -> p kt m", p=128))
            wd_t = p4w.tile([128, 8, H], BF16, tag="wd_t", name="wd_t")
            nc.sync.dma_start(wd_t[:],
                              io['w_down'].rearrange("(kt p) m -> p kt m", p=128))

            def p4_prep(j):
                tok0 = j * NT
                mt = p4mt.tile([128, 16, NT], BF16, tag="mt", name="mt")
                nc.sync.dma_start(mt[:], scr['ar1_out8'][j]
                                  .rearrange("(kt p) n -> p kt n", p=128))
                ssq_t = p4row.tile([1, NT], F32, tag="ssq_t", name="ssq_t")
                nc.sync.dma_start(ssq_t[:], scr['ssq_out8'][j])
                ssq_l = p4row.tile([1, NT], F32, tag="ssq_l", name="ssq_l")
                nc.scalar.activation(ssq_l[:], ssq_t[:], AF.Ln,
                                     bias=eps1[:], scale=float(1.0 / DIN))
                s3_row = p4row.tile([1, NT], F32R, tag="s3row", name="s3row")
                nc.scalar.activation(s3_row[:], ssq_l[:], AF.Exp, scale=-0.5)
                s3b_ps = p4ps_s.tile([128, NT], F32, tag="s3bps", name="s3bps")
                mm(s3b_ps[:], ones1[:], s3_row[:], start=True, stop=True)
                s3b = p4.tile([128, NT], BF16, tag="s3b", name="s3b")
                nc.any.tensor_copy(s3b[:], s3b_ps[:])
                # s3-scale + residual add + ln2 stats
                ssq_ps = p4ps_s.tile([1, NT], F32, tag="ssq", name="ssq")
                for k in range(16):
                    ht = p4.tile([128, NT], BF16, tag="ht", name="ht")
                    nc.sync.dma_start(
                        ht[:], io['hsT'][k * 128:(k + 1) * 128, tok0:tok0 + NT])
                    nc.vector.tensor_tensor(mt[:, k, :], mt[:, k, :], s3b[:],
                                            Op.mult)
                    nc.vector.tensor_tensor(mt[:, k, :], mt[:, k, :], ht[:],
                                            Op.add)
                    nc.sync.dma_start(
                        io['resid2T'][k * 128:(k + 1) * 128, tok0:tok0 + NT],
                        mt[:, k, :])
                    sq = p4.tile([128, NT], F32R, tag="sq", name="sq")
                    nc.scalar.activation(sq[:], mt[:, k, :], AF.Square)
                    mm(ssq_ps[:], ones128[:], sq[:],
                       start=(k == 0), stop=(k == 15))
                sr0 = p4row.tile([1, NT], F32, tag="sr0", name="sr0")
                nc.scalar.activation(sr0[:], ssq_ps[:], AF.Ln,
                                     bias=eps1[:], scale=float(1.0 / H))
                s_row = p4row.tile([1, NT], F32R, tag="srow", name="srow")
                nc.scalar.activation(s_row[:], sr0[:], AF.Exp, scale=-0.5)
                sb_ps = p4ps_s.tile([128, NT], F32, tag="sbps", name="sbps")
                mm(sb_ps[:], ones1[:], s_row[:], start=True, stop=True)
                sb = p4.tile([128, NT], BF16, tag="sb", name="sb")
                nc.any.tensor_copy(sb[:], sb_ps[:])
                mtn = p4.tile([128, 16, NT], BF16, tag="mtn", name="mtn")
                for k in range(16):
                    nc.vector.scalar_tensor_tensor(
                        mtn[:, k, :], mt[:, k, :], C['ln2_c'][:, k:k + 1],
                        sb[:], Op.mult, Op.mult)
                return mtn

            mtn_cur = p4.tile([128, 16, NT], BF16, tag="mtn", name="mtn")
            nc.sync.dma_start(mtn_cur[:], scr['mtn0']
                              .rearrange("(kt p) n -> p kt n", p=128))
            for nt in range(n8):
                tok0 = nt * NT
                # gate_up + silu*up (av kept in SBUF as down-proj k-tiles)
                av = p4av.tile([128, 8, NT], BF16, tag="av", name="av")
                for mi in range(8):
                    gp = p4ps_g.tile([128, NT], F32, tag="gp", name="gp")
                    up = p4ps_g.tile([128, NT], F32, tag="up", name="up")
                    for k in range(16):
                        mm(gp[:], wg_t[:, k, mi * 128:(mi + 1) * 128],
                           mtn_cur[:, k, :], start=(k == 0), stop=(k == 15))
                    for k in range(16):
                        mm(up[:], wu_t[:, k, mi * 128:(mi + 1) * 128],
                           mtn_cur[:, k, :], start=(k == 0), stop=(k == 15))
                    sg = p4.tile([128, NT], BF16, tag="sg", name="sg")
                    silu(sg[:], gp[:], pool=p4, tag="sgsig")
                    nc.vector.tensor_tensor(av[:, mi, :], sg[:], up[:], Op.mult)
                # next tile's prep lands between the gate and down matmuls so
                # its stats/DVE chain hides under this tile's PE work
                mtn_next = p4_prep(nt + 1) if nt + 1 < n8 else None
                # down proj -> ReduceScatter chunk (host concats slices)
                for mo in range(16):
                    ps = p4ps_d.tile([128, NT], F32, tag="dps", name="dps")
                    for k in range(8):
                        mm(ps[:], wd_t[:, k, mo * 128:(mo + 1) * 128],
                           av[:, k, :], start=(k == 0), stop=(k == 7))
                    ot = p4.tile([128, NT], BF16, tag="ot4", name="ot4")
                    nc.any.tensor_copy(ot[:], ps[:])
                    nc.sync.dma_start(
                        scr['rs2_in8'][nt][mo * 128:(mo + 1) * 128, :], ot[:])
                if world > 1:
                    nc.gpsimd.collective_compute(
                        "ReduceScatter", Op.add,
                        replica_groups=[list(range(world))],
                        ins=[scr['rs2_in8'][nt]], outs=[scr['rs2_out8'][nt]])
                else:
                    nc.sync.dma_start(scr['rs2_out8'][nt],
                                      scr['rs2_in8'][nt][0:H // world, :])
                nc.sync.dma_start(io['out1T'][:, tok0:tok0 + NT],
                                  scr['rs2_out8'][nt])
                mtn_cur = mtn_next


# ================================================================ entry point
def kernel(**inputs):
    from concourse import bass_utils

    nc = build(world=TP, debug=False)
    in_maps = [shard_core_inputs(inputs, r) for r in range(TP)]
    res = bass_utils.run_bass_kernel_spmd(nc, in_maps, core_ids=list(range(TP)))
    out1T = np.concatenate(
        [np.asarray(res.results[r]['out1T'], dtype=np.float32)
         for r in range(TP)], axis=0)                # [H, T] feature-major
    out1 = np.ascontiguousarray(out1T.T).reshape(B, L, H)
    resid2 = np.ascontiguousarray(
        np.asarray(res.results[0]['resid2T'], dtype=np.float32).T
    ).reshape(B, L, H)
    return out1, resid2


if __name__ == '__main__':
    nc = build(world=1)
    print("built ok")


# revision 53
# speedup vs baseline: 1.1364x; 1.0020x over previous
"""Trainium2 Bass kernel for nn_BambaMixerDecoderLayer_84696755077458.

Tensor-parallel over 8 NeuronCores (vLLM-style), v2 (bf16):
  - in_proj / gate_up column-sharded, out_proj / down row-sharded
  - heads + conv channels sharded with d_inner; B/C conv streams replicated
  - SSM scan via chunked SSD (Mamba2): intra-chunk matmuls + small
    cross-chunk state recurrence.
  - bf16 weights/activations for all large GEMMs, scratch and collectives;
    fp32 for stats, decay rows and the SSD state.
  - Single merged in_proj pass; MLP (gate_up+down) fused in one pass.
  - Collectives chunked 8x along tokens and issued inline so they overlap
    with compute (no global barrier between SSD and MLP phases).
Everything on-device is feature-major ([feature, token]); host does layout
transforms (transpose / shard / concat) only.

Self-contained: hardcodes all shapes; needs only /opt/trn_rl_repo on sys.path.
"""
import sys
from contextlib import ExitStack

if '/opt/trn_rl_repo' not in sys.path:
    sys.path.insert(0, '/opt/trn_rl_repo')

import numpy as np

# ---------------------------------------------------------------- constants
H = 2048          # hidden
DIN = 4096        # mamba intermediate
DS = 128          # ssm state
DCONV = 4
NH = 64
HD = 64
FF = 8192
EPS = 1e-5
B, L = 2, 2048
T = B * L                         # 4096 tokens
CONV_DIM = DIN + 2 * DS           # 4352
D_IN_PROJ = 2 * DIN + 2 * DS + NH  # 8512

TP = 8
NHr = NH // TP                    # 8 heads / core
DINr = DIN // TP                  # 512
FFr = FF // TP                    # 1024
CONVr = DINr + 2 * DS             # 768 conv channels / core
MPROJ = DINr + CONVr + NHr        # 1288 in_proj cols / core

Q = 128                           # SSD chunk
NT = 512                          # token tile (also the collective chunk)
NEG = -3.0e38
SIM_SILU = False   # True: emit sigmoid+mul instead of Silu (CoreSim support)


def _f32(x):
    return np.ascontiguousarray(np.asarray(x, dtype=np.float32))


def _bf16(x):
    import ml_dtypes
    return np.ascontiguousarray(
        np.asarray(x, dtype=np.float32).astype(ml_dtypes.bfloat16))


# ================================================================ host prep
def host_constants():
    import ml_dtypes
    identb = np.eye(128, dtype=ml_dtypes.bfloat16)
    i8 = np.eye(8, dtype=np.float32)
    sel8 = np.zeros((8, 8 * 128), np.float32)
    for h in range(8):
        sel8[h, h * 128:(h + 1) * 128] = 1.0
    negselpair = np.zeros((8, 4 * 256), np.float32)
    for p in range(4):
        negselpair[2 * p, p * 256:p * 256 + 128] = -1.0
        negselpair[2 * p + 1, p * 256 + 128:p * 256 + 256] = -1.0
    ones8 = np.ones((8, 128), np.float32)
    ones1 = np.ones((1, 128), np.float32)
    ones128 = np.ones((128, 1), np.float32)
    tri = np.where(np.arange(Q)[:, None] > np.arange(Q)[None, :], NEG, 0.0)
    trimask2 = np.concatenate([tri, tri], axis=1).astype(np.float32)
    return dict(c_identb=identb, c_i8=i8, c_sel8=sel8, c_negselpair=negselpair,
                c_ones8=ones8, c_ones1=ones1, c_ones128=ones128,
                c_trimask2=trimask2)


def shard_core_inputs(inputs, r):
    """Build the per-core input map (all feature-major)."""
    w_in = _f32(inputs['w_in'])
    zs = slice(DINr * r, DINr * (r + 1))
    xs = slice(DIN + DINr * r, DIN + DINr * (r + 1))
    bs = slice(2 * DIN, 2 * DIN + DS)
    cs = slice(2 * DIN + DS, 2 * DIN + 2 * DS)
    dts = slice(2 * DIN + 2 * DS + NHr * r, 2 * DIN + 2 * DS + NHr * (r + 1))
    w_in_r = np.concatenate(
        [w_in[:, zs], w_in[:, xs], w_in[:, bs], w_in[:, cs], w_in[:, dts]], axis=1)

    conv_w = _f32(inputs['conv_w'])
    conv_w_r = np.concatenate([conv_w[DINr * r:DINr * (r + 1)], conv_w[DIN:]], axis=0)
    conv_b = _f32(inputs['conv_b'])
    conv_b_r = np.concatenate([conv_b[DINr * r:DINr * (r + 1)], conv_b[DIN:]], axis=0)

    hs = _f32(inputs['hidden_states'])
    hs = hs.reshape(-1, H)

    A_r = _f32(inputs['A_log'])[NHr * r:NHr * (r + 1)]
    dtb_r = _f32(inputs['dt_bias'])[NHr * r:NHr * (r + 1)]
    D_r = _f32(inputs['D_ssm'])[NHr * r:NHr * (r + 1)]

    m = dict(host_constants())
    m['hsT'] = _bf16(hs.T)                                      # [2048, T]
    m['w_in'] = _bf16(w_in_r)                                   # [2048, 1288]
    # per-k-tile column form of per-feature vectors: [128, n_tiles]
    m['ln1_c'] = np.ascontiguousarray(_f32(inputs['ln1_w']).reshape(16, 128).T)
    m['ln2_c'] = np.ascontiguousarray(_f32(inputs['ln2_w']).reshape(16, 128).T)
    m['normw_c'] = np.ascontiguousarray(
        _f32(inputs['norm_w'])[DINr * r:DINr * (r + 1)].reshape(4, 128).T)
    m['dssm_c'] = np.ascontiguousarray(
        np.repeat(D_r, HD).reshape(4, 128).T)                   # [128, 4]
    # conv weights: [128, 6*4] with [p, pt*4+d]
    m['conv_w'] = np.ascontiguousarray(
        conv_w_r.reshape(6, 128, DCONV).transpose(1, 0, 2).reshape(128, 6 * DCONV))
    m['conv_b'] = np.ascontiguousarray(conv_b_r.reshape(6, 128).T)  # [128, 6]
    m['a_col'] = np.ascontiguousarray((-np.exp(A_r))[:, None])   # [8,1]
    m['dtb_col'] = np.ascontiguousarray(dtb_r[:, None])          # [8,1]
    m['w_out'] = _bf16(_f32(inputs['w_out'])[DINr * r:DINr * (r + 1)])
    wgu = _f32(inputs['w_gate_up'])
    m['w_gate'] = _bf16(wgu[:, FFr * r:FFr * (r + 1)])
    m['w_up'] = _bf16(wgu[:, FF + FFr * r:FF + FFr * (r + 1)])
    m['w_down'] = _bf16(_f32(inputs['w_down'])[FFr * r:FFr * (r + 1)])
    return m


# ================================================================ the kernel
def build(world=TP, debug=False, T_=T):
    import concourse.mybir as mybir
    import concourse.tile as tile
    from concourse import bacc
    from concourse.alu_op_type import AluOpType as Op

    AF = mybir.ActivationFunctionType
    F32 = mybir.dt.float32
    BF16 = mybir.dt.bfloat16

    nc = bacc.Bacc("TRN2", target_bir_lowering=False, debug=False,
                   num_devices=world)

    F32R = mybir.dt.float32r
    n8 = T_ // NT

    def din(name, shape, dt):
        return nc.dram_tensor(name, list(shape), dt, kind="ExternalInput").ap()

    BIN = {'hsT', 'w_in', 'w_out', 'w_gate', 'w_up', 'w_down', 'c_identb'}
    RIN = {'c_i8', 'c_sel8', 'c_negselpair', 'c_ones8', 'c_ones1', 'c_ones128'}
    io = {}
    for name, shape in [
            ('hsT', (H, T_)), ('w_in', (H, MPROJ)),
            ('ln1_c', (128, 16)), ('ln2_c', (128, 16)),
            ('normw_c', (128, 4)), ('dssm_c', (128, 4)),
            ('conv_w', (128, 24)), ('conv_b', (128, 6)),
            ('a_col', (8, 1)), ('dtb_col', (8, 1)),
            ('w_out', (DINr, H)), ('w_gate', (H, FFr)), ('w_up', (H, FFr)),
            ('w_down', (FFr, H)),
            ('c_identb', (128, 128)), ('c_i8', (8, 8)), ('c_sel8', (8, 1024)),
            ('c_negselpair', (8, 1024)), ('c_ones8', (8, 128)),
            ('c_ones1', (1, 128)), ('c_ones128', (128, 1)),
            ('c_trimask2', (128, 256))]:
        dt = BF16 if name in BIN else (F32R if name in RIN else F32)
        io[name] = din(name, shape, dt)

    io['out1T'] = nc.dram_tensor("out1T", [H // world, T_], BF16,
                                 kind="ExternalOutput").ap()
    io['resid2T'] = nc.dram_tensor("resid2T", [H, T_], BF16,
                                   kind="ExternalOutput").ap()

    skind = "ExternalOutput" if debug else "Internal"
    scr = {}
    scr['z'] = nc.dram_tensor("z_s", [DINr, T_], BF16, kind=skind).ap()
    scr['x'] = nc.dram_tensor("x_s", [DINr, T_], BF16, kind=skind).ap()
    scr['b'] = nc.dram_tensor("b_s", [DS, T_], BF16, kind=skind).ap()
    scr['c'] = nc.dram_tensor("c_s", [DS, T_], BF16, kind=skind).ap()
    scr['ar1_in8'] = [
        nc.dram_tensor(f"ar1_in{q}", [H, NT], BF16, kind="Internal").ap()
        for q in range(n8)]
    scr['ar1_out8'] = [
        nc.dram_tensor(f"ar1_out{q}", [H, NT], BF16, kind="Internal",
                       addr_space="Shared").ap() for q in range(n8)]
    scr['ssq_in8'] = [
        nc.dram_tensor(f"ssq_in{q}", [1, NT], F32, kind="Internal").ap()
        for q in range(n8)]
    scr['ssq_out8'] = [
        nc.dram_tensor(f"ssq_out{q}", [1, NT], F32, kind="Internal",
                       addr_space="Shared").ap() for q in range(n8)]
    scr['rs2_in8'] = [
        nc.dram_tensor(f"rs2_in{q}", [H, NT], BF16, kind="Internal").ap()
        for q in range(n8)]
    scr['rs2_out8'] = [
        nc.dram_tensor(f"rs2_out{q}", [H // world, NT], BF16,
                       kind="Internal").ap() for q in range(n8)]
    scr['mtn0'] = nc.dram_tensor("mtn0_s", [H, NT], BF16, kind="Internal").ap()

    with tile.TileContext(nc) as tc:
        _body(tc, io, scr, world, debug, mybir, tile, Op, AF, F32, T_)

    nc.compile()
    return nc


def _body(tc, io, scr, world, debug, mybir, tile, Op, AF, F32, T_):
    nc = tc.nc
    F32R = mybir.dt.float32r
    BF16 = mybir.dt.bfloat16
    n8 = T_ // NT
    NCHUNK = T_ // Q
    CPS = (T_ // B) // Q          # chunks per sequence

    def mm(out, lhsT, rhs, start, stop, skip=False):
        if lhsT.dtype == F32:
            lhsT = lhsT.bitcast(F32R)
        if rhs.dtype == F32:
            rhs = rhs.bitcast(F32R)
        nc.tensor.matmul(out, lhsT, rhs, start=start, stop=stop,
                         skip_group_check=skip)

    def silu(out_ap, in_ap, bias=0.0, pool=None, tag="silu_tmp"):
        if SIM_SILU:
            tmp = pool.tile(list(out_ap.shape), F32, tag=tag, name=tag)
            nc.scalar.activation(tmp[:], in_ap, AF.Sigmoid, bias=bias, scale=1.0)
            if isinstance(bias, float) and bias == 0.0:
                nc.vector.tensor_tensor(out_ap, in_ap, tmp[:], Op.mult)
            else:
                raise NotImplementedError("SIM_SILU with bias AP")
        else:
            nc.scalar.activation(out_ap, in_ap, AF.Silu, bias=bias, scale=1.0)

    def allreduce(in_ap, out_ap):
        if world > 1:
            nc.gpsimd.collective_compute(
                "AllReduce", Op.add, replica_groups=[list(range(world))],
                ins=[in_ap], outs=[out_ap])
        else:
            nc.sync.dma_start(out_ap, in_ap)

    with ExitStack() as ES:
        cpool = ES.enter_context(tc.tile_pool(name="consts", bufs=1))

        # -------------------------------------------------------- constants
        C = {}
        RT = {'c_i8', 'c_sel8', 'c_negselpair', 'c_ones8', 'c_ones1',
              'c_ones128'}
        for nm, shape in [('c_identb', (128, 128)), ('c_i8', (8, 8)),
                          ('c_sel8', (8, 1024)), ('c_negselpair', (8, 1024)),
                          ('c_ones8', (8, 128)), ('c_ones1', (1, 128)),
                          ('c_ones128', (128, 1)), ('c_trimask2', (128, 256)),
                          ('ln1_c', (128, 16)), ('ln2_c', (128, 16)),
                          ('normw_c', (128, 4)), ('dssm_c', (128, 4)),
                          ('conv_w', (128, 24)), ('conv_b', (128, 6)),
                          ('a_col', (8, 1)), ('dtb_col', (8, 1))]:
            dt = BF16 if nm == 'c_identb' else (F32R if nm in RT else F32)
            t = cpool.tile(list(shape), dt, tag=nm)
            nc.sync.dma_start(t[:], io[nm])
            C[nm] = t
        identb, i8 = C['c_identb'], C['c_i8']
        sel8, negselp = C['c_sel8'], C['c_negselpair']
        ones8, ones1, ones128 = C['c_ones8'], C['c_ones1'], C['c_ones128']
        trimask2 = C['c_trimask2']

        eps1 = cpool.tile([1, 1], F32, tag="eps1", name="eps1")
        nc.vector.memset(eps1[:], float(EPS))

        # ======================================================== Phase 1
        # merged single pass over hsT: ln1 stats + z + dt + xBC + conv
        rows_a_es = ExitStack()
        rows_a = rows_a_es.enter_context(tc.tile_pool(name="rows_a", bufs=1))
        dt_rows = rows_a.tile([8, T_], F32R, tag="dt_rows", name="dt_rows")
        lA_rows = rows_a.tile([8, T_], F32R, tag="lA_rows", name="lA_rows")
        ssq_yz_row = rows_a.tile([1, T_], F32, tag="ssq_yz", name="ssq_yz")

        with tc.tile_pool(name="p1w", bufs=1) as p1w, \
             tc.tile_pool(name="p1", bufs=2) as p1, \
             tc.tile_pool(name="convp", bufs=2) as convp, \
             tc.tile_pool(name="p1ps_s", bufs=1, space="PSUM") as p1ps_s, \
             tc.tile_pool(name="p1ps_m", bufs=2, space="PSUM") as p1ps_m:

            # first token tile is prefetched BEFORE the weights so the ln1
            # stats matmuls warm up the PE while w_in streams in
            hst0 = p1.tile([128, 16, NT], BF16, tag="hst", name="hst")
            nc.sync.dma_start(hst0[:], io['hsT'][:, 0:NT]
                              .rearrange("(kt p) n -> p kt n", p=128))
            # all in_proj columns per core: [z | xBC | dt] = 1288
            w1 = p1w.tile([128, 16, MPROJ], BF16, tag="w1", name="w1")
            nc.sync.dma_start(
                w1[:], io['w_in'].rearrange("(kt p) m -> p kt m", p=128))
            for k in range(16):
                nc.vector.tensor_scalar_mul(w1[:, k, :], w1[:, k, :],
                                            C['ln1_c'][:, k:k + 1])

            halo_prev = None
            for nt in range(n8):
                tok0 = nt * NT
                seq_start = (tok0 % (T_ // B)) == 0
                if nt == 0:
                    hst = hst0
                else:
                    hst = p1.tile([128, 16, NT], BF16, tag="hst", name="hst")
                    nc.sync.dma_start(hst[:], io['hsT'][:, tok0:tok0 + NT]
                                      .rearrange("(kt p) n -> p kt n", p=128))
                # ln1 stats (ACT squares; matmuls never wait on these)
                ssq_ps = p1ps_s.tile([1, NT], F32, tag="ssq", name="ssq")
                for k in range(16):
                    sq = p1.tile([128, NT], F32R, tag="sq", name="sq")
                    nc.scalar.activation(sq[:], hst[:, k, :], AF.Square)
                    mm(ssq_ps[:], ones128[:], sq[:],
                       start=(k == 0), stop=(k == 15))
                sr0 = p1.tile([1, NT], F32, tag="sr0", name="sr0", bufs=1)
                nc.scalar.activation(sr0[:], ssq_ps[:], AF.Ln,
                                     bias=eps1[:], scale=float(1.0 / H))
                s_row = p1.tile([1, NT], F32R, tag="s_row", name="s_row",
                                bufs=1)
                nc.scalar.activation(s_row[:], sr0[:], AF.Exp, scale=-0.5)
                sb_ps = p1ps_s.tile([128, NT], F32, tag="sbps", name="sbps")
                mm(sb_ps[:], ones1[:], s_row[:], start=True, stop=True)
                sb = p1.tile([128, NT], F32, tag="sb", name="sb")
                nc.any.tensor_copy(sb[:], sb_ps[:])
                # z m-tiles: matmul on RAW hidden, scale on the way out
                for mi in range(4):
                    ps = p1ps_m.tile([128, NT], F32, tag="mt", name="mt")
                    for k in range(16):
                        mm(ps[:], w1[:, k, mi * 128:(mi + 1) * 128],
                           hst[:, k, :], start=(k == 0), stop=(k == 15))
                    zt = p1.tile([128, NT], BF16, tag="z", name="z")
                    nc.vector.tensor_tensor(zt[:], ps[:], sb[:], Op.mult)
                    nc.sync.dma_start(
                        scr['z'][mi * 128:(mi + 1) * 128, tok0:tok0 + NT], zt[:])
                # dt m-tile (8 wide)
                dtp = p1ps_s.tile([8, NT], F32, tag="mtdt", name="mtdt")
                for k in range(16):
                    mm(dtp[:], w1[:, k, DINr + CONVr:MPROJ], hst[:, k, :],
                       start=(k == 0), stop=(k == 15))
                dt_raw = p1.tile([8, NT], F32, tag="dtraw", name="dtraw",
                                 bufs=1)
                nc.vector.tensor_tensor(dt_raw[:], dtp[:], sb[:8, :], Op.mult)
                e8 = p1.tile([8, NT], F32, tag="e8", name="e8", bufs=1)
                nc.scalar.activation(e8[:], dt_raw[:], AF.Exp,
                                     bias=C['dtb_col'][:], scale=1.0)
                e8p = p1.tile([8, NT], F32, tag="e8p", name="e8p", bufs=1)
                nc.vector.tensor_scalar_add(e8p[:], e8[:], 1.0)
                nc.scalar.activation(dt_rows[:, tok0:tok0 + NT], e8p[:], AF.Ln)
                logda = p1.tile([8, NT], F32, tag="logda", name="logda",
                                bufs=1)
                nc.vector.tensor_scalar_mul(logda[:], dt_rows[:, tok0:tok0 + NT],
                                            C['a_col'][:])
                for c in range(NT // Q):
                    nc.vector.tensor_tensor_scan(
                        lA_rows[:, tok0 + c * Q:tok0 + (c + 1) * Q],
                        ones8[:, :Q].bitcast(F32), logda[:, c * Q:(c + 1) * Q],
                        0.0, Op.mult, Op.add)

                # xBC m-tiles + causal conv
                halo = [convp.tile([128, NT + 3], BF16, tag=f"halo{pt}",
                                   name=f"halo{pt}") for pt in range(6)]
                for pt in range(6):
                    ps = p1ps_m.tile([128, NT], F32, tag="mt", name="mt")
                    for k in range(16):
                        mm(ps[:], w1[:, k, DINr + pt * 128:DINr + (pt + 1) * 128],
                           hst[:, k, :], start=(k == 0), stop=(k == 15))
                    nc.vector.tensor_tensor(halo[pt][:, 3:3 + NT], ps[:], sb[:],
                                            Op.mult)
                for pt in range(6):
                    if seq_start:
                        nc.vector.memset(halo[pt][:, 0:3], 0.0)
                    else:
                        nc.vector.tensor_copy(halo[pt][:, 0:3],
                                              halo_prev[pt][:, NT:NT + 3])
                    acc = convp.tile([128, NT], BF16, tag="cacc", name="cacc")
                    nc.vector.tensor_scalar_mul(
                        acc[:], halo[pt][:, 0:NT],
                        C['conv_w'][:, pt * 4:pt * 4 + 1])
                    for d in range(1, 4):
                        nc.vector.scalar_tensor_tensor(
                            acc[:], halo[pt][:, d:d + NT],
                            C['conv_w'][:, pt * 4 + d:pt * 4 + d + 1],
                            acc[:], Op.mult, Op.add)
                    cact = convp.tile([128, NT], BF16, tag="cact", name="cact")
                    if SIM_SILU:
                        nc.vector.tensor_scalar_add(acc[:], acc[:],
                                                    C['conv_b'][:, pt:pt + 1])
                        silu(cact[:], acc[:], pool=convp, tag="cvsig")
                    else:
                        nc.scalar.activation(cact[:], acc[:], AF.Silu,
                                             bias=C['conv_b'][:, pt:pt + 1],
                                             scale=1.0)
                    if pt < 4:
                        nc.sync.dma_start(
                            scr['x'][pt * 128:(pt + 1) * 128, tok0:tok0 + NT],
                            cact[:])
                    elif pt == 4:
                        nc.sync.dma_start(scr['b'][:, tok0:tok0 + NT], cact[:])
                    else:
                        nc.sync.dma_start(scr['c'][:, tok0:tok0 + NT], cact[:])
                halo_prev = halo

        # ============================================ Phase 2: SSD + gated
        # norm + out_proj, fused per token-tile. out_proj runs on UNSCALED
        # yz — the rms scale s3 commutes through the matmul and the
        # AllReduce, and is applied in Phase 4. AR chunks issued inline.
        with tc.tile_pool(name="p2", bufs=3) as p2, \
             tc.tile_pool(name="p2s", bufs=2) as p2s, \
             tc.tile_pool(name="state", bufs=1) as spool, \
             tc.tile_pool(name="p2pre", bufs=1) as p2pre, \
             tc.tile_pool(name="p3f", bufs=2) as p3f, \
             tc.tile_pool(name="p3w", bufs=1) as p3w, \
             tc.tile_pool(name="p2ps1", bufs=1, space="PSUM") as p2ps1, \
             tc.tile_pool(name="p2ps2", bufs=2, space="PSUM") as p2ps2, \
             tc.tile_pool(name="p3ps", bufs=2, space="PSUM") as p3ps:

            w_out_t = p3w.tile([128, 4, H], BF16, tag="w_out_t", name="w_out_t")
            nc.sync.dma_start(w_out_t[:],
                              io['w_out'].rearrange("(kt p) m -> p kt m", p=128))
            for k in range(4):
                nc.vector.tensor_scalar_mul(w_out_t[:, k, :], w_out_t[:, k, :],
                                            C['normw_c'][:, k:k + 1])


            S_all = spool.tile([128, NHr * HD], F32R, tag="S_all", name="S_all")
            nc.vector.memset(S_all[:].bitcast(F32), 0.0)

            for nt in range(n8):
                y_sb = p3f.tile([128, 4, NT], BF16, tag="ysb", name="ysb")
                for cc_ in range(NT // Q):
                    ch = nt * (NT // Q) + cc_
                    t0 = ch * Q
                    xf = p2.tile([128, 4, Q], BF16, tag="xf", name="xf")
                    nc.sync.dma_start(xf[:], scr['x'][:, t0:t0 + Q]
                                      .rearrange("(pt p) n -> p pt n", p=128))
                    bf = p2.tile([128, Q], BF16, tag="bf", name="bf")
                    nc.sync.dma_start(bf[:], scr['b'][:, t0:t0 + Q])
                    cf = p2.tile([128, Q], BF16, tag="cf", name="cf")
                    nc.sync.dma_start(cf[:], scr['c'][:, t0:t0 + Q])

                    lrow = lA_rows[:, t0:t0 + Q]
                    dtrow = dt_rows[:, t0:t0 + Q]

                    expl = p2s.tile([8, Q], F32R, tag="expl", name="expl")
                    nc.scalar.activation(expl[:], lrow, AF.Exp)
                    ddr0 = p2s.tile([8, Q], F32, tag="ddr0", name="ddr0")
                    nc.vector.tensor_scalar(ddr0[:], lrow, -1.0,
                                            lrow[:, Q - 1:Q].bitcast(F32),
                                            Op.mult, Op.add)
                    dd_rows = p2s.tile([8, Q], F32R, tag="ddrows", name="ddrows")
                    nc.scalar.activation(dd_rows[:], ddr0[:], AF.Exp)
                    nc.vector.tensor_tensor(dd_rows[:], dd_rows[:], dtrow,
                                            Op.mult)
                    dg = p2s.tile([8, 8], F32R, tag="dg", name="dg")
                    nc.vector.tensor_scalar_mul(dg[:], i8[:],
                                                expl[:, Q - 1:Q].bitcast(F32))

                    misc = p2ps1.tile([128, 160], F32, tag="misc", name="misc")
                    g_ps = misc[:, 0:128]
                    ddcol_ps = misc[:, 128:136]
                    decay_ps = misc[:, 136:144]
                    dtcol_ps = misc[:, 144:152]

                    mm(g_ps, bf[:], cf[:], start=True, stop=True)
                    mm(ddcol_ps, dd_rows[:], i8[:], start=True, stop=True)
                    mm(decay_ps, ones8[:], dg[:], start=True, stop=True)
                    mm(dtcol_ps, dtrow, i8[:], start=True, stop=True)
                    g2 = p2s.tile([128, 256], F32R, tag="g2", name="g2")
                    nc.any.tensor_copy(g2[:, 0:128], g_ps)
                    nc.any.tensor_copy(g2[:, 128:256], g_ps)
                    cf2 = p2s.tile([128, 256], BF16, tag="cf2", name="cf2")
                    nc.vector.tensor_copy(cf2[:, 0:128], cf[:])
                    nc.vector.tensor_copy(cf2[:, 128:256], cf[:])
                    # late-read scalars leave PSUM early so misc can
                    # single-buffer without serializing chunks
                    dsc = p2s.tile([128, 16], F32, tag="dsc", name="dsc")
                    nc.any.tensor_copy(dsc[:], misc[:, 136:152])
                    decay_sb = dsc[:, 0:8]
                    dtcol_sb = dsc[:, 8:16]

                    tps = p2ps1.tile([128, 5, 128], BF16, tag="xtm", name="xtm")
                    nc.tensor.transpose(tps[:, 4, :], bf[:], identb[:])
                    btm = p2s.tile([128, Q], BF16, tag="btm", name="btm")
                    nc.any.tensor_copy(btm[:], tps[:, 4, :])

                    for pt in range(4):
                        nc.tensor.transpose(tps[:, pt, :],
                                            xf[:, pt, :], identb[:])
                    xtm = p2s.tile([128, NHr * HD], BF16, tag="xtm_sb",
                                   name="xtm_sb")
                    nc.any.tensor_copy(xtm[:], tps[:, 0:4, :])
                    xw = p2s.tile([128, NHr * HD], BF16, tag="xw", name="xw")
                    for h in range(NHr):
                        nc.vector.tensor_scalar_mul(
                            xw[:, h * HD:(h + 1) * HD],
                            xtm[:, h * HD:(h + 1) * HD], ddcol_ps[:, h:h + 1])

                    def y_readout(yp, pt):
                        base = (pt % 2) * 256
                        ysl0 = y_sb[0:64, pt, cc_ * Q:(cc_ + 1) * Q]
                        ysl1 = y_sb[64:128, pt, cc_ * Q:(cc_ + 1) * Q]
                        nc.vector.scalar_tensor_tensor(
                            ysl0, xf[0:64, pt, :], C['dssm_c'][0:64, pt:pt + 1],
                            yp[0:64, base:base + 128], Op.mult, Op.add)
                        nc.vector.scalar_tensor_tensor(
                            ysl1, xf[64:128, pt, :],
                            C['dssm_c'][64:128, pt:pt + 1],
                            yp[0:64, base + 128:base + 256], Op.mult, Op.add)

                    y_cur = None
                    for pr in range(4):
                        h0, h1 = 2 * pr, 2 * pr + 1
                        if pr % 2 == 0:
                            y_cur = p2ps1.tile([64, 512], F32, tag="y",
                                               name="y")
                        pairps = p2ps2.tile([128, 512], F32, tag="pairps",
                                            name="pairps")
                        dpair = pairps[:, 0:256]
                        d2 = pairps[:, 256:512]
                        for i, h in enumerate((h0, h1)):
                            half = dpair[:, i * 128:(i + 1) * 128]
                            mm(half, sel8[:, h * 128:(h + 1) * 128], lrow,
                               start=True, stop=False)
                            mm(half, lrow,
                               negselp[:, pr * 256 + i * 128:
                                       pr * 256 + (i + 1) * 128],
                               start=False, stop=True)
                        dmask = p2s.tile([128, 256], F32, tag="dmask",
                                         name="dmask")
                        nc.vector.tensor_tensor(dmask[:], dpair, trimask2[:],
                                                Op.add)
                        w0 = p2s.tile([128, 256], F32, tag="w0", name="w0")
                        nc.scalar.activation(w0[:], dmask[:], AF.Exp)
                        mm(d2[:, 0:128], sel8[:, h0 * 128:(h0 + 1) * 128],
                           expl[:], start=True, stop=True)
                        mm(d2[:, 128:256], sel8[:, h1 * 128:(h1 + 1) * 128],
                           expl[:], start=True, stop=True)
                        wt = p2s.tile([128, 256], BF16, tag="wt", name="wt")
                        for i, h in enumerate((h0, h1)):
                            nc.vector.scalar_tensor_tensor(
                                wt[:, i * 128:(i + 1) * 128],
                                w0[:, i * 128:(i + 1) * 128],
                                dtcol_sb[:, h:h + 1],
                                g2[:, i * 128:(i + 1) * 128],
                                Op.mult, Op.mult)
                        ce = p2s.tile([128, 256], F32R, tag="ce", name="ce")
                        nc.vector.tensor_tensor(ce[:], d2, cf2[:], Op.mult)
                        for i, h in enumerate((h0, h1)):
                            ysl = y_cur[:, (h % 4) * 128:(h % 4 + 1) * 128]
                            mm(ysl, xtm[:, h * HD:(h + 1) * HD],
                               wt[:, i * 128:(i + 1) * 128],
                               start=True, stop=False)
                            mm(ysl, S_all[:, h * HD:(h + 1) * HD],
                               ce[:, i * 128:(i + 1) * 128],
                               start=False, stop=True)
                        if pr % 2 == 1:
                            y_readout(y_cur, pr // 2 * 2)
                            y_readout(y_cur, pr // 2 * 2 + 1)

                    tp_ps = p2ps1.tile([128, 512], F32, tag="tp", name="tp")
                    mm(tp_ps[:], btm[:], xw[:], start=True, stop=True)
                    for h in range(NHr):
                        nc.vector.scalar_tensor_tensor(
                            S_all[:, h * HD:(h + 1) * HD],
                            S_all[:, h * HD:(h + 1) * HD],
                            decay_sb[:, h:h + 1], tp_ps[:, h * HD:(h + 1) * HD],
                            Op.mult, Op.add)

                    if (ch + 1) % CPS == 0 and ch + 1 < NCHUNK:
                        nc.vector.memset(S_all[:].bitcast(F32), 0.0)

                # gated product + stats + out_proj for this token tile
                tok0 = nt * NT
                zt = p3f.tile([128, 4, NT], BF16, tag="zt", name="zt")
                nc.sync.dma_start(zt[:], scr['z'][:, tok0:tok0 + NT]
                                  .rearrange("(pt p) n -> p pt n", p=128))
                yz_all = p3f.tile([128, 4, NT], BF16, tag="yzall", name="yzall")
                ssq_full = p3ps.tile([128, NT], F32, tag="mt3", name="mt3ssq")
                ssq_ps = ssq_full[0:1, :]
                for pt in range(4):
                    sz = p3f.tile([128, NT], BF16, tag="sz", name="sz")
                    silu(sz[:], zt[:, pt, :], pool=p3f, tag="szsig")
                    nc.vector.tensor_tensor(yz_all[:, pt, :], y_sb[:, pt, :],
                                            sz[:], Op.mult)
                    sqz = p3f.tile([128, NT], F32R, tag="sqz", name="sqz")
                    nc.scalar.activation(sqz[:], yz_all[:, pt, :], AF.Square)
                    mm(ssq_ps[:], ones128[:], sqz[:],
                       start=(pt == 0), stop=(pt == 3))
                nc.any.tensor_copy(ssq_yz_row[:, tok0:tok0 + NT], ssq_ps[:])

                for mi in range(16):
                    ps = p3ps.tile([128, NT], F32, tag="mt3", name="mt3")
                    for k in range(4):
                        mm(ps[:], w_out_t[:, k, mi * 128:(mi + 1) * 128],
                           yz_all[:, k, :], start=(k == 0), stop=(k == 3))
                    ot = p3f.tile([128, NT], BF16, tag="ot", name="ot")
                    nc.any.tensor_copy(ot[:], ps[:])
                    nc.sync.dma_start(
                        scr['ar1_in8'][nt][mi * 128:(mi + 1) * 128, :], ot[:])

                # inline chunked collectives: tiny stats AR then the big AR
                nc.sync.dma_start(scr['ssq_in8'][nt],
                                  ssq_yz_row[:, tok0:tok0 + NT])
                allreduce(scr['ssq_in8'][nt], scr['ssq_out8'][nt])
                allreduce(scr['ar1_in8'][nt], scr['ar1_out8'][nt])

                if nt == max(0, n8 - 3):
                    # Phase-4 prep for tile 0, overlapped with the P2 tail.
                    # Result (mtn for tile 0) is bounced via DRAM.
                    mt0 = p2pre.tile([128, 16, NT], BF16, tag="mt0",
                                     name="mt0")
                    nc.sync.dma_start(mt0[:], scr['ar1_out8'][0]
                                      .rearrange("(kt p) n -> p kt n", p=128))
                    sqt0 = p2s.tile([1, NT], F32, tag="sqt0", name="sqt0")
                    nc.sync.dma_start(sqt0[:], scr['ssq_out8'][0])
                    sql0 = p2s.tile([1, NT], F32, tag="sql0", name="sql0")
                    nc.scalar.activation(sql0[:], sqt0[:], AF.Ln,
                                         bias=eps1[:], scale=float(1.0 / DIN))
                    s3r0 = p2s.tile([1, NT], F32R, tag="s3r0", name="s3r0")
                    nc.scalar.activation(s3r0[:], sql0[:], AF.Exp, scale=-0.5)
                    bps = p3ps.tile([128, NT], F32, tag="mt3", name="mt3s3b")
                    mm(bps[:], ones1[:], s3r0[:], start=True, stop=True)
                    s3b0 = p2pre.tile([128, NT], BF16, tag="s3b0", name="s3b0")
                    nc.any.tensor_copy(s3b0[:], bps[:])
                    sqf = p3ps.tile([128, NT], F32, tag="mt3", name="mt3ssq0")
                    for k in range(16):
                        ht0 = p2.tile([128, NT], BF16, tag="ht0", name="ht0")
                        nc.sync.dma_start(ht0[:],
                                          io['hsT'][k * 128:(k + 1) * 128,
                                                    0:NT])
                        nc.vector.tensor_tensor(mt0[:, k, :], mt0[:, k, :],
                                                s3b0[:], Op.mult)
                        nc.vector.tensor_tensor(mt0[:, k, :], mt0[:, k, :],
                                                ht0[:], Op.add)
                        nc.sync.dma_start(
                            io['resid2T'][k * 128:(k + 1) * 128, 0:NT],
                            mt0[:, k, :])
                        sq0 = p2s.tile([128, NT], F32R, tag="sq0", name="sq0")
                        nc.scalar.activation(sq0[:], mt0[:, k, :], AF.Square)
                        mm(sqf[0:1, :], ones128[:], sq0[:],
                           start=(k == 0), stop=(k == 15))
                    slr0 = p2s.tile([1, NT], F32, tag="slr0", name="slr0")
                    nc.scalar.activation(slr0[:], sqf[0:1, :], AF.Ln,
                                         bias=eps1[:], scale=float(1.0 / H))
                    sr_0 = p2s.tile([1, NT], F32R, tag="sr_0", name="sr_0")
                    nc.scalar.activation(sr_0[:], slr0[:], AF.Exp, scale=-0.5)
                    sbp0 = p3ps.tile([128, NT], F32, tag="mt3", name="mt3sb0")
                    mm(sbp0[:], ones1[:], sr_0[:], start=True, stop=True)
                    sb0 = p2pre.tile([128, NT], BF16, tag="sb0", name="sb0")
                    nc.any.tensor_copy(sb0[:], sbp0[:])
                    mtn0 = p2pre.tile([128, 16, NT], BF16, tag="mtn0",
                                      name="mtn0")
                    for k in range(16):
                        nc.vector.scalar_tensor_tensor(
                            mtn0[:, k, :], mt0[:, k, :], C['ln2_c'][:, k:k + 1],
                            sb0[:], Op.mult, Op.mult)
                    nc.sync.dma_start(
                        scr['mtn0'].rearrange("(kt p) n -> p kt n", p=128),
                        mtn0[:])

        rows_a_es.close()

        # ================================= Phase 4: resid + ln2 + MLP + RS
        # Software-pipelined: tile j+1's resid/stats/mtn prep is emitted
        # between tile j's gate_up and down matmuls; tile 0's prep ran in
        # Phase 2 (bounced via scr['mtn0']).
        with tc.tile_pool(name="p4w", bufs=1) as p4w, \
             tc.tile_pool(name="p4", bufs=2) as p4, \
             tc.tile_pool(name="p4mt", bufs=1) as p4mt, \
             tc.tile_pool(name="p4row", bufs=1) as p4row, \
             tc.tile_pool(name="p4av", bufs=1) as p4av, \
             tc.tile_pool(name="p4ps_s", bufs=1, space="PSUM") as p4ps_s, \
             tc.tile_pool(name="p4ps_g", bufs=1, space="PSUM") as p4ps_g, \
             tc.tile_pool(name="p4ps_d", bufs=2, space="PSUM") as p4ps_d:
            # tile 0's mtn (precomputed in Phase 2) is fetched before the
            # weights so the first gate matmul isn't stuck behind 12MB of
            # weight DMA on the queue
            mtn_cur = p4.tile([128, 16, NT], BF16, tag="mtn", name="mtn")
            nc.sync.dma_start(mtn_cur[:], scr['mtn0']
                              .rearrange("(kt p) n -> p kt n", p=128))
            wg_t = p4w.tile([128, 16, FFr], BF16, tag="wg_t", name="wg_t")
            nc.sync.dma_start(wg_t[:],
                              io['w_gate'].rearrange("(kt p) m -> p kt m", p=128))
            wu_t = p4w.tile([128, 16, FFr], BF16, tag="wu_t", name="wu_t")
            nc.sync.dma_start(wu_t[:],
                              io['w_up'].rearrange("(kt p) m -> p kt m", p=128))
            wd_t = p4w.tile([128, 8, H], BF16, tag="wd_t", name="wd_t")
            nc.sync.dma_start(wd_t[:],
                              io['w_down'].rearrange("(kt p) m -> p kt m", p=128))

            def p4_prep(j):
                tok0 = j * NT
                mt = p4mt.tile([128, 16, NT], BF16, tag="mt", name="mt")
                nc.sync.dma_start(mt[:], scr['ar1_out8'][j]
                                  .rearrange("(kt p) n -> p kt n", p=128))
                ssq_t = p4row.tile([1, NT], F32, tag="ssq_t", name="ssq_t")
                nc.sync.dma_start(ssq_t[:], scr['ssq_out8'][j])
                ssq_l = p4row.tile([1, NT], F32, tag="ssq_l", name="ssq_l")
                nc.scalar.activation(ssq_l[:], ssq_t[:], AF.Ln,
                                     bias=eps1[:], scale=float(1.0 / DIN))
                s3_row = p4row.tile([1, NT], F32R, tag="s3row", name="s3row")
                nc.scalar.activation(s3_row[:], ssq_l[:], AF.Exp, scale=-0.5)
                s3b_ps = p4ps_s.tile([128, NT], F32, tag="s3bps", name="s3bps")
                mm(s3b_ps[:], ones1[:], s3_row[:], start=True, stop=True)
                s3b = p4.tile([128, NT], BF16, tag="s3b", name="s3b")
                nc.any.tensor_copy(s3b[:], s3b_ps[:])
                # s3-scale + residual add + ln2 stats
                ssq_ps = p4ps_s.tile([1, NT], F32, tag="ssq", name="ssq")
                for k in range(16):
                    ht = p4.tile([128, NT], BF16, tag="ht", name="ht")
                    nc.sync.dma_start(
                        ht[:], io['hsT'][k * 128:(k + 1) * 128, tok0:tok0 + NT])
                    nc.vector.tensor_tensor(mt[:, k, :], mt[:, k, :], s3b[:],
                                            Op.mult)
                    nc.vector.tensor_tensor(mt[:, k, :], mt[:, k, :], ht[:],
                                            Op.add)
                    nc.sync.dma_start(
                        io['resid2T'][k * 128:(k + 1) * 128, tok0:tok0 + NT],
                        mt[:, k, :])
                    sq = p4.tile([128, NT], F32R, tag="sq", name="sq")
                    nc.scalar.activation(sq[:], mt[:, k, :], AF.Square)
                    mm(ssq_ps[:], ones128[:], sq[:],
                       start=(k == 0), stop=(k == 15))
                sr0 = p4row.tile([1, NT], F32, tag="sr0", name="sr0")
                nc.scalar.activation(sr0[:], ssq_ps[:], AF.Ln,
                                     bias=eps1[:], scale=float(1.0 / H))
                s_row = p4row.tile([1, NT], F32R, tag="srow", name="srow")
                nc.scalar.activation(s_row[:], sr0[:], AF.Exp, scale=-0.5)
                sb_ps = p4ps_s.tile([128, NT], F32, tag="sbps", name="sbps")
                mm(sb_ps[:], ones1[:], s_row[:], start=True, stop=True)
                sb = p4.tile([128, NT], BF16, tag="sb", name="sb")
                nc.any.tensor_copy(sb[:], sb_ps[:])
                mtn = p4.tile([128, 16, NT], BF16, tag="mtn", name="mtn")
                for k in range(16):
                    nc.vector.scalar_tensor_tensor(
                        mtn[:, k, :], mt[:, k, :], C['ln2_c'][:, k:k + 1],
                        sb[:], Op.mult, Op.mult)
                return mtn

            for nt in range(n8):
                tok0 = nt * NT
                # gate_up + silu*up (av kept in SBUF as down-proj k-tiles)
                av = p4av.tile([128, 8, NT], BF16, tag="av", name="av")
                for mi in range(8):
                    gp = p4ps_g.tile([128, NT], F32, tag="gp", name="gp")
                    up = p4ps_g.tile([128, NT], F32, tag="up", name="up")
                    for k in range(16):
                        mm(gp[:], wg_t[:, k, mi * 128:(mi + 1) * 128],
                           mtn_cur[:, k, :], start=(k == 0), stop=(k == 15))
                    for k in range(16):
                        mm(up[:], wu_t[:, k, mi * 128:(mi + 1) * 128],
                           mtn_cur[:, k, :], start=(k == 0), stop=(k == 15))
                    sg = p4.tile([128, NT], BF16, tag="sg", name="sg")
                    silu(sg[:], gp[:], pool=p4, tag="sgsig")
                    nc.vector.tensor_tensor(av[:, mi, :], sg[:], up[:], Op.mult)
                # next tile's prep lands between the gate and down matmuls so
                # its stats/DVE chain hides under this tile's PE work
                mtn_next = p4_prep(nt + 1) if nt + 1 < n8 else None
                # down proj -> ReduceScatter chunk (host concats slices)
                for mo in range(16):
                    ps = p4ps_d.tile([128, NT], F32, tag="dps", name="dps")
                    for k in range(8):
                        mm(ps[:], wd_t[:, k, mo * 128:(mo + 1) * 128],
                           av[:, k, :], start=(k == 0), stop=(k == 7))
                    ot = p4.tile([128, NT], BF16, tag="ot4", name="ot4")
                    nc.any.tensor_copy(ot[:], ps[:])
                    nc.sync.dma_start(
                        scr['rs2_in8'][nt][mo * 128:(mo + 1) * 128, :], ot[:])
                if world > 1:
                    nc.gpsimd.collective_compute(
                        "ReduceScatter", Op.add,
                        replica_groups=[list(range(world))],
                        ins=[scr['rs2_in8'][nt]], outs=[scr['rs2_out8'][nt]])
                else:
                    nc.sync.dma_start(scr['rs2_out8'][nt],
                                      scr['rs2_in8'][nt][0:H // world, :])
                nc.sync.dma_start(io['out1T'][:, tok0:tok0 + NT],
                                  scr['rs2_out8'][nt])
                mtn_cur = mtn_next


# ================================================================ entry point
def kernel(**inputs):
    from concourse import bass_utils

    nc = build(world=TP, debug=False)
    in_maps = [shard_core_inputs(inputs, r) for r in range(TP)]
    res = bass_utils.run_bass_kernel_spmd(nc, in_maps, core_ids=list(range(TP)))
    out1T = np.concatenate(
        [np.asarray(res.results[r]['out1T'], dtype=np.float32)
         for r in range(TP)], axis=0)                # [H, T] feature-major
    out1 = np.ascontiguousarray(out1T.T).reshape(B, L, H)
    resid2 = np.ascontiguousarray(
        np.asarray(res.results[0]['resid2T'], dtype=np.float32).T
    ).reshape(B, L, H)
    return out1, resid2


if __name__ == '__main__':
    nc = build(world=1)
    print("built ok")


# revision 56
# speedup vs baseline: 1.1559x; 1.0172x over previous
"""Trainium2 Bass kernel for nn_BambaMixerDecoderLayer_84696755077458.

Tensor-parallel over 8 NeuronCores (vLLM-style), v2 (bf16):
  - in_proj / gate_up column-sharded, out_proj / down row-sharded
  - heads + conv channels sharded with d_inner; B/C conv streams replicated
  - SSM scan via chunked SSD (Mamba2): intra-chunk matmuls + small
    cross-chunk state recurrence.
  - bf16 weights/activations for all large GEMMs, scratch and collectives;
    fp32 for stats, decay rows and the SSD state.
  - Single merged in_proj pass; MLP (gate_up+down) fused in one pass.
  - Collectives chunked 8x along tokens and issued inline so they overlap
    with compute (no global barrier between SSD and MLP phases).
Everything on-device is feature-major ([feature, token]); host does layout
transforms (transpose / shard / concat) only.

Self-contained: hardcodes all shapes; needs only /opt/trn_rl_repo on sys.path.
"""
import sys
from contextlib import ExitStack

if '/opt/trn_rl_repo' not in sys.path:
    sys.path.insert(0, '/opt/trn_rl_repo')

import numpy as np

# ---------------------------------------------------------------- constants
H = 2048          # hidden
DIN = 4096        # mamba intermediate
DS = 128          # ssm state
DCONV = 4
NH = 64
HD = 64
FF = 8192
EPS = 1e-5
B, L = 2, 2048
T = B * L                         # 4096 tokens
CONV_DIM = DIN + 2 * DS           # 4352
D_IN_PROJ = 2 * DIN + 2 * DS + NH  # 8512

TP = 8
NHr = NH // TP                    # 8 heads / core
DINr = DIN // TP                  # 512
FFr = FF // TP                    # 1024
CONVr = DINr + 2 * DS             # 768 conv channels / core
MPROJ = DINr + CONVr + NHr        # 1288 in_proj cols / core

Q = 128                           # SSD chunk
NT = 512                          # token tile (also the collective chunk)
NEG = -3.0e38
SIM_SILU = False   # True: emit sigmoid+mul instead of Silu (CoreSim support)


def _f32(x):
    return np.ascontiguousarray(np.asarray(x, dtype=np.float32))


def _bf16(x):
    import ml_dtypes
    return np.ascontiguousarray(
        np.asarray(x, dtype=np.float32).astype(ml_dtypes.bfloat16))


# ================================================================ host prep
def host_constants():
    import ml_dtypes
    identb = np.eye(128, dtype=ml_dtypes.bfloat16)
    i8 = np.eye(8, dtype=np.float32)
    sel8 = np.zeros((8, 8 * 128), np.float32)
    for h in range(8):
        sel8[h, h * 128:(h + 1) * 128] = 1.0
    negselpair = np.zeros((8, 4 * 256), np.float32)
    for p in range(4):
        negselpair[2 * p, p * 256:p * 256 + 128] = -1.0
        negselpair[2 * p + 1, p * 256 + 128:p * 256 + 256] = -1.0
    ones8 = np.ones((8, 128), np.float32)
    ones1 = np.ones((1, 128), np.float32)
    ones128 = np.ones((128, 1), np.float32)
    tri = np.where(np.arange(Q)[:, None] > np.arange(Q)[None, :], NEG, 0.0)
    trimask2 = np.concatenate([tri, tri], axis=1).astype(np.float32)
    return dict(c_identb=identb, c_i8=i8, c_sel8=sel8, c_negselpair=negselpair,
                c_ones8=ones8, c_ones1=ones1, c_ones128=ones128,
                c_trimask2=trimask2)


def shard_core_inputs(inputs, r):
    """Build the per-core input map (all feature-major)."""
    w_in = _f32(inputs['w_in'])
    zs = slice(DINr * r, DINr * (r + 1))
    xs = slice(DIN + DINr * r, DIN + DINr * (r + 1))
    bs = slice(2 * DIN, 2 * DIN + DS)
    cs = slice(2 * DIN + DS, 2 * DIN + 2 * DS)
    dts = slice(2 * DIN + 2 * DS + NHr * r, 2 * DIN + 2 * DS + NHr * (r + 1))
    w_in_r = np.concatenate(
        [w_in[:, zs], w_in[:, xs], w_in[:, bs], w_in[:, cs], w_in[:, dts]], axis=1)

    conv_w = _f32(inputs['conv_w'])
    conv_w_r = np.concatenate([conv_w[DINr * r:DINr * (r + 1)], conv_w[DIN:]], axis=0)
    conv_b = _f32(inputs['conv_b'])
    conv_b_r = np.concatenate([conv_b[DINr * r:DINr * (r + 1)], conv_b[DIN:]], axis=0)

    hs = _f32(inputs['hidden_states'])
    hs = hs.reshape(-1, H)

    A_r = _f32(inputs['A_log'])[NHr * r:NHr * (r + 1)]
    dtb_r = _f32(inputs['dt_bias'])[NHr * r:NHr * (r + 1)]
    D_r = _f32(inputs['D_ssm'])[NHr * r:NHr * (r + 1)]

    m = dict(host_constants())
    m['hsT'] = _bf16(hs.T)                                      # [2048, T]
    m['w_in'] = _bf16(w_in_r)                                   # [2048, 1288]
    # per-k-tile column form of per-feature vectors: [128, n_tiles]
    m['ln1_c'] = np.ascontiguousarray(_f32(inputs['ln1_w']).reshape(16, 128).T)
    m['ln2_c'] = np.ascontiguousarray(_f32(inputs['ln2_w']).reshape(16, 128).T)
    m['normw_c'] = np.ascontiguousarray(
        _f32(inputs['norm_w'])[DINr * r:DINr * (r + 1)].reshape(4, 128).T)
    m['dssm_c'] = np.ascontiguousarray(
        np.repeat(D_r, HD).reshape(4, 128).T)                   # [128, 4]
    # conv weights: [128, 6*4] with [p, pt*4+d]
    m['conv_w'] = np.ascontiguousarray(
        conv_w_r.reshape(6, 128, DCONV).transpose(1, 0, 2).reshape(128, 6 * DCONV))
    m['conv_b'] = np.ascontiguousarray(conv_b_r.reshape(6, 128).T)  # [128, 6]
    m['a_col'] = np.ascontiguousarray((-np.exp(A_r))[:, None])   # [8,1]
    m['dtb_col'] = np.ascontiguousarray(dtb_r[:, None])          # [8,1]
    m['w_out'] = _bf16(_f32(inputs['w_out'])[DINr * r:DINr * (r + 1)])
    wgu = _f32(inputs['w_gate_up'])
    m['w_gate'] = _bf16(wgu[:, FFr * r:FFr * (r + 1)])
    m['w_up'] = _bf16(wgu[:, FF + FFr * r:FF + FFr * (r + 1)])
    m['w_down'] = _bf16(_f32(inputs['w_down'])[FFr * r:FFr * (r + 1)])
    return m


# ================================================================ the kernel
def build(world=TP, debug=False, T_=T):
    import concourse.mybir as mybir
    import concourse.tile as tile
    from concourse import bacc
    from concourse.alu_op_type import AluOpType as Op

    AF = mybir.ActivationFunctionType
    F32 = mybir.dt.float32
    BF16 = mybir.dt.bfloat16

    nc = bacc.Bacc("TRN2", target_bir_lowering=False, debug=False,
                   num_devices=world)

    F32R = mybir.dt.float32r
    n8 = T_ // NT

    def din(name, shape, dt):
        return nc.dram_tensor(name, list(shape), dt, kind="ExternalInput").ap()

    BIN = {'hsT', 'w_in', 'w_out', 'w_gate', 'w_up', 'w_down', 'c_identb'}
    RIN = {'c_i8', 'c_sel8', 'c_negselpair', 'c_ones8', 'c_ones1', 'c_ones128'}
    io = {}
    for name, shape in [
            ('hsT', (H, T_)), ('w_in', (H, MPROJ)),
            ('ln1_c', (128, 16)), ('ln2_c', (128, 16)),
            ('normw_c', (128, 4)), ('dssm_c', (128, 4)),
            ('conv_w', (128, 24)), ('conv_b', (128, 6)),
            ('a_col', (8, 1)), ('dtb_col', (8, 1)),
            ('w_out', (DINr, H)), ('w_gate', (H, FFr)), ('w_up', (H, FFr)),
            ('w_down', (FFr, H)),
            ('c_identb', (128, 128)), ('c_i8', (8, 8)), ('c_sel8', (8, 1024)),
            ('c_negselpair', (8, 1024)), ('c_ones8', (8, 128)),
            ('c_ones1', (1, 128)), ('c_ones128', (128, 1)),
            ('c_trimask2', (128, 256))]:
        dt = BF16 if name in BIN else (F32R if name in RIN else F32)
        io[name] = din(name, shape, dt)

    io['out1T'] = nc.dram_tensor("out1T", [H // world, T_], BF16,
                                 kind="ExternalOutput").ap()
    io['resid2T'] = nc.dram_tensor("resid2T", [H, T_], BF16,
                                   kind="ExternalOutput").ap()

    skind = "ExternalOutput" if debug else "Internal"
    scr = {}
    scr['z'] = nc.dram_tensor("z_s", [DINr, T_], BF16, kind=skind).ap()
    scr['x'] = nc.dram_tensor("x_s", [DINr, T_], BF16, kind=skind).ap()
    scr['b'] = nc.dram_tensor("b_s", [DS, T_], BF16, kind=skind).ap()
    scr['c'] = nc.dram_tensor("c_s", [DS, T_], BF16, kind=skind).ap()
    scr['ar1_in8'] = [
        nc.dram_tensor(f"ar1_in{q}", [H, NT], BF16, kind="Internal").ap()
        for q in range(n8)]
    scr['ar1_out8'] = [
        nc.dram_tensor(f"ar1_out{q}", [H, NT], BF16, kind="Internal",
                       addr_space="Shared").ap() for q in range(n8)]
    scr['ssq_in8'] = [
        nc.dram_tensor(f"ssq_in{q}", [1, NT], F32, kind="Internal").ap()
        for q in range(n8)]
    scr['ssq_out8'] = [
        nc.dram_tensor(f"ssq_out{q}", [1, NT], F32, kind="Internal",
                       addr_space="Shared").ap() for q in range(n8)]
    scr['rs2_in8'] = [
        nc.dram_tensor(f"rs2_in{q}", [H, NT], BF16, kind="Internal").ap()
        for q in range(n8)]
    scr['rs2_out8'] = [
        nc.dram_tensor(f"rs2_out{q}", [H // world, NT], BF16,
                       kind="Internal").ap() for q in range(n8)]
    scr['mtn0'] = nc.dram_tensor("mtn0_s", [H, NT], BF16, kind="Internal").ap()

    with tile.TileContext(nc) as tc:
        _body(tc, io, scr, world, debug, mybir, tile, Op, AF, F32, T_)

    nc.compile()
    return nc


def _body(tc, io, scr, world, debug, mybir, tile, Op, AF, F32, T_):
    nc = tc.nc
    F32R = mybir.dt.float32r
    BF16 = mybir.dt.bfloat16
    n8 = T_ // NT
    NCHUNK = T_ // Q
    CPS = (T_ // B) // Q          # chunks per sequence

    def mm(out, lhsT, rhs, start, stop, skip=False):
        if lhsT.dtype == F32:
            lhsT = lhsT.bitcast(F32R)
        if rhs.dtype == F32:
            rhs = rhs.bitcast(F32R)
        nc.tensor.matmul(out, lhsT, rhs, start=start, stop=stop,
                         skip_group_check=skip)

    def silu(out_ap, in_ap, bias=0.0, pool=None, tag="silu_tmp"):
        if SIM_SILU:
            tmp = pool.tile(list(out_ap.shape), F32, tag=tag, name=tag)
            nc.scalar.activation(tmp[:], in_ap, AF.Sigmoid, bias=bias, scale=1.0)
            if isinstance(bias, float) and bias == 0.0:
                nc.vector.tensor_tensor(out_ap, in_ap, tmp[:], Op.mult)
            else:
                raise NotImplementedError("SIM_SILU with bias AP")
        else:
            nc.scalar.activation(out_ap, in_ap, AF.Silu, bias=bias, scale=1.0)

    def allreduce(in_ap, out_ap):
        if world > 1:
            nc.gpsimd.collective_compute(
                "AllReduce", Op.add, replica_groups=[list(range(world))],
                ins=[in_ap], outs=[out_ap])
        else:
            nc.sync.dma_start(out_ap, in_ap)

    with ExitStack() as ES:
        cpool = ES.enter_context(tc.tile_pool(name="consts", bufs=1))

        # -------------------------------------------------------- constants
        C = {}
        RT = {'c_i8', 'c_sel8', 'c_negselpair', 'c_ones8', 'c_ones1',
              'c_ones128'}
        for nm, shape in [('c_identb', (128, 128)), ('c_i8', (8, 8)),
                          ('c_sel8', (8, 1024)), ('c_negselpair', (8, 1024)),
                          ('c_ones8', (8, 128)), ('c_ones1', (1, 128)),
                          ('c_ones128', (128, 1)), ('c_trimask2', (128, 256)),
                          ('ln1_c', (128, 16)), ('ln2_c', (128, 16)),
                          ('normw_c', (128, 4)), ('dssm_c', (128, 4)),
                          ('conv_w', (128, 24)), ('conv_b', (128, 6)),
                          ('a_col', (8, 1)), ('dtb_col', (8, 1))]:
            dt = BF16 if nm == 'c_identb' else (F32R if nm in RT else F32)
            t = cpool.tile(list(shape), dt, tag=nm)
            nc.sync.dma_start(t[:], io[nm])
            C[nm] = t
        identb, i8 = C['c_identb'], C['c_i8']
        sel8, negselp = C['c_sel8'], C['c_negselpair']
        ones8, ones1, ones128 = C['c_ones8'], C['c_ones1'], C['c_ones128']
        trimask2 = C['c_trimask2']

        eps1 = cpool.tile([1, 1], F32, tag="eps1", name="eps1")
        nc.vector.memset(eps1[:], float(EPS))

        # ======================================================== Phase 1
        # merged single pass over hsT: ln1 stats + z + dt + xBC + conv
        rows_a_es = ExitStack()
        rows_a = rows_a_es.enter_context(tc.tile_pool(name="rows_a", bufs=1))
        dt_rows = rows_a.tile([8, T_], F32R, tag="dt_rows", name="dt_rows")
        lA_rows = rows_a.tile([8, T_], F32R, tag="lA_rows", name="lA_rows")
        ssq_yz_row = rows_a.tile([1, T_], F32, tag="ssq_yz", name="ssq_yz")

        with tc.tile_pool(name="p1w", bufs=1) as p1w, \
             tc.tile_pool(name="p1", bufs=2) as p1, \
             tc.tile_pool(name="convp", bufs=2) as convp, \
             tc.tile_pool(name="p1ps_s", bufs=1, space="PSUM") as p1ps_s, \
             tc.tile_pool(name="p1ps_m", bufs=2, space="PSUM") as p1ps_m:

            # first token tile is prefetched BEFORE the weights so the ln1
            # stats matmuls warm up the PE while w_in streams in
            hst0 = p1.tile([128, 16, NT], BF16, tag="hst", name="hst")
            nc.sync.dma_start(hst0[:], io['hsT'][:, 0:NT]
                              .rearrange("(kt p) n -> p kt n", p=128))
            # all in_proj columns per core: [z | xBC | dt] = 1288
            w1 = p1w.tile([128, 16, MPROJ], BF16, tag="w1", name="w1")
            nc.sync.dma_start(
                w1[:], io['w_in'].rearrange("(kt p) m -> p kt m", p=128))
            for k in range(16):
                nc.vector.tensor_scalar_mul(w1[:, k, :], w1[:, k, :],
                                            C['ln1_c'][:, k:k + 1])

            halo_prev = None
            for nt in range(n8):
                tok0 = nt * NT
                seq_start = (tok0 % (T_ // B)) == 0
                if nt == 0:
                    hst = hst0
                else:
                    hst = p1.tile([128, 16, NT], BF16, tag="hst", name="hst")
                    nc.sync.dma_start(hst[:], io['hsT'][:, tok0:tok0 + NT]
                                      .rearrange("(kt p) n -> p kt n", p=128))
                # ln1 stats (ACT squares; matmuls never wait on these)
                ssq_ps = p1ps_s.tile([1, NT], F32, tag="ssq", name="ssq")
                for k in range(16):
                    sq = p1.tile([128, NT], F32R, tag="sq", name="sq")
                    nc.scalar.activation(sq[:], hst[:, k, :], AF.Square)
                    mm(ssq_ps[:], ones128[:], sq[:],
                       start=(k == 0), stop=(k == 15))
                sr0 = p1.tile([1, NT], F32, tag="sr0", name="sr0", bufs=1)
                nc.scalar.activation(sr0[:], ssq_ps[:], AF.Ln,
                                     bias=eps1[:], scale=float(1.0 / H))
                s_row = p1.tile([1, NT], F32R, tag="s_row", name="s_row",
                                bufs=1)
                nc.scalar.activation(s_row[:], sr0[:], AF.Exp, scale=-0.5)
                sb_ps = p1ps_s.tile([128, NT], F32, tag="sbps", name="sbps")
                mm(sb_ps[:], ones1[:], s_row[:], start=True, stop=True)
                sb = p1.tile([128, NT], F32, tag="sb", name="sb")
                nc.any.tensor_copy(sb[:], sb_ps[:])
                # z m-tiles: matmul on RAW hidden, scale on the way out
                for mi in range(4):
                    ps = p1ps_m.tile([128, NT], F32, tag="mt", name="mt")
                    for k in range(16):
                        mm(ps[:], w1[:, k, mi * 128:(mi + 1) * 128],
                           hst[:, k, :], start=(k == 0), stop=(k == 15))
                    zt = p1.tile([128, NT], BF16, tag="z", name="z")
                    nc.vector.tensor_tensor(zt[:], ps[:], sb[:], Op.mult)
                    nc.sync.dma_start(
                        scr['z'][mi * 128:(mi + 1) * 128, tok0:tok0 + NT], zt[:])
                # dt m-tile (8 wide)
                dtp = p1ps_s.tile([8, NT], F32, tag="mtdt", name="mtdt")
                for k in range(16):
                    mm(dtp[:], w1[:, k, DINr + CONVr:MPROJ], hst[:, k, :],
                       start=(k == 0), stop=(k == 15))
                dt_raw = p1.tile([8, NT], F32, tag="dtraw", name="dtraw",
                                 bufs=1)
                nc.vector.tensor_tensor(dt_raw[:], dtp[:], sb[:8, :], Op.mult)
                e8 = p1.tile([8, NT], F32, tag="e8", name="e8", bufs=1)
                nc.scalar.activation(e8[:], dt_raw[:], AF.Exp,
                                     bias=C['dtb_col'][:], scale=1.0)
                e8p = p1.tile([8, NT], F32, tag="e8p", name="e8p", bufs=1)
                nc.vector.tensor_scalar_add(e8p[:], e8[:], 1.0)
                nc.scalar.activation(dt_rows[:, tok0:tok0 + NT], e8p[:], AF.Ln)
                logda = p1.tile([8, NT], F32, tag="logda", name="logda",
                                bufs=1)
                nc.vector.tensor_scalar_mul(logda[:], dt_rows[:, tok0:tok0 + NT],
                                            C['a_col'][:])
                for c in range(NT // Q):
                    nc.vector.tensor_tensor_scan(
                        lA_rows[:, tok0 + c * Q:tok0 + (c + 1) * Q],
                        ones8[:, :Q].bitcast(F32), logda[:, c * Q:(c + 1) * Q],
                        0.0, Op.mult, Op.add)

                # xBC m-tiles + causal conv
                halo = [convp.tile([128, NT + 3], BF16, tag=f"halo{pt}",
                                   name=f"halo{pt}") for pt in range(6)]
                for pt in range(6):
                    ps = p1ps_m.tile([128, NT], F32, tag="mt", name="mt")
                    for k in range(16):
                        mm(ps[:], w1[:, k, DINr + pt * 128:DINr + (pt + 1) * 128],
                           hst[:, k, :], start=(k == 0), stop=(k == 15))
                    nc.vector.tensor_tensor(halo[pt][:, 3:3 + NT], ps[:], sb[:],
                                            Op.mult)
                for pt in range(6):
                    if seq_start:
                        nc.vector.memset(halo[pt][:, 0:3], 0.0)
                    else:
                        nc.vector.tensor_copy(halo[pt][:, 0:3],
                                              halo_prev[pt][:, NT:NT + 3])
                    acc = convp.tile([128, NT], BF16, tag="cacc", name="cacc")
                    nc.vector.tensor_scalar_mul(
                        acc[:], halo[pt][:, 0:NT],
                        C['conv_w'][:, pt * 4:pt * 4 + 1])
                    for d in range(1, 4):
                        nc.vector.scalar_tensor_tensor(
                            acc[:], halo[pt][:, d:d + NT],
                            C['conv_w'][:, pt * 4 + d:pt * 4 + d + 1],
                            acc[:], Op.mult, Op.add)
                    cact = convp.tile([128, NT], BF16, tag="cact", name="cact")
                    if SIM_SILU:
                        nc.vector.tensor_scalar_add(acc[:], acc[:],
                                                    C['conv_b'][:, pt:pt + 1])
                        silu(cact[:], acc[:], pool=convp, tag="cvsig")
                    else:
                        nc.scalar.activation(cact[:], acc[:], AF.Silu,
                                             bias=C['conv_b'][:, pt:pt + 1],
                                             scale=1.0)
                    if pt < 4:
                        nc.sync.dma_start(
                            scr['x'][pt * 128:(pt + 1) * 128, tok0:tok0 + NT],
                            cact[:])
                    elif pt == 4:
                        nc.sync.dma_start(scr['b'][:, tok0:tok0 + NT], cact[:])
                    else:
                        nc.sync.dma_start(scr['c'][:, tok0:tok0 + NT], cact[:])
                halo_prev = halo

        # ============================================ Phase 2: SSD + gated
        # norm + out_proj, fused per token-tile. out_proj runs on UNSCALED
        # yz — the rms scale s3 commutes through the matmul and the
        # AllReduce, and is applied in Phase 4. AR chunks issued inline.
        with tc.tile_pool(name="p2", bufs=3) as p2, \
             tc.tile_pool(name="p2s", bufs=2) as p2s, \
             tc.tile_pool(name="state", bufs=1) as spool, \
             tc.tile_pool(name="p2pre", bufs=1) as p2pre, \
             tc.tile_pool(name="p3f", bufs=2) as p3f, \
             tc.tile_pool(name="p3w", bufs=1) as p3w, \
             tc.tile_pool(name="p2ps1", bufs=1, space="PSUM") as p2ps1, \
             tc.tile_pool(name="p2ps2", bufs=2, space="PSUM") as p2ps2, \
             tc.tile_pool(name="p3ps", bufs=2, space="PSUM") as p3ps:

            w_out_t = p3w.tile([128, 4, H], BF16, tag="w_out_t", name="w_out_t")
            nc.sync.dma_start(w_out_t[:],
                              io['w_out'].rearrange("(kt p) m -> p kt m", p=128))
            for k in range(4):
                nc.vector.tensor_scalar_mul(w_out_t[:, k, :], w_out_t[:, k, :],
                                            C['normw_c'][:, k:k + 1])


            S_all = spool.tile([128, NHr * HD], F32R, tag="S_all", name="S_all")
            nc.vector.memset(S_all[:].bitcast(F32), 0.0)

            for nt in range(n8):
                y_sb = p3f.tile([128, 4, NT], BF16, tag="ysb", name="ysb")
                for cc_ in range(NT // Q):
                    ch = nt * (NT // Q) + cc_
                    t0 = ch * Q
                    xf = p2.tile([128, 4, Q], BF16, tag="xf", name="xf")
                    nc.sync.dma_start(xf[:], scr['x'][:, t0:t0 + Q]
                                      .rearrange("(pt p) n -> p pt n", p=128))
                    bf = p2.tile([128, Q], BF16, tag="bf", name="bf")
                    nc.sync.dma_start(bf[:], scr['b'][:, t0:t0 + Q])
                    cf = p2.tile([128, Q], BF16, tag="cf", name="cf")
                    nc.sync.dma_start(cf[:], scr['c'][:, t0:t0 + Q])

                    lrow = lA_rows[:, t0:t0 + Q]
                    dtrow = dt_rows[:, t0:t0 + Q]

                    expl = p2s.tile([8, Q], F32R, tag="expl", name="expl")
                    nc.scalar.activation(expl[:], lrow, AF.Exp)
                    ddr0 = p2s.tile([8, Q], F32, tag="ddr0", name="ddr0")
                    nc.vector.tensor_scalar(ddr0[:], lrow, -1.0,
                                            lrow[:, Q - 1:Q].bitcast(F32),
                                            Op.mult, Op.add)
                    dd_rows = p2s.tile([8, Q], F32R, tag="ddrows", name="ddrows")
                    nc.scalar.activation(dd_rows[:], ddr0[:], AF.Exp)
                    nc.vector.tensor_tensor(dd_rows[:], dd_rows[:], dtrow,
                                            Op.mult)
                    dg = p2s.tile([8, 8], F32R, tag="dg", name="dg")
                    nc.vector.tensor_scalar_mul(dg[:], i8[:],
                                                expl[:, Q - 1:Q].bitcast(F32))

                    misc = p2ps1.tile([128, 160], F32, tag="misc", name="misc")
                    g_ps = misc[:, 0:128]
                    ddcol_ps = misc[:, 128:136]
                    decay_ps = misc[:, 136:144]
                    dtcol_ps = misc[:, 144:152]

                    mm(g_ps, bf[:], cf[:], start=True, stop=True)
                    mm(ddcol_ps, dd_rows[:], i8[:], start=True, stop=True)
                    mm(decay_ps, ones8[:], dg[:], start=True, stop=True)
                    mm(dtcol_ps, dtrow, i8[:], start=True, stop=True)
                    g2 = p2s.tile([128, 256], F32R, tag="g2", name="g2")
                    nc.any.tensor_copy(g2[:, 0:128], g_ps)
                    nc.any.tensor_copy(g2[:, 128:256], g_ps)
                    cf2 = p2s.tile([128, 256], BF16, tag="cf2", name="cf2")
                    nc.vector.tensor_copy(cf2[:, 0:128], cf[:])
                    nc.vector.tensor_copy(cf2[:, 128:256], cf[:])
                    # late-read scalars leave PSUM early so misc can
                    # single-buffer without serializing chunks
                    dsc = p2s.tile([128, 16], F32, tag="dsc", name="dsc")
                    nc.any.tensor_copy(dsc[:], misc[:, 136:152])
                    decay_sb = dsc[:, 0:8]
                    dtcol_sb = dsc[:, 8:16]

                    tps = p2ps1.tile([128, 5, 128], BF16, tag="xtm", name="xtm")
                    nc.tensor.transpose(tps[:, 4, :], bf[:], identb[:])
                    btm = p2s.tile([128, Q], BF16, tag="btm", name="btm")
                    nc.any.tensor_copy(btm[:], tps[:, 4, :])

                    for pt in range(4):
                        nc.tensor.transpose(tps[:, pt, :],
                                            xf[:, pt, :], identb[:])
                    xtm = p2s.tile([128, NHr * HD], BF16, tag="xtm_sb",
                                   name="xtm_sb")
                    nc.any.tensor_copy(xtm[:], tps[:, 0:4, :])
                    xw = p2s.tile([128, NHr * HD], BF16, tag="xw", name="xw")
                    for h in range(NHr):
                        nc.vector.tensor_scalar_mul(
                            xw[:, h * HD:(h + 1) * HD],
                            xtm[:, h * HD:(h + 1) * HD], ddcol_ps[:, h:h + 1])

                    def y_readout(yp, pt):
                        base = (pt % 2) * 256
                        ysl0 = y_sb[0:64, pt, cc_ * Q:(cc_ + 1) * Q]
                        ysl1 = y_sb[64:128, pt, cc_ * Q:(cc_ + 1) * Q]
                        nc.vector.scalar_tensor_tensor(
                            ysl0, xf[0:64, pt, :], C['dssm_c'][0:64, pt:pt + 1],
                            yp[0:64, base:base + 128], Op.mult, Op.add)
                        nc.vector.scalar_tensor_tensor(
                            ysl1, xf[64:128, pt, :],
                            C['dssm_c'][64:128, pt:pt + 1],
                            yp[0:64, base + 128:base + 256], Op.mult, Op.add)

                    y_cur = None
                    for pr in range(4):
                        h0, h1 = 2 * pr, 2 * pr + 1
                        if pr % 2 == 0:
                            y_cur = p2ps1.tile([64, 512], F32, tag="y",
                                               name="y")
                        pairps = p2ps2.tile([128, 512], F32, tag="pairps",
                                            name="pairps")
                        dpair = pairps[:, 0:256]
                        d2 = pairps[:, 256:512]
                        for i, h in enumerate((h0, h1)):
                            half = dpair[:, i * 128:(i + 1) * 128]
                            mm(half, sel8[:, h * 128:(h + 1) * 128], lrow,
                               start=True, stop=False)
                            mm(half, lrow,
                               negselp[:, pr * 256 + i * 128:
                                       pr * 256 + (i + 1) * 128],
                               start=False, stop=True)
                        dmask = p2s.tile([128, 256], F32, tag="dmask",
                                         name="dmask")
                        nc.vector.tensor_tensor(dmask[:], dpair, trimask2[:],
                                                Op.add)
                        w0 = p2s.tile([128, 256], F32, tag="w0", name="w0")
                        nc.scalar.activation(w0[:], dmask[:], AF.Exp)
                        mm(d2[:, 0:128], sel8[:, h0 * 128:(h0 + 1) * 128],
                           expl[:], start=True, stop=True)
                        mm(d2[:, 128:256], sel8[:, h1 * 128:(h1 + 1) * 128],
                           expl[:], start=True, stop=True)
                        wt = p2s.tile([128, 256], BF16, tag="wt", name="wt")
                        for i, h in enumerate((h0, h1)):
                            nc.vector.scalar_tensor_tensor(
                                wt[:, i * 128:(i + 1) * 128],
                                w0[:, i * 128:(i + 1) * 128],
                                dtcol_sb[:, h:h + 1],
                                g2[:, i * 128:(i + 1) * 128],
                                Op.mult, Op.mult)
                        ce = p2s.tile([128, 256], F32R, tag="ce", name="ce")
                        nc.vector.tensor_tensor(ce[:], d2, cf2[:], Op.mult)
                        for i, h in enumerate((h0, h1)):
                            ysl = y_cur[:, (h % 4) * 128:(h % 4 + 1) * 128]
                            mm(ysl, xtm[:, h * HD:(h + 1) * HD],
                               wt[:, i * 128:(i + 1) * 128],
                               start=True, stop=False)
                            mm(ysl, S_all[:, h * HD:(h + 1) * HD],
                               ce[:, i * 128:(i + 1) * 128],
                               start=False, stop=True)
                        if pr % 2 == 1:
                            y_readout(y_cur, pr // 2 * 2)
                            y_readout(y_cur, pr // 2 * 2 + 1)

                    tp_ps = p2ps1.tile([128, 512], F32, tag="tp", name="tp")
                    mm(tp_ps[:], btm[:], xw[:], start=True, stop=True)
                    for h in range(NHr):
                        nc.vector.scalar_tensor_tensor(
                            S_all[:, h * HD:(h + 1) * HD],
                            S_all[:, h * HD:(h + 1) * HD],
                            decay_sb[:, h:h + 1], tp_ps[:, h * HD:(h + 1) * HD],
                            Op.mult, Op.add)

                    if (ch + 1) % CPS == 0 and ch + 1 < NCHUNK:
                        nc.vector.memset(S_all[:].bitcast(F32), 0.0)

                # gated product + stats + out_proj for this token tile
                tok0 = nt * NT
                zt = p3f.tile([128, 4, NT], BF16, tag="zt", name="zt")
                nc.sync.dma_start(zt[:], scr['z'][:, tok0:tok0 + NT]
                                  .rearrange("(pt p) n -> p pt n", p=128))
                yz_all = p3f.tile([128, 4, NT], BF16, tag="yzall", name="yzall")
                ssq_full = p3ps.tile([128, NT], F32, tag="mt3", name="mt3ssq")
                ssq_ps = ssq_full[0:1, :]
                for pt in range(4):
                    sz = p3f.tile([128, NT], BF16, tag="sz", name="sz")
                    silu(sz[:], zt[:, pt, :], pool=p3f, tag="szsig")
                    nc.vector.tensor_tensor(yz_all[:, pt, :], y_sb[:, pt, :],
                                            sz[:], Op.mult)
                    sqz = p3f.tile([128, NT], F32R, tag="sqz", name="sqz")
                    nc.scalar.activation(sqz[:], yz_all[:, pt, :], AF.Square)
                    mm(ssq_ps[:], ones128[:], sqz[:],
                       start=(pt == 0), stop=(pt == 3))
                nc.any.tensor_copy(ssq_yz_row[:, tok0:tok0 + NT], ssq_ps[:])

                for mi in range(16):
                    ps = p3ps.tile([128, NT], F32, tag="mt3", name="mt3")
                    for k in range(4):
                        mm(ps[:], w_out_t[:, k, mi * 128:(mi + 1) * 128],
                           yz_all[:, k, :], start=(k == 0), stop=(k == 3))
                    ot = p3f.tile([128, NT], BF16, tag="ot", name="ot")
                    nc.any.tensor_copy(ot[:], ps[:])
                    nc.sync.dma_start(
                        scr['ar1_in8'][nt][mi * 128:(mi + 1) * 128, :], ot[:])

                # inline chunked collectives: tiny stats AR then the big AR
                nc.sync.dma_start(scr['ssq_in8'][nt],
                                  ssq_yz_row[:, tok0:tok0 + NT])
                allreduce(scr['ssq_in8'][nt], scr['ssq_out8'][nt])
                allreduce(scr['ar1_in8'][nt], scr['ar1_out8'][nt])

                if nt == max(0, n8 - 3):
                    # Phase-4 prep for tile 0, overlapped with the P2 tail.
                    # Result (mtn for tile 0) is bounced via DRAM.
                    mt0 = p2pre.tile([128, 16, NT], BF16, tag="mt0",
                                     name="mt0")
                    nc.sync.dma_start(mt0[:], scr['ar1_out8'][0]
                                      .rearrange("(kt p) n -> p kt n", p=128))
                    sqt0 = p2s.tile([1, NT], F32, tag="sqt0", name="sqt0")
                    nc.sync.dma_start(sqt0[:], scr['ssq_out8'][0])
                    sql0 = p2s.tile([1, NT], F32, tag="sql0", name="sql0")
                    nc.scalar.activation(sql0[:], sqt0[:], AF.Ln,
                                         bias=eps1[:], scale=float(1.0 / DIN))
                    s3r0 = p2s.tile([1, NT], F32R, tag="s3r0", name="s3r0")
                    nc.scalar.activation(s3r0[:], sql0[:], AF.Exp, scale=-0.5)
                    bps = p3ps.tile([128, NT], F32, tag="mt3", name="mt3s3b")
                    mm(bps[:], ones1[:], s3r0[:], start=True, stop=True)
                    s3b0 = p2pre.tile([128, NT], BF16, tag="s3b0", name="s3b0")
                    nc.any.tensor_copy(s3b0[:], bps[:])
                    sqf = p3ps.tile([128, NT], F32, tag="mt3", name="mt3ssq0")
                    for k in range(16):
                        ht0 = p2.tile([128, NT], BF16, tag="ht0", name="ht0")
                        nc.sync.dma_start(ht0[:],
                                          io['hsT'][k * 128:(k + 1) * 128,
                                                    0:NT])
                        nc.vector.tensor_tensor(mt0[:, k, :], mt0[:, k, :],
                                                s3b0[:], Op.mult)
                        nc.vector.tensor_tensor(mt0[:, k, :], mt0[:, k, :],
                                                ht0[:], Op.add)
                        nc.sync.dma_start(
                            io['resid2T'][k * 128:(k + 1) * 128, 0:NT],
                            mt0[:, k, :])
                        sq0 = p2s.tile([128, NT], F32R, tag="sq0", name="sq0")
                        nc.scalar.activation(sq0[:], mt0[:, k, :], AF.Square)
                        mm(sqf[0:1, :], ones128[:], sq0[:],
                           start=(k == 0), stop=(k == 15))
                    slr0 = p2s.tile([1, NT], F32, tag="slr0", name="slr0")
                    nc.scalar.activation(slr0[:], sqf[0:1, :], AF.Ln,
                                         bias=eps1[:], scale=float(1.0 / H))
                    sr_0 = p2s.tile([1, NT], F32R, tag="sr_0", name="sr_0")
                    nc.scalar.activation(sr_0[:], slr0[:], AF.Exp, scale=-0.5)
                    sbp0 = p3ps.tile([128, NT], F32, tag="mt3", name="mt3sb0")
                    mm(sbp0[:], ones1[:], sr_0[:], start=True, stop=True)
                    sb0 = p2pre.tile([128, NT], BF16, tag="sb0", name="sb0")
                    nc.any.tensor_copy(sb0[:], sbp0[:])
                    mtn0 = p2pre.tile([128, 16, NT], BF16, tag="mtn0",
                                      name="mtn0")
                    for k in range(16):
                        nc.vector.scalar_tensor_tensor(
                            mtn0[:, k, :], mt0[:, k, :], C['ln2_c'][:, k:k + 1],
                            sb0[:], Op.mult, Op.mult)
                    nc.sync.dma_start(
                        scr['mtn0'].rearrange("(kt p) n -> p kt n", p=128),
                        mtn0[:])

        rows_a_es.close()

        # ================================= Phase 4: resid + ln2 + MLP + RS
        # Software-pipelined: tile j+1's resid/stats/mtn prep is emitted
        # between tile j's gate_up and down matmuls; tile 0's prep ran in
        # Phase 2 (bounced via scr['mtn0']).
        with tc.tile_pool(name="p4w", bufs=1) as p4w, \
             tc.tile_pool(name="p4", bufs=2) as p4, \
             tc.tile_pool(name="p4mt", bufs=1) as p4mt, \
             tc.tile_pool(name="p4row", bufs=1) as p4row, \
             tc.tile_pool(name="p4av", bufs=1) as p4av, \
             tc.tile_pool(name="p4ps_s", bufs=1, space="PSUM") as p4ps_s, \
             tc.tile_pool(name="p4ps_g", bufs=2, space="PSUM") as p4ps_g, \
             tc.tile_pool(name="p4ps_d", bufs=2, space="PSUM") as p4ps_d:
            # tile 0's mtn (precomputed in Phase 2) is fetched before the
            # weights so the first gate matmul isn't stuck behind 12MB of
            # weight DMA on the queue
            mtn_cur = p4.tile([128, 16, NT], BF16, tag="mtn", name="mtn")
            nc.sync.dma_start(mtn_cur[:], scr['mtn0']
                              .rearrange("(kt p) n -> p kt n", p=128))
            wg_t = p4w.tile([128, 16, FFr], BF16, tag="wg_t", name="wg_t")
            nc.sync.dma_start(wg_t[:],
                              io['w_gate'].rearrange("(kt p) m -> p kt m", p=128))
            wu_t = p4w.tile([128, 16, FFr], BF16, tag="wu_t", name="wu_t")
            nc.sync.dma_start(wu_t[:],
                              io['w_up'].rearrange("(kt p) m -> p kt m", p=128))
            wd_t = p4w.tile([128, 8, H], BF16, tag="wd_t", name="wd_t")
            nc.sync.dma_start(wd_t[:],
                              io['w_down'].rearrange("(kt p) m -> p kt m", p=128))

            def p4_prep(j):
                tok0 = j * NT
                mt = p4mt.tile([128, 16, NT], BF16, tag="mt", name="mt")
                nc.sync.dma_start(mt[:], scr['ar1_out8'][j]
                                  .rearrange("(kt p) n -> p kt n", p=128))
                ssq_t = p4row.tile([1, NT], F32, tag="ssq_t", name="ssq_t")
                nc.sync.dma_start(ssq_t[:], scr['ssq_out8'][j])
                ssq_l = p4row.tile([1, NT], F32, tag="ssq_l", name="ssq_l")
                nc.scalar.activation(ssq_l[:], ssq_t[:], AF.Ln,
                                     bias=eps1[:], scale=float(1.0 / DIN))
                s3_row = p4row.tile([1, NT], F32R, tag="s3row", name="s3row")
                nc.scalar.activation(s3_row[:], ssq_l[:], AF.Exp, scale=-0.5)
                s3b_ps = p4ps_s.tile([128, NT], F32, tag="bps", name="s3bps")
                mm(s3b_ps[:], ones1[:], s3_row[:], start=True, stop=True)
                s3b = p4.tile([128, NT], BF16, tag="s3b", name="s3b")
                nc.any.tensor_copy(s3b[:], s3b_ps[:])
                # s3-scale + residual add + ln2 stats
                ssq_ps = p4ps_s.tile([1, NT], F32, tag="ssq", name="ssq")
                for k in range(16):
                    ht = p4.tile([128, NT], BF16, tag="ht", name="ht")
                    nc.sync.dma_start(
                        ht[:], io['hsT'][k * 128:(k + 1) * 128, tok0:tok0 + NT])
                    nc.vector.tensor_tensor(mt[:, k, :], mt[:, k, :], s3b[:],
                                            Op.mult)
                    nc.vector.tensor_tensor(mt[:, k, :], mt[:, k, :], ht[:],
                                            Op.add)
                    nc.sync.dma_start(
                        io['resid2T'][k * 128:(k + 1) * 128, tok0:tok0 + NT],
                        mt[:, k, :])
                    sq = p4.tile([128, NT], F32R, tag="sq", name="sq")
                    nc.scalar.activation(sq[:], mt[:, k, :], AF.Square)
                    mm(ssq_ps[:], ones128[:], sq[:],
                       start=(k == 0), stop=(k == 15))
                sr0 = p4row.tile([1, NT], F32, tag="sr0", name="sr0")
                nc.scalar.activation(sr0[:], ssq_ps[:], AF.Ln,
                                     bias=eps1[:], scale=float(1.0 / H))
                s_row = p4row.tile([1, NT], F32R, tag="srow", name="srow")
                nc.scalar.activation(s_row[:], sr0[:], AF.Exp, scale=-0.5)
                sb_ps = p4ps_s.tile([128, NT], F32, tag="bps", name="sbps")
                mm(sb_ps[:], ones1[:], s_row[:], start=True, stop=True)
                sb = p4.tile([128, NT], BF16, tag="sb", name="sb")
                nc.any.tensor_copy(sb[:], sb_ps[:])
                mtn = p4.tile([128, 16, NT], BF16, tag="mtn", name="mtn")
                for k in range(16):
                    nc.vector.scalar_tensor_tensor(
                        mtn[:, k, :], mt[:, k, :], C['ln2_c'][:, k:k + 1],
                        sb[:], Op.mult, Op.mult)
                return mtn

            for nt in range(n8):
                tok0 = nt * NT
                # gate_up + silu*up (av kept in SBUF as down-proj k-tiles)
                av = p4av.tile([128, 8, NT], BF16, tag="av", name="av")
                for mi in range(8):
                    gp = p4ps_g.tile([128, NT], F32, tag="gp", name="gp")
                    up = p4ps_g.tile([128, NT], F32, tag="up", name="up")
                    for k in range(16):
                        mm(gp[:], wg_t[:, k, mi * 128:(mi + 1) * 128],
                           mtn_cur[:, k, :], start=(k == 0), stop=(k == 15))
                    for k in range(16):
                        mm(up[:], wu_t[:, k, mi * 128:(mi + 1) * 128],
                           mtn_cur[:, k, :], start=(k == 0), stop=(k == 15))
                    sg = p4.tile([128, NT], BF16, tag="sg", name="sg")
                    silu(sg[:], gp[:], pool=p4, tag="sgsig")
                    nc.vector.tensor_tensor(av[:, mi, :], sg[:], up[:], Op.mult)
                # next tile's prep lands between the gate and down matmuls so
                # its stats/DVE chain hides under this tile's PE work
                mtn_next = p4_prep(nt + 1) if nt + 1 < n8 else None
                # down proj -> ReduceScatter chunk (host concats slices)
                for mo in range(16):
                    ps = p4ps_d.tile([128, NT], F32, tag="dps", name="dps")
                    for k in range(8):
                        mm(ps[:], wd_t[:, k, mo * 128:(mo + 1) * 128],
                           av[:, k, :], start=(k == 0), stop=(k == 7))
                    ot = p4.tile([128, NT], BF16, tag="ot4", name="ot4")
                    nc.any.tensor_copy(ot[:], ps[:])
                    nc.sync.dma_start(
                        scr['rs2_in8'][nt][mo * 128:(mo + 1) * 128, :], ot[:])
                if world > 1:
                    nc.gpsimd.collective_compute(
                        "ReduceScatter", Op.add,
                        replica_groups=[list(range(world))],
                        ins=[scr['rs2_in8'][nt]], outs=[scr['rs2_out8'][nt]])
                else:
                    nc.sync.dma_start(scr['rs2_out8'][nt],
                                      scr['rs2_in8'][nt][0:H // world, :])
                nc.sync.dma_start(io['out1T'][:, tok0:tok0 + NT],
                                  scr['rs2_out8'][nt])
                mtn_cur = mtn_next


# ================================================================ entry point
def kernel(**inputs):
    from concourse import bass_utils

    nc = build(world=TP, debug=False)
    in_maps = [shard_core_inputs(inputs, r) for r in range(TP)]
    res = bass_utils.run_bass_kernel_spmd(nc, in_maps, core_ids=list(range(TP)))
    out1T = np.concatenate(
        [np.asarray(res.results[r]['out1T'], dtype=np.float32)
         for r in range(TP)], axis=0)                # [H, T] feature-major
    out1 = np.ascontiguousarray(out1T.T).reshape(B, L, H)
    resid2 = np.ascontiguousarray(
        np.asarray(res.results[0]['resid2T'], dtype=np.float32).T
    ).reshape(B, L, H)
    return out1, resid2


if __name__ == '__main__':
    nc = build(world=1)
    print("built ok")


# revision 57
# speedup vs baseline: 1.1651x; 1.0079x over previous
"""Trainium2 Bass kernel for nn_BambaMixerDecoderLayer_84696755077458.

Tensor-parallel over 8 NeuronCores (vLLM-style), v2 (bf16):
  - in_proj / gate_up column-sharded, out_proj / down row-sharded
  - heads + conv channels sharded with d_inner; B/C conv streams replicated
  - SSM scan via chunked SSD (Mamba2): intra-chunk matmuls + small
    cross-chunk state recurrence.
  - bf16 weights/activations for all large GEMMs, scratch and collectives;
    fp32 for stats, decay rows and the SSD state.
  - Single merged in_proj pass; MLP (gate_up+down) fused in one pass.
  - Collectives chunked 8x along tokens and issued inline so they overlap
    with compute (no global barrier between SSD and MLP phases).
Everything on-device is feature-major ([feature, token]); host does layout
transforms (transpose / shard / concat) only.

Self-contained: hardcodes all shapes; needs only /opt/trn_rl_repo on sys.path.
"""
import sys
from contextlib import ExitStack

if '/opt/trn_rl_repo' not in sys.path:
    sys.path.insert(0, '/opt/trn_rl_repo')

import numpy as np

# ---------------------------------------------------------------- constants
H = 2048          # hidden
DIN = 4096        # mamba intermediate
DS = 128          # ssm state
DCONV = 4
NH = 64
HD = 64
FF = 8192
EPS = 1e-5
B, L = 2, 2048
T = B * L                         # 4096 tokens
CONV_DIM = DIN + 2 * DS           # 4352
D_IN_PROJ = 2 * DIN + 2 * DS + NH  # 8512

TP = 8
NHr = NH // TP                    # 8 heads / core
DINr = DIN // TP                  # 512
FFr = FF // TP                    # 1024
CONVr = DINr + 2 * DS             # 768 conv channels / core
MPROJ = DINr + CONVr + NHr        # 1288 in_proj cols / core

Q = 128                           # SSD chunk
NT = 512                          # token tile (also the collective chunk)
NEG = -3.0e38
SIM_SILU = False   # True: emit sigmoid+mul instead of Silu (CoreSim support)


def _f32(x):
    return np.ascontiguousarray(np.asarray(x, dtype=np.float32))


def _bf16(x):
    import ml_dtypes
    return np.ascontiguousarray(
        np.asarray(x, dtype=np.float32).astype(ml_dtypes.bfloat16))


# ================================================================ host prep
def host_constants():
    import ml_dtypes
    identb = np.eye(128, dtype=ml_dtypes.bfloat16)
    i8 = np.eye(8, dtype=np.float32)
    sel8 = np.zeros((8, 8 * 128), np.float32)
    for h in range(8):
        sel8[h, h * 128:(h + 1) * 128] = 1.0
    negselpair = np.zeros((8, 4 * 256), np.float32)
    for p in range(4):
        negselpair[2 * p, p * 256:p * 256 + 128] = -1.0
        negselpair[2 * p + 1, p * 256 + 128:p * 256 + 256] = -1.0
    ones8 = np.ones((8, 128), np.float32)
    ones1 = np.ones((1, 128), np.float32)
    ones128 = np.ones((128, 1), np.float32)
    tri = np.where(np.arange(Q)[:, None] > np.arange(Q)[None, :], NEG, 0.0)
    trimask2 = np.concatenate([tri, tri], axis=1).astype(np.float32)
    return dict(c_identb=identb, c_i8=i8, c_sel8=sel8, c_negselpair=negselpair,
                c_ones8=ones8, c_ones1=ones1, c_ones128=ones128,
                c_trimask2=trimask2)


def shard_core_inputs(inputs, r):
    """Build the per-core input map (all feature-major)."""
    w_in = _f32(inputs['w_in'])
    zs = slice(DINr * r, DINr * (r + 1))
    xs = slice(DIN + DINr * r, DIN + DINr * (r + 1))
    bs = slice(2 * DIN, 2 * DIN + DS)
    cs = slice(2 * DIN + DS, 2 * DIN + 2 * DS)
    dts = slice(2 * DIN + 2 * DS + NHr * r, 2 * DIN + 2 * DS + NHr * (r + 1))
    w_in_r = np.concatenate(
        [w_in[:, zs], w_in[:, xs], w_in[:, bs], w_in[:, cs], w_in[:, dts]], axis=1)

    conv_w = _f32(inputs['conv_w'])
    conv_w_r = np.concatenate([conv_w[DINr * r:DINr * (r + 1)], conv_w[DIN:]], axis=0)
    conv_b = _f32(inputs['conv_b'])
    conv_b_r = np.concatenate([conv_b[DINr * r:DINr * (r + 1)], conv_b[DIN:]], axis=0)

    hs = _f32(inputs['hidden_states'])
    hs = hs.reshape(-1, H)

    A_r = _f32(inputs['A_log'])[NHr * r:NHr * (r + 1)]
    dtb_r = _f32(inputs['dt_bias'])[NHr * r:NHr * (r + 1)]
    D_r = _f32(inputs['D_ssm'])[NHr * r:NHr * (r + 1)]

    m = dict(host_constants())
    m['hsT'] = _bf16(hs.T)                                      # [2048, T]
    m['w_in'] = _bf16(w_in_r)                                   # [2048, 1288]
    # per-k-tile column form of per-feature vectors: [128, n_tiles]
    m['ln1_c'] = np.ascontiguousarray(_f32(inputs['ln1_w']).reshape(16, 128).T)
    m['ln2_c'] = np.ascontiguousarray(_f32(inputs['ln2_w']).reshape(16, 128).T)
    m['normw_c'] = np.ascontiguousarray(
        _f32(inputs['norm_w'])[DINr * r:DINr * (r + 1)].reshape(4, 128).T)
    m['dssm_c'] = np.ascontiguousarray(
        np.repeat(D_r, HD).reshape(4, 128).T)                   # [128, 4]
    # conv weights: [128, 6*4] with [p, pt*4+d]
    m['conv_w'] = np.ascontiguousarray(
        conv_w_r.reshape(6, 128, DCONV).transpose(1, 0, 2).reshape(128, 6 * DCONV))
    m['conv_b'] = np.ascontiguousarray(conv_b_r.reshape(6, 128).T)  # [128, 6]
    m['a_col'] = np.ascontiguousarray((-np.exp(A_r))[:, None])   # [8,1]
    m['dtb_col'] = np.ascontiguousarray(dtb_r[:, None])          # [8,1]
    m['w_out'] = _bf16(_f32(inputs['w_out'])[DINr * r:DINr * (r + 1)])
    wgu = _f32(inputs['w_gate_up'])
    m['w_gate'] = _bf16(wgu[:, FFr * r:FFr * (r + 1)])
    m['w_up'] = _bf16(wgu[:, FF + FFr * r:FF + FFr * (r + 1)])
    m['w_down'] = _bf16(_f32(inputs['w_down'])[FFr * r:FFr * (r + 1)])
    return m


# ================================================================ the kernel
def build(world=TP, debug=False, T_=T):
    import concourse.mybir as mybir
    import concourse.tile as tile
    from concourse import bacc
    from concourse.alu_op_type import AluOpType as Op

    AF = mybir.ActivationFunctionType
    F32 = mybir.dt.float32
    BF16 = mybir.dt.bfloat16

    nc = bacc.Bacc("TRN2", target_bir_lowering=False, debug=False,
                   num_devices=world)

    F32R = mybir.dt.float32r
    n8 = T_ // NT

    def din(name, shape, dt):
        return nc.dram_tensor(name, list(shape), dt, kind="ExternalInput").ap()

    BIN = {'hsT', 'w_in', 'w_out', 'w_gate', 'w_up', 'w_down', 'c_identb'}
    RIN = {'c_i8', 'c_sel8', 'c_negselpair', 'c_ones8', 'c_ones1', 'c_ones128'}
    io = {}
    for name, shape in [
            ('hsT', (H, T_)), ('w_in', (H, MPROJ)),
            ('ln1_c', (128, 16)), ('ln2_c', (128, 16)),
            ('normw_c', (128, 4)), ('dssm_c', (128, 4)),
            ('conv_w', (128, 24)), ('conv_b', (128, 6)),
            ('a_col', (8, 1)), ('dtb_col', (8, 1)),
            ('w_out', (DINr, H)), ('w_gate', (H, FFr)), ('w_up', (H, FFr)),
            ('w_down', (FFr, H)),
            ('c_identb', (128, 128)), ('c_i8', (8, 8)), ('c_sel8', (8, 1024)),
            ('c_negselpair', (8, 1024)), ('c_ones8', (8, 128)),
            ('c_ones1', (1, 128)), ('c_ones128', (128, 1)),
            ('c_trimask2', (128, 256))]:
        dt = BF16 if name in BIN else (F32R if name in RIN else F32)
        io[name] = din(name, shape, dt)

    io['out1T'] = nc.dram_tensor("out1T", [H // world, T_], BF16,
                                 kind="ExternalOutput").ap()
    io['resid2T'] = nc.dram_tensor("resid2T", [H, T_], BF16,
                                   kind="ExternalOutput").ap()

    skind = "ExternalOutput" if debug else "Internal"
    scr = {}
    scr['z'] = nc.dram_tensor("z_s", [DINr, T_], BF16, kind=skind).ap()
    scr['x'] = nc.dram_tensor("x_s", [DINr, T_], BF16, kind=skind).ap()
    scr['b'] = nc.dram_tensor("b_s", [DS, T_], BF16, kind=skind).ap()
    scr['c'] = nc.dram_tensor("c_s", [DS, T_], BF16, kind=skind).ap()
    scr['ar1_in8'] = [
        nc.dram_tensor(f"ar1_in{q}", [H, NT], BF16, kind="Internal").ap()
        for q in range(n8)]
    scr['ar1_out8'] = [
        nc.dram_tensor(f"ar1_out{q}", [H, NT], BF16, kind="Internal",
                       addr_space="Shared").ap() for q in range(n8)]
    scr['ssq_in8'] = [
        nc.dram_tensor(f"ssq_in{q}", [1, NT], F32, kind="Internal").ap()
        for q in range(n8)]
    scr['ssq_out8'] = [
        nc.dram_tensor(f"ssq_out{q}", [1, NT], F32, kind="Internal",
                       addr_space="Shared").ap() for q in range(n8)]
    scr['rs2_in8'] = [
        nc.dram_tensor(f"rs2_in{q}", [H, NT], BF16, kind="Internal").ap()
        for q in range(n8)]
    scr['rs2_out8'] = [
        nc.dram_tensor(f"rs2_out{q}", [H // world, NT], BF16,
                       kind="Internal").ap() for q in range(n8)]
    scr['mtn0'] = nc.dram_tensor("mtn0_s", [H, NT], BF16, kind="Internal").ap()

    with tile.TileContext(nc) as tc:
        _body(tc, io, scr, world, debug, mybir, tile, Op, AF, F32, T_)

    nc.compile()
    return nc


def _body(tc, io, scr, world, debug, mybir, tile, Op, AF, F32, T_):
    nc = tc.nc
    F32R = mybir.dt.float32r
    BF16 = mybir.dt.bfloat16
    n8 = T_ // NT
    NCHUNK = T_ // Q
    CPS = (T_ // B) // Q          # chunks per sequence

    def mm(out, lhsT, rhs, start, stop, skip=False):
        if lhsT.dtype == F32:
            lhsT = lhsT.bitcast(F32R)
        if rhs.dtype == F32:
            rhs = rhs.bitcast(F32R)
        nc.tensor.matmul(out, lhsT, rhs, start=start, stop=stop,
                         skip_group_check=skip)

    def silu(out_ap, in_ap, bias=0.0, pool=None, tag="silu_tmp"):
        if SIM_SILU:
            tmp = pool.tile(list(out_ap.shape), F32, tag=tag, name=tag)
            nc.scalar.activation(tmp[:], in_ap, AF.Sigmoid, bias=bias, scale=1.0)
            if isinstance(bias, float) and bias == 0.0:
                nc.vector.tensor_tensor(out_ap, in_ap, tmp[:], Op.mult)
            else:
                raise NotImplementedError("SIM_SILU with bias AP")
        else:
            nc.scalar.activation(out_ap, in_ap, AF.Silu, bias=bias, scale=1.0)

    def allreduce(in_ap, out_ap):
        if world > 1:
            nc.gpsimd.collective_compute(
                "AllReduce", Op.add, replica_groups=[list(range(world))],
                ins=[in_ap], outs=[out_ap])
        else:
            nc.sync.dma_start(out_ap, in_ap)

    with ExitStack() as ES:
        cpool = ES.enter_context(tc.tile_pool(name="consts", bufs=1))

        # -------------------------------------------------------- constants
        C = {}
        RT = {'c_i8', 'c_sel8', 'c_negselpair', 'c_ones8', 'c_ones1',
              'c_ones128'}
        for nm, shape in [('c_identb', (128, 128)), ('c_i8', (8, 8)),
                          ('c_sel8', (8, 1024)), ('c_negselpair', (8, 1024)),
                          ('c_ones8', (8, 128)), ('c_ones1', (1, 128)),
                          ('c_ones128', (128, 1)), ('c_trimask2', (128, 256)),
                          ('ln1_c', (128, 16)), ('ln2_c', (128, 16)),
                          ('normw_c', (128, 4)), ('dssm_c', (128, 4)),
                          ('conv_w', (128, 24)), ('conv_b', (128, 6)),
                          ('a_col', (8, 1)), ('dtb_col', (8, 1))]:
            dt = BF16 if nm == 'c_identb' else (F32R if nm in RT else F32)
            t = cpool.tile(list(shape), dt, tag=nm)
            nc.sync.dma_start(t[:], io[nm])
            C[nm] = t
        identb, i8 = C['c_identb'], C['c_i8']
        sel8, negselp = C['c_sel8'], C['c_negselpair']
        ones8, ones1, ones128 = C['c_ones8'], C['c_ones1'], C['c_ones128']
        trimask2 = C['c_trimask2']

        eps1 = cpool.tile([1, 1], F32, tag="eps1", name="eps1")
        nc.vector.memset(eps1[:], float(EPS))

        # ======================================================== Phase 1
        # merged single pass over hsT: ln1 stats + z + dt + xBC + conv
        rows_a_es = ExitStack()
        rows_a = rows_a_es.enter_context(tc.tile_pool(name="rows_a", bufs=1))
        dt_rows = rows_a.tile([8, T_], F32R, tag="dt_rows", name="dt_rows")
        lA_rows = rows_a.tile([8, T_], F32R, tag="lA_rows", name="lA_rows")
        ssq_yz_row = rows_a.tile([1, T_], F32, tag="ssq_yz", name="ssq_yz")

        with tc.tile_pool(name="p1w", bufs=1) as p1w, \
             tc.tile_pool(name="p1", bufs=2) as p1, \
             tc.tile_pool(name="convp", bufs=2) as convp, \
             tc.tile_pool(name="p1ps_s", bufs=1, space="PSUM") as p1ps_s, \
             tc.tile_pool(name="p1ps_m", bufs=2, space="PSUM") as p1ps_m:

            # first token tile is prefetched BEFORE the weights so the ln1
            # stats matmuls warm up the PE while w_in streams in
            hst0 = p1.tile([128, 16, NT], BF16, tag="hst", name="hst")
            nc.sync.dma_start(hst0[:], io['hsT'][:, 0:NT]
                              .rearrange("(kt p) n -> p kt n", p=128))
            # all in_proj columns per core: [z | xBC | dt] = 1288
            w1 = p1w.tile([128, 16, MPROJ], BF16, tag="w1", name="w1")
            nc.sync.dma_start(
                w1[:], io['w_in'].rearrange("(kt p) m -> p kt m", p=128))
            for k in range(16):
                nc.vector.tensor_scalar_mul(w1[:, k, :], w1[:, k, :],
                                            C['ln1_c'][:, k:k + 1])

            halo_prev = None
            for nt in range(n8):
                tok0 = nt * NT
                seq_start = (tok0 % (T_ // B)) == 0
                if nt == 0:
                    hst = hst0
                else:
                    hst = p1.tile([128, 16, NT], BF16, tag="hst", name="hst")
                    nc.sync.dma_start(hst[:], io['hsT'][:, tok0:tok0 + NT]
                                      .rearrange("(kt p) n -> p kt n", p=128))
                # ln1 stats (ACT squares; matmuls never wait on these)
                ssq_ps = p1ps_s.tile([1, NT], F32, tag="ssq", name="ssq")
                for k in range(16):
                    sq = p1.tile([128, NT], F32R, tag="sq", name="sq")
                    nc.scalar.activation(sq[:], hst[:, k, :], AF.Square)
                    mm(ssq_ps[:], ones128[:], sq[:],
                       start=(k == 0), stop=(k == 15))
                sr0 = p1.tile([1, NT], F32, tag="sr0", name="sr0", bufs=1)
                nc.scalar.activation(sr0[:], ssq_ps[:], AF.Ln,
                                     bias=eps1[:], scale=float(1.0 / H))
                s_row = p1.tile([1, NT], F32R, tag="s_row", name="s_row",
                                bufs=1)
                nc.scalar.activation(s_row[:], sr0[:], AF.Exp, scale=-0.5)
                sb_ps = p1ps_s.tile([128, NT], F32, tag="sbps", name="sbps")
                mm(sb_ps[:], ones1[:], s_row[:], start=True, stop=True)
                sb = p1.tile([128, NT], F32, tag="sb", name="sb")
                nc.any.tensor_copy(sb[:], sb_ps[:])
                # z m-tiles: matmul on RAW hidden, scale on the way out
                for mi in range(4):
                    ps = p1ps_m.tile([128, NT], F32, tag="mt", name="mt")
                    for k in range(16):
                        mm(ps[:], w1[:, k, mi * 128:(mi + 1) * 128],
                           hst[:, k, :], start=(k == 0), stop=(k == 15))
                    zt = p1.tile([128, NT], BF16, tag="z", name="z")
                    nc.vector.tensor_tensor(zt[:], ps[:], sb[:], Op.mult)
                    nc.sync.dma_start(
                        scr['z'][mi * 128:(mi + 1) * 128, tok0:tok0 + NT], zt[:])
                # dt m-tile (8 wide)
                dtp = p1ps_s.tile([8, NT], F32, tag="mtdt", name="mtdt")
                for k in range(16):
                    mm(dtp[:], w1[:, k, DINr + CONVr:MPROJ], hst[:, k, :],
                       start=(k == 0), stop=(k == 15))
                dt_raw = p1.tile([8, NT], F32, tag="dtraw", name="dtraw",
                                 bufs=1)
                nc.vector.tensor_tensor(dt_raw[:], dtp[:], sb[:8, :], Op.mult)
                e8 = p1.tile([8, NT], F32, tag="e8", name="e8", bufs=1)
                nc.scalar.activation(e8[:], dt_raw[:], AF.Exp,
                                     bias=C['dtb_col'][:], scale=1.0)
                e8p = p1.tile([8, NT], F32, tag="e8p", name="e8p", bufs=1)
                nc.vector.tensor_scalar_add(e8p[:], e8[:], 1.0)
                nc.scalar.activation(dt_rows[:, tok0:tok0 + NT], e8p[:], AF.Ln)
                logda = p1.tile([8, NT], F32, tag="logda", name="logda",
                                bufs=1)
                nc.vector.tensor_scalar_mul(logda[:], dt_rows[:, tok0:tok0 + NT],
                                            C['a_col'][:])
                for c in range(NT // Q):
                    nc.vector.tensor_tensor_scan(
                        lA_rows[:, tok0 + c * Q:tok0 + (c + 1) * Q],
                        ones8[:, :Q].bitcast(F32), logda[:, c * Q:(c + 1) * Q],
                        0.0, Op.mult, Op.add)

                # xBC m-tiles + causal conv
                halo = [convp.tile([128, NT + 3], BF16, tag=f"halo{pt}",
                                   name=f"halo{pt}") for pt in range(6)]
                for pt in range(6):
                    ps = p1ps_m.tile([128, NT], F32, tag="mt", name="mt")
                    for k in range(16):
                        mm(ps[:], w1[:, k, DINr + pt * 128:DINr + (pt + 1) * 128],
                           hst[:, k, :], start=(k == 0), stop=(k == 15))
                    nc.vector.tensor_tensor(halo[pt][:, 3:3 + NT], ps[:], sb[:],
                                            Op.mult)
                for pt in range(6):
                    if seq_start:
                        nc.vector.memset(halo[pt][:, 0:3], 0.0)
                    else:
                        nc.vector.tensor_copy(halo[pt][:, 0:3],
                                              halo_prev[pt][:, NT:NT + 3])
                    acc = convp.tile([128, NT], BF16, tag="cacc", name="cacc")
                    nc.vector.tensor_scalar_mul(
                        acc[:], halo[pt][:, 0:NT],
                        C['conv_w'][:, pt * 4:pt * 4 + 1])
                    for d in range(1, 4):
                        nc.vector.scalar_tensor_tensor(
                            acc[:], halo[pt][:, d:d + NT],
                            C['conv_w'][:, pt * 4 + d:pt * 4 + d + 1],
                            acc[:], Op.mult, Op.add)
                    cact = convp.tile([128, NT], BF16, tag="cact", name="cact")
                    if SIM_SILU:
                        nc.vector.tensor_scalar_add(acc[:], acc[:],
                                                    C['conv_b'][:, pt:pt + 1])
                        silu(cact[:], acc[:], pool=convp, tag="cvsig")
                    else:
                        nc.scalar.activation(cact[:], acc[:], AF.Silu,
                                             bias=C['conv_b'][:, pt:pt + 1],
                                             scale=1.0)
                    if pt < 4:
                        nc.sync.dma_start(
                            scr['x'][pt * 128:(pt + 1) * 128, tok0:tok0 + NT],
                            cact[:])
                    elif pt == 4:
                        nc.sync.dma_start(scr['b'][:, tok0:tok0 + NT], cact[:])
                    else:
                        nc.sync.dma_start(scr['c'][:, tok0:tok0 + NT], cact[:])
                halo_prev = halo

        # ============================================ Phase 2: SSD + gated
        # norm + out_proj, fused per token-tile. out_proj runs on UNSCALED
        # yz — the rms scale s3 commutes through the matmul and the
        # AllReduce, and is applied in Phase 4. AR chunks issued inline.
        with tc.tile_pool(name="p2", bufs=3) as p2, \
             tc.tile_pool(name="p2s", bufs=2) as p2s, \
             tc.tile_pool(name="state", bufs=1) as spool, \
             tc.tile_pool(name="p2pre", bufs=1) as p2pre, \
             tc.tile_pool(name="p3f", bufs=2) as p3f, \
             tc.tile_pool(name="p3w", bufs=1) as p3w, \
             tc.tile_pool(name="p2ps1", bufs=1, space="PSUM") as p2ps1, \
             tc.tile_pool(name="p2ps2", bufs=2, space="PSUM") as p2ps2, \
             tc.tile_pool(name="p3ps", bufs=2, space="PSUM") as p3ps:

            w_out_t = p3w.tile([128, 4, H], BF16, tag="w_out_t", name="w_out_t")
            nc.sync.dma_start(w_out_t[:],
                              io['w_out'].rearrange("(kt p) m -> p kt m", p=128))
            for k in range(4):
                nc.vector.tensor_scalar_mul(w_out_t[:, k, :], w_out_t[:, k, :],
                                            C['normw_c'][:, k:k + 1])


            S_all = spool.tile([128, NHr * HD], F32R, tag="S_all", name="S_all")
            nc.vector.memset(S_all[:].bitcast(F32), 0.0)

            for nt in range(n8):
                y_sb = p3f.tile([128, 4, NT], BF16, tag="ysb", name="ysb")
                for cc_ in range(NT // Q):
                    ch = nt * (NT // Q) + cc_
                    t0 = ch * Q
                    xf = p2.tile([128, 4, Q], BF16, tag="xf", name="xf")
                    nc.sync.dma_start(xf[:], scr['x'][:, t0:t0 + Q]
                                      .rearrange("(pt p) n -> p pt n", p=128))
                    bf = p2.tile([128, Q], BF16, tag="bf", name="bf")
                    nc.sync.dma_start(bf[:], scr['b'][:, t0:t0 + Q])
                    cf = p2.tile([128, Q], BF16, tag="cf", name="cf")
                    nc.sync.dma_start(cf[:], scr['c'][:, t0:t0 + Q])

                    lrow = lA_rows[:, t0:t0 + Q]
                    dtrow = dt_rows[:, t0:t0 + Q]

                    expl = p2s.tile([8, Q], F32R, tag="expl", name="expl")
                    nc.scalar.activation(expl[:], lrow, AF.Exp)
                    ddr0 = p2s.tile([8, Q], F32, tag="ddr0", name="ddr0")
                    nc.vector.tensor_scalar(ddr0[:], lrow, -1.0,
                                            lrow[:, Q - 1:Q].bitcast(F32),
                                            Op.mult, Op.add)
                    dd_rows = p2s.tile([8, Q], F32R, tag="ddrows", name="ddrows")
                    nc.scalar.activation(dd_rows[:], ddr0[:], AF.Exp)
                    nc.vector.tensor_tensor(dd_rows[:], dd_rows[:], dtrow,
                                            Op.mult)
                    dg = p2s.tile([8, 8], F32R, tag="dg", name="dg")
                    nc.vector.tensor_scalar_mul(dg[:], i8[:],
                                                expl[:, Q - 1:Q].bitcast(F32))

                    misc = p2ps1.tile([128, 160], F32, tag="misc", name="misc")
                    g_ps = misc[:, 0:128]
                    ddcol_ps = misc[:, 128:136]
                    decay_ps = misc[:, 136:144]
                    dtcol_ps = misc[:, 144:152]

                    mm(g_ps, bf[:], cf[:], start=True, stop=True)
                    mm(ddcol_ps, dd_rows[:], i8[:], start=True, stop=True)
                    mm(decay_ps, ones8[:], dg[:], start=True, stop=True)
                    mm(dtcol_ps, dtrow, i8[:], start=True, stop=True)
                    g2 = p2s.tile([128, 256], F32R, tag="g2", name="g2")
                    nc.any.tensor_copy(g2[:, 0:128], g_ps)
                    nc.any.tensor_copy(g2[:, 128:256], g_ps)
                    cf2 = p2s.tile([128, 256], BF16, tag="cf2", name="cf2")
                    nc.vector.tensor_copy(cf2[:, 0:128], cf[:])
                    nc.vector.tensor_copy(cf2[:, 128:256], cf[:])
                    # late-read scalars leave PSUM early so misc can
                    # single-buffer without serializing chunks
                    dsc = p2s.tile([128, 16], F32, tag="dsc", name="dsc")
                    nc.any.tensor_copy(dsc[:], misc[:, 136:152])
                    decay_sb = dsc[:, 0:8]
                    dtcol_sb = dsc[:, 8:16]

                    tps = p2ps1.tile([128, 5, 128], BF16, tag="xtm", name="xtm")
                    nc.tensor.transpose(tps[:, 4, :], bf[:], identb[:])
                    btm = p2s.tile([128, Q], BF16, tag="btm", name="btm")
                    nc.any.tensor_copy(btm[:], tps[:, 4, :])

                    for pt in range(4):
                        nc.tensor.transpose(tps[:, pt, :],
                                            xf[:, pt, :], identb[:])
                    xtm = p2s.tile([128, NHr * HD], BF16, tag="xtm_sb",
                                   name="xtm_sb")
                    nc.any.tensor_copy(xtm[:], tps[:, 0:4, :])
                    xw = p2s.tile([128, NHr * HD], BF16, tag="xw", name="xw")
                    for h in range(NHr):
                        nc.vector.tensor_scalar_mul(
                            xw[:, h * HD:(h + 1) * HD],
                            xtm[:, h * HD:(h + 1) * HD], ddcol_ps[:, h:h + 1])

                    def y_readout(yp, pt):
                        base = (pt % 2) * 256
                        ysl0 = y_sb[0:64, pt, cc_ * Q:(cc_ + 1) * Q]
                        ysl1 = y_sb[64:128, pt, cc_ * Q:(cc_ + 1) * Q]
                        nc.vector.scalar_tensor_tensor(
                            ysl0, xf[0:64, pt, :], C['dssm_c'][0:64, pt:pt + 1],
                            yp[0:64, base:base + 128], Op.mult, Op.add)
                        nc.vector.scalar_tensor_tensor(
                            ysl1, xf[64:128, pt, :],
                            C['dssm_c'][64:128, pt:pt + 1],
                            yp[0:64, base + 128:base + 256], Op.mult, Op.add)

                    # pairs are processed two at a time: both pairs' segsum
                    # matmuls are emitted back-to-back so the PE chews on
                    # pair b while the DVE/ACT chain produces pair a's wt/ce
                    # (keeps the MM bursts long -> HAM stays at full clock)
                    for prg in range(2):
                        prs = (2 * prg, 2 * prg + 1)
                        y_cur = p2ps1.tile([64, 512], F32, tag="y", name="y")
                        ptile = {}
                        for pr in prs:
                            h0, h1 = 2 * pr, 2 * pr + 1
                            pairps = p2ps2.tile([128, 512], F32, tag="pairps",
                                                name="pairps")
                            ptile[pr] = pairps
                            dpair = pairps[:, 0:256]
                            d2 = pairps[:, 256:512]
                            for i, h in enumerate((h0, h1)):
                                half = dpair[:, i * 128:(i + 1) * 128]
                                mm(half, sel8[:, h * 128:(h + 1) * 128], lrow,
                                   start=True, stop=False)
                                mm(half, lrow,
                                   negselp[:, pr * 256 + i * 128:
                                           pr * 256 + (i + 1) * 128],
                                   start=False, stop=True)
                            mm(d2[:, 0:128], sel8[:, h0 * 128:(h0 + 1) * 128],
                               expl[:], start=True, stop=True)
                            mm(d2[:, 128:256], sel8[:, h1 * 128:(h1 + 1) * 128],
                               expl[:], start=True, stop=True)
                        wts, ces = {}, {}
                        for pr in prs:
                            h0, h1 = 2 * pr, 2 * pr + 1
                            dpair = ptile[pr][:, 0:256]
                            d2 = ptile[pr][:, 256:512]
                            dmask = p2s.tile([128, 256], F32, tag="dmask",
                                             name="dmask")
                            nc.vector.tensor_tensor(dmask[:], dpair,
                                                    trimask2[:], Op.add)
                            w0 = p2s.tile([128, 256], F32, tag="w0", name="w0")
                            nc.scalar.activation(w0[:], dmask[:], AF.Exp)
                            wt = p2s.tile([128, 256], BF16, tag="wt", name="wt")
                            for i, h in enumerate((h0, h1)):
                                nc.vector.scalar_tensor_tensor(
                                    wt[:, i * 128:(i + 1) * 128],
                                    w0[:, i * 128:(i + 1) * 128],
                                    dtcol_sb[:, h:h + 1],
                                    g2[:, i * 128:(i + 1) * 128],
                                    Op.mult, Op.mult)
                            ce = p2s.tile([128, 256], F32R, tag="ce", name="ce")
                            nc.vector.tensor_tensor(ce[:], d2, cf2[:], Op.mult)
                            wts[pr], ces[pr] = wt, ce
                        for pr in prs:
                            h0, h1 = 2 * pr, 2 * pr + 1
                            for i, h in enumerate((h0, h1)):
                                ysl = y_cur[:, (h % 4) * 128:(h % 4 + 1) * 128]
                                mm(ysl, xtm[:, h * HD:(h + 1) * HD],
                                   wts[pr][:, i * 128:(i + 1) * 128],
                                   start=True, stop=False)
                                mm(ysl, S_all[:, h * HD:(h + 1) * HD],
                                   ces[pr][:, i * 128:(i + 1) * 128],
                                   start=False, stop=True)
                        y_readout(y_cur, prg * 2)
                        y_readout(y_cur, prg * 2 + 1)

                    tp_ps = p2ps1.tile([128, 512], F32, tag="tp", name="tp")
                    mm(tp_ps[:], btm[:], xw[:], start=True, stop=True)
                    for h in range(NHr):
                        nc.vector.scalar_tensor_tensor(
                            S_all[:, h * HD:(h + 1) * HD],
                            S_all[:, h * HD:(h + 1) * HD],
                            decay_sb[:, h:h + 1], tp_ps[:, h * HD:(h + 1) * HD],
                            Op.mult, Op.add)

                    if (ch + 1) % CPS == 0 and ch + 1 < NCHUNK:
                        nc.vector.memset(S_all[:].bitcast(F32), 0.0)

                # gated product + stats + out_proj for this token tile
                tok0 = nt * NT
                zt = p3f.tile([128, 4, NT], BF16, tag="zt", name="zt")
                nc.sync.dma_start(zt[:], scr['z'][:, tok0:tok0 + NT]
                                  .rearrange("(pt p) n -> p pt n", p=128))
                yz_all = p3f.tile([128, 4, NT], BF16, tag="yzall", name="yzall")
                ssq_full = p3ps.tile([128, NT], F32, tag="mt3", name="mt3ssq")
                ssq_ps = ssq_full[0:1, :]
                for pt in range(4):
                    sz = p3f.tile([128, NT], BF16, tag="sz", name="sz")
                    silu(sz[:], zt[:, pt, :], pool=p3f, tag="szsig")
                    nc.vector.tensor_tensor(yz_all[:, pt, :], y_sb[:, pt, :],
                                            sz[:], Op.mult)
                    sqz = p3f.tile([128, NT], F32R, tag="sqz", name="sqz")
                    nc.scalar.activation(sqz[:], yz_all[:, pt, :], AF.Square)
                    mm(ssq_ps[:], ones128[:], sqz[:],
                       start=(pt == 0), stop=(pt == 3))
                nc.any.tensor_copy(ssq_yz_row[:, tok0:tok0 + NT], ssq_ps[:])

                for mi in range(16):
                    ps = p3ps.tile([128, NT], F32, tag="mt3", name="mt3")
                    for k in range(4):
                        mm(ps[:], w_out_t[:, k, mi * 128:(mi + 1) * 128],
                           yz_all[:, k, :], start=(k == 0), stop=(k == 3))
                    ot = p3f.tile([128, NT], BF16, tag="ot", name="ot")
                    nc.any.tensor_copy(ot[:], ps[:])
                    nc.sync.dma_start(
                        scr['ar1_in8'][nt][mi * 128:(mi + 1) * 128, :], ot[:])

                # inline chunked collectives: tiny stats AR then the big AR
                nc.sync.dma_start(scr['ssq_in8'][nt],
                                  ssq_yz_row[:, tok0:tok0 + NT])
                allreduce(scr['ssq_in8'][nt], scr['ssq_out8'][nt])
                allreduce(scr['ar1_in8'][nt], scr['ar1_out8'][nt])

                if nt == max(0, n8 - 3):
                    # Phase-4 prep for tile 0, overlapped with the P2 tail.
                    # Result (mtn for tile 0) is bounced via DRAM.
                    mt0 = p2pre.tile([128, 16, NT], BF16, tag="mt0",
                                     name="mt0")
                    nc.sync.dma_start(mt0[:], scr['ar1_out8'][0]
                                      .rearrange("(kt p) n -> p kt n", p=128))
                    sqt0 = p2s.tile([1, NT], F32, tag="sqt0", name="sqt0")
                    nc.sync.dma_start(sqt0[:], scr['ssq_out8'][0])
                    sql0 = p2s.tile([1, NT], F32, tag="sql0", name="sql0")
                    nc.scalar.activation(sql0[:], sqt0[:], AF.Ln,
                                         bias=eps1[:], scale=float(1.0 / DIN))
                    s3r0 = p2s.tile([1, NT], F32R, tag="s3r0", name="s3r0")
                    nc.scalar.activation(s3r0[:], sql0[:], AF.Exp, scale=-0.5)
                    bps = p3ps.tile([128, NT], F32, tag="mt3", name="mt3s3b")
                    mm(bps[:], ones1[:], s3r0[:], start=True, stop=True)
                    s3b0 = p2pre.tile([128, NT], BF16, tag="s3b0", name="s3b0")
                    nc.any.tensor_copy(s3b0[:], bps[:])
                    sqf = p3ps.tile([128, NT], F32, tag="mt3", name="mt3ssq0")
                    for k in range(16):
                        ht0 = p2.tile([128, NT], BF16, tag="ht0", name="ht0")
                        nc.sync.dma_start(ht0[:],
                                          io['hsT'][k * 128:(k + 1) * 128,
                                                    0:NT])
                        nc.vector.tensor_tensor(mt0[:, k, :], mt0[:, k, :],
                                                s3b0[:], Op.mult)
                        nc.vector.tensor_tensor(mt0[:, k, :], mt0[:, k, :],
                                                ht0[:], Op.add)
                        nc.sync.dma_start(
                            io['resid2T'][k * 128:(k + 1) * 128, 0:NT],
                            mt0[:, k, :])
                        sq0 = p2s.tile([128, NT], F32R, tag="sq0", name="sq0")
                        nc.scalar.activation(sq0[:], mt0[:, k, :], AF.Square)
                        mm(sqf[0:1, :], ones128[:], sq0[:],
                           start=(k == 0), stop=(k == 15))
                    slr0 = p2s.tile([1, NT], F32, tag="slr0", name="slr0")
                    nc.scalar.activation(slr0[:], sqf[0:1, :], AF.Ln,
                                         bias=eps1[:], scale=float(1.0 / H))
                    sr_0 = p2s.tile([1, NT], F32R, tag="sr_0", name="sr_0")
                    nc.scalar.activation(sr_0[:], slr0[:], AF.Exp, scale=-0.5)
                    sbp0 = p3ps.tile([128, NT], F32, tag="mt3", name="mt3sb0")
                    mm(sbp0[:], ones1[:], sr_0[:], start=True, stop=True)
                    sb0 = p2pre.tile([128, NT], BF16, tag="sb0", name="sb0")
                    nc.any.tensor_copy(sb0[:], sbp0[:])
                    mtn0 = p2pre.tile([128, 16, NT], BF16, tag="mtn0",
                                      name="mtn0")
                    for k in range(16):
                        nc.vector.scalar_tensor_tensor(
                            mtn0[:, k, :], mt0[:, k, :], C['ln2_c'][:, k:k + 1],
                            sb0[:], Op.mult, Op.mult)
                    nc.sync.dma_start(
                        scr['mtn0'].rearrange("(kt p) n -> p kt n", p=128),
                        mtn0[:])

        rows_a_es.close()

        # ================================= Phase 4: resid + ln2 + MLP + RS
        # Software-pipelined: tile j+1's resid/stats/mtn prep is emitted
        # between tile j's gate_up and down matmuls; tile 0's prep ran in
        # Phase 2 (bounced via scr['mtn0']).
        with tc.tile_pool(name="p4w", bufs=1) as p4w, \
             tc.tile_pool(name="p4", bufs=2) as p4, \
             tc.tile_pool(name="p4mt", bufs=1) as p4mt, \
             tc.tile_pool(name="p4row", bufs=1) as p4row, \
             tc.tile_pool(name="p4av", bufs=1) as p4av, \
             tc.tile_pool(name="p4ps_s", bufs=1, space="PSUM") as p4ps_s, \
             tc.tile_pool(name="p4ps_g", bufs=2, space="PSUM") as p4ps_g, \
             tc.tile_pool(name="p4ps_d", bufs=2, space="PSUM") as p4ps_d:
            # tile 0's mtn (precomputed in Phase 2) is fetched before the
            # weights so the first gate matmul isn't stuck behind 12MB of
            # weight DMA on the queue
            mtn_cur = p4.tile([128, 16, NT], BF16, tag="mtn", name="mtn")
            nc.sync.dma_start(mtn_cur[:], scr['mtn0']
                              .rearrange("(kt p) n -> p kt n", p=128))
            wg_t = p4w.tile([128, 16, FFr], BF16, tag="wg_t", name="wg_t")
            nc.sync.dma_start(wg_t[:],
                              io['w_gate'].rearrange("(kt p) m -> p kt m", p=128))
            wu_t = p4w.tile([128, 16, FFr], BF16, tag="wu_t", name="wu_t")
            nc.sync.dma_start(wu_t[:],
                              io['w_up'].rearrange("(kt p) m -> p kt m", p=128))
            wd_t = p4w.tile([128, 8, H], BF16, tag="wd_t", name="wd_t")
            nc.sync.dma_start(wd_t[:],
                              io['w_down'].rearrange("(kt p) m -> p kt m", p=128))

            def p4_prep(j):
                tok0 = j * NT
                mt = p4mt.tile([128, 16, NT], BF16, tag="mt", name="mt")
                nc.sync.dma_start(mt[:], scr['ar1_out8'][j]
                                  .rearrange("(kt p) n -> p kt n", p=128))
                ssq_t = p4row.tile([1, NT], F32, tag="ssq_t", name="ssq_t")
                nc.sync.dma_start(ssq_t[:], scr['ssq_out8'][j])
                ssq_l = p4row.tile([1, NT], F32, tag="ssq_l", name="ssq_l")
                nc.scalar.activation(ssq_l[:], ssq_t[:], AF.Ln,
                                     bias=eps1[:], scale=float(1.0 / DIN))
                s3_row = p4row.tile([1, NT], F32R, tag="s3row", name="s3row")
                nc.scalar.activation(s3_row[:], ssq_l[:], AF.Exp, scale=-0.5)
                s3b_ps = p4ps_s.tile([128, NT], F32, tag="bps", name="s3bps")
                mm(s3b_ps[:], ones1[:], s3_row[:], start=True, stop=True)
                s3b = p4.tile([128, NT], BF16, tag="s3b", name="s3b")
                nc.any.tensor_copy(s3b[:], s3b_ps[:])
                # s3-scale + residual add + ln2 stats
                ssq_ps = p4ps_s.tile([1, NT], F32, tag="ssq", name="ssq")
                for k in range(16):
                    ht = p4.tile([128, NT], BF16, tag="ht", name="ht")
                    nc.sync.dma_start(
                        ht[:], io['hsT'][k * 128:(k + 1) * 128, tok0:tok0 + NT])
                    nc.vector.tensor_tensor(mt[:, k, :], mt[:, k, :], s3b[:],
                                            Op.mult)
                    nc.vector.tensor_tensor(mt[:, k, :], mt[:, k, :], ht[:],
                                            Op.add)
                    nc.sync.dma_start(
                        io['resid2T'][k * 128:(k + 1) * 128, tok0:tok0 + NT],
                        mt[:, k, :])
                    sq = p4.tile([128, NT], F32R, tag="sq", name="sq")
                    nc.scalar.activation(sq[:], mt[:, k, :], AF.Square)
                    mm(ssq_ps[:], ones128[:], sq[:],
                       start=(k == 0), stop=(k == 15))
                sr0 = p4row.tile([1, NT], F32, tag="sr0", name="sr0")
                nc.scalar.activation(sr0[:], ssq_ps[:], AF.Ln,
                                     bias=eps1[:], scale=float(1.0 / H))
                s_row = p4row.tile([1, NT], F32R, tag="srow", name="srow")
                nc.scalar.activation(s_row[:], sr0[:], AF.Exp, scale=-0.5)
                sb_ps = p4ps_s.tile([128, NT], F32, tag="bps", name="sbps")
                mm(sb_ps[:], ones1[:], s_row[:], start=True, stop=True)
                sb = p4.tile([128, NT], BF16, tag="sb", name="sb")
                nc.any.tensor_copy(sb[:], sb_ps[:])
                mtn = p4.tile([128, 16, NT], BF16, tag="mtn", name="mtn")
                for k in range(16):
                    nc.vector.scalar_tensor_tensor(
                        mtn[:, k, :], mt[:, k, :], C['ln2_c'][:, k:k + 1],
                        sb[:], Op.mult, Op.mult)
                return mtn

            for nt in range(n8):
                tok0 = nt * NT
                # gate_up + silu*up (av kept in SBUF as down-proj k-tiles)
                av = p4av.tile([128, 8, NT], BF16, tag="av", name="av")
                for mi in range(8):
                    gp = p4ps_g.tile([128, NT], F32, tag="gp", name="gp")
                    up = p4ps_g.tile([128, NT], F32, tag="up", name="up")
                    for k in range(16):
                        mm(gp[:], wg_t[:, k, mi * 128:(mi + 1) * 128],
                           mtn_cur[:, k, :], start=(k == 0), stop=(k == 15))
                    for k in range(16):
                        mm(up[:], wu_t[:, k, mi * 128:(mi + 1) * 128],
                           mtn_cur[:, k, :], start=(k == 0), stop=(k == 15))
                    sg = p4.tile([128, NT], BF16, tag="sg", name="sg")
                    silu(sg[:], gp[:], pool=p4, tag="sgsig")
                    nc.vector.tensor_tensor(av[:, mi, :], sg[:], up[:], Op.mult)
                # next tile's prep lands between the gate and down matmuls so
                # its stats/DVE chain hides under this tile's PE work
                mtn_next = p4_prep(nt + 1) if nt + 1 < n8 else None
                # down proj -> ReduceScatter chunk (host concats slices)
                for mo in range(16):
                    ps = p4ps_d.tile([128, NT], F32, tag="dps", name="dps")
                    for k in range(8):
                        mm(ps[:], wd_t[:, k, mo * 128:(mo + 1) * 128],
                           av[:, k, :], start=(k == 0), stop=(k == 7))
                    ot = p4.tile([128, NT], BF16, tag="ot4", name="ot4")
                    nc.any.tensor_copy(ot[:], ps[:])
                    nc.sync.dma_start(
                        scr['rs2_in8'][nt][mo * 128:(mo + 1) * 128, :], ot[:])
                if world > 1:
                    nc.gpsimd.collective_compute(
                        "ReduceScatter", Op.add,
                        replica_groups=[list(range(world))],
                        ins=[scr['rs2_in8'][nt]], outs=[scr['rs2_out8'][nt]])
                else:
                    nc.sync.dma_start(scr['rs2_out8'][nt],
                                      scr['rs2_in8'][nt][0:H // world, :])
                nc.sync.dma_start(io['out1T'][:, tok0:tok0 + NT],
                                  scr['rs2_out8'][nt])
                mtn_cur = mtn_next


# ================================================================ entry point
def kernel(**inputs):
    from concourse import bass_utils

    nc = build(world=TP, debug=False)
    in_maps = [shard_core_inputs(inputs, r) for r in range(TP)]
    res = bass_utils.run_bass_kernel_spmd(nc, in_maps, core_ids=list(range(TP)))
    out1T = np.concatenate(
        [np.asarray(res.results[r]['out1T'], dtype=np.float32)
         for r in range(TP)], axis=0)                # [H, T] feature-major
    out1 = np.ascontiguousarray(out1T.T).reshape(B, L, H)
    resid2 = np.ascontiguousarray(
        np.asarray(res.results[0]['resid2T'], dtype=np.float32).T
    ).reshape(B, L, H)
    return out1, resid2


if __name__ == '__main__':
    nc = build(world=1)
    print("built ok")
